# revision 2
# baseline (speedup 1.0000x reference)
"""NeuroSAT GNN message passing on 8 Trainium2 NeuronCores — v2.

Speedups over the v1 graph-data-parallel kernel:
  * All large matmuls run as fp32r (hw-rounded fp32, ~11 mantissa bits) at
    1 cycle/row instead of fp32's 4. Accuracy is restored with a hi/lo
    split: x = hi + lo with hi = round_f32r(x) (free: the producing op
    writes an f32r tile), lo = x - hi. A matmul A@B becomes
    Ah@Bh + Al@Bh + Ah@Bl (dropped lo*lo term is ~2^-24 relative).
    Aggregation matmuls need only 2 terms: the adjacency matrices are
    small integers, exact in f32r.
  * Gate matmuls pair two gates on the 128 output partitions (M=128
    instead of 64), halving streamed rows. The pair-packed PSUM is
    repacked to graph-packed tiles by the sigmoid activations themselves
    (single-input acts may cross partition offsets; 2-input DVE ops may
    not), so the LSTM pointwise stays full-height.
  * The per-literal degree bias (+ lit gate biases) is added once per
    gate-pair psum on DVE; clause gate biases ride the activation bias.
    The g-gate's tanh(x)=2*sigmoid(2x)-1 input doubling is pre-folded
    into the host-side weights/biases, keeping every activation a plain
    table sigmoid.

Layout: per core 2 graphs; feature-major state tiles [128, nodes] with
graph0 on partitions 0:64, graph1 on 64:128, kept in split (hi, lo)
f32r form. Row-major (transposed) hi/lo copies feed the aggregation
matmuls against constant f32r adjacency chunk tiles.
"""

import numpy as np

H = 64
ITERS = 24
B, NV, NC, K = 16, 400, 440, 12
NL = 2 * NV                  # literals/graph = 800
NPG = NL + NC                # nodes/graph = 1240
N = B * NPG                  # 19840
NCORES = 8
GPC = B // NCORES            # graphs per core = 2
CHK = 400                    # literal column chunk (aligned to NV flip halves)

_PROGRAM_CACHE = {}


def _build_program():
    from contextlib import ExitStack

    import concourse.bacc as bacc
    import concourse.mybir as mybir
    from concourse.masks import make_identity
    from concourse.tile import TileContext, add_dep_helper

    F32 = mybir.dt.float32
    F32R = mybir.dt.float32r
    SIG = mybir.ActivationFunctionType.Sigmoid
    MULT = mybir.AluOpType.mult
    SUB = mybir.AluOpType.subtract

    nc = bacc.Bacc(
        "TRN2", target_bir_lowering=False, debug=False, num_devices=NCORES
    )

    # ---- DRAM I/O (per-core shards; weights replicated) ----
    d_xt_lit = nc.dram_tensor("xt_lit", [3, GPC * NL], F32, kind="ExternalInput")
    d_xt_cl = nc.dram_tensor("xt_cl", [3, GPC * NC], F32, kind="ExternalInput")
    d_at = nc.dram_tensor("at_rm", [GPC, 7, 128, NC], F32, kind="ExternalInput")
    d_a = nc.dram_tensor("a_rm", [GPC, 4, 128, NL], F32, kind="ExternalInput")
    WNAMES = ("wc_a", "wc_b", "wc_1", "wl_a", "wl_b", "w_lh_dup", "w_cl2_dup")
    d_w = {nm: nc.dram_tensor(nm, [128, 256], F32, kind="ExternalInput")
           for nm in WNAMES}
    d_wv = nc.dram_tensor("wv_dup", [128, 1], F32, kind="ExternalInput")
    d_liw = nc.dram_tensor("li_w3", [3, H], F32, kind="ExternalInput")
    d_ciw = nc.dram_tensor("ci_w3", [3, H], F32, kind="ExternalInput")
    d_bias = nc.dram_tensor("bias_q", [128, 5], F32, kind="ExternalInput")
    d_dxr = nc.dram_tensor("dxr", [2, GPC * NL], F32, kind="ExternalInput")
    d_wdq = nc.dram_tensor("wdq", [2, 256], F32, kind="ExternalInput")
    d_out = nc.dram_tensor("vote", [1, GPC * NL], F32, kind="ExternalOutput")

    with TileContext(nc) as tc, ExitStack() as ctx:
        const = ctx.enter_context(tc.tile_pool(name="const", bufs=1))
        state = ctx.enter_context(tc.tile_pool(name="state", bufs=2))
        work = ctx.enter_context(tc.tile_pool(name="work", bufs=1))
        pstp = ctx.enter_context(tc.tile_pool(name="pstp", bufs=2, space="PSUM"))
        psag = ctx.enter_context(tc.tile_pool(name="psag", bufs=2, space="PSUM"))
        psg = ctx.enter_context(tc.tile_pool(name="psg", bufs=4, space="PSUM"))

        LO, HI = slice(0, 64), slice(64, 128)
        HALF = (LO, HI)

        # ---- constants ----
        ident = const.tile([128, 128], F32, name="ident")
        make_identity(nc, ident)
        identr = const.tile([128, 128], F32R, name="identr")
        nc.scalar.copy(identr[:, :], ident[:, :])

        # adjacency chunks -> f32r const tiles (integers: cvt exact)
        at_r = const.tile([128, GPC * 7 * NC], F32R, name="at_r")
        for g in range(GPC):
            for kk in range(7):
                stg = work.tile([128, NC], F32, tag="ld", bufs=1,
                                name=f"ld_at_{g}_{kk}")
                nc.sync.dma_start(out=stg[:, :], in_=d_at[g, kk])
                c0 = NC * (7 * g + kk)
                nc.scalar.copy(at_r[:, c0:c0 + NC], stg[:, :])
        a_r = const.tile([128, GPC * 4 * NL], F32R, name="a_r")
        for g in range(GPC):
            for kk in range(4):
                stg = work.tile([128, NL], F32, tag="ld", bufs=1,
                                name=f"ld_a_{g}_{kk}")
                nc.sync.dma_start(out=stg[:, :], in_=d_a[g, kk])
                c0 = NL * (4 * g + kk)
                nc.scalar.copy(a_r[:, c0:c0 + NL], stg[:, :])

        # gate weights -> (hi, lo) f32r pairs
        wsp = {}
        for nm in WNAMES:
            stg = work.tile([128, 256], F32, tag="ld", bufs=1, name=f"ldw_{nm}")
            nc.sync.dma_start(out=stg[:, :], in_=d_w[nm][:, :])
            wh = const.tile([128, 256], F32R, name=f"{nm}_h")
            wl = const.tile([128, 256], F32R, name=f"{nm}_l")
            nc.scalar.copy(wh[:, :], stg[:, :])
            nc.vector.tensor_tensor(wl[:, :], stg[:, :], wh[:, :], op=SUB)
            wsp[nm] = (wh, wl)
        stg = work.tile([128, 1], F32, tag="ld", bufs=1, name="ldw_wv")
        nc.sync.dma_start(out=stg[:, :], in_=d_wv[:, :])
        wv_h = const.tile([128, 1], F32R, name="wv_h")
        wv_l = const.tile([128, 1], F32R, name="wv_l")
        nc.scalar.copy(wv_h[:, :], stg[:, :])
        nc.vector.tensor_tensor(wv_l[:, :], stg[:, :], wv_h[:, :], op=SUB)

        def load(dram, shape, nm):
            t = const.tile(shape, F32, name=nm)
            nc.sync.dma_start(out=t[:, :], in_=dram[:, :])
            return t

        xt_lit = load(d_xt_lit, [3, GPC * NL], "xt_lit_sb")
        xt_cl = load(d_xt_cl, [3, GPC * NC], "xt_cl_sb")
        li_w = load(d_liw, [3, H], "li_w_sb")
        ci_w = load(d_ciw, [3, H], "ci_w_sb")
        bias = load(d_bias, [128, 5], "bias_sb")
        stg = work.tile([2, GPC * NL], F32, tag="ld2", bufs=1, name="ld_dxr")
        nc.sync.dma_start(out=stg[:, :], in_=d_dxr[:, :])
        dxr = const.tile([2, GPC * NL], F32R, name="dxr_sb")
        nc.scalar.copy(dxr[:, :], stg[:, :])
        stg = work.tile([2, 256], F32, tag="ld3", bufs=1, name="ld_wdq")
        nc.sync.dma_start(out=stg[:, :], in_=d_wdq[:, :])
        wdq_h = const.tile([2, 256], F32R, name="wdq_h")
        wdq_l = const.tile([2, 256], F32R, name="wdq_l")
        nc.scalar.copy(wdq_h[:, :], stg[:, :])
        nc.vector.tensor_tensor(wdq_l[:, :], stg[:, :], wdq_h[:, :], op=SUB)

        def MM(*a, **kw):
            kw.setdefault("skip_group_check", True)
            return nc.tensor.matmul(*a, **kw)

        # ---- initial node states (bias via ones row of xt) ----
        Lh = state.tile([128, NL], F32R, tag="Lh", name="Lh0")
        Ll = state.tile([128, NL], F32R, tag="Ll", name="Ll0")
        lit_hf = state.tile([128, NL], F32, tag="lit_hf", name="lit_hf0")
        for hf in range(2):
            p = psg.tile([128, CHK], F32, tag="g", name=f"ini_{hf}")
            prev = None
            for g in range(GPC):
                mm = MM(p[HALF[g], :], li_w[0:3, :],
                        xt_lit[0:3, g * NL + hf * CHK:g * NL + (hf + 1) * CHK],
                        start=True, stop=True, tile_position=(0, 64 * g))
                if prev is not None:
                    add_dep_helper(mm.ins, prev.ins, sync=True,
                                   reason="psum half order")
                prev = mm
            cs = slice(hf * CHK, (hf + 1) * CHK)
            nc.vector.tensor_copy(lit_hf[:, cs], p[:, :])
            nc.scalar.copy(Lh[:, cs], p[:, :])
            nc.vector.tensor_tensor(Ll[:, cs], p[:, :], Lh[:, cs], op=SUB)
        Chh = state.tile([128, NC], F32R, tag="Chh", name="Chh0")
        Chl = state.tile([128, NC], F32R, tag="Chl", name="Chl0")
        cl_hf = state.tile([128, NC], F32, tag="cl_hf", name="cl_hf0")
        pc = psg.tile([128, NC], F32, tag="g", name="ini_c")
        prev = None
        for g in range(GPC):
            mm = MM(pc[HALF[g], :], ci_w[0:3, :], xt_cl[0:3, g * NC:(g + 1) * NC],
                    start=True, stop=True, tile_position=(0, 64 * g))
            if prev is not None:
                add_dep_helper(mm.ins, prev.ins, sync=True, reason="psum half order")
            prev = mm
        nc.vector.tensor_copy(cl_hf[:, :], pc[:, :])
        nc.scalar.copy(Chh[:, :], pc[:, :])
        nc.vector.tensor_tensor(Chl[:, :], pc[:, :], Chh[:, :], op=SUB)

        lit_c = None
        cl_c = None

        for t in range(1, ITERS):
            first = t == 1

            # ==== clause phase ====
            # dependency-free copies first: clause stack state-halves and the
            # lit-phase flip halves (keeps Pool busy off the critical path)
            st0h = work.tile([128, NC], F32R, tag="st0h", name=f"st0h_{t}")
            st0l = work.tile([128, NC], F32R, tag="st0l", name=f"st0l_{t}")
            st1h = work.tile([128, NC], F32R, tag="st1h", name=f"st1h_{t}")
            st1l = work.tile([128, NC], F32R, tag="st1l", name=f"st1l_{t}")
            nc.scalar.copy(st0h[HI, :], Chh[LO, :])
            nc.scalar.copy(st0l[HI, :], Chl[LO, :])
            nc.gpsimd.tensor_copy(st1h[HI, :], Chh[HI, :])
            nc.gpsimd.tensor_copy(st1l[HI, :], Chl[HI, :])
            lst = []
            for hf in range(2):
                fs = slice((1 - hf) * CHK, (2 - hf) * CHK)
                s0h = work.tile([128, CHK], F32R, tag="s0h", bufs=2, name=f"s0h_{t}_{hf}")
                s0l = work.tile([128, CHK], F32R, tag="s0l", bufs=2, name=f"s0l_{t}_{hf}")
                s1h = work.tile([128, CHK], F32R, tag="s1h", bufs=2, name=f"s1h_{t}_{hf}")
                s1l = work.tile([128, CHK], F32R, tag="s1l", bufs=2, name=f"s1l_{t}_{hf}")
                nc.scalar.copy(s0h[HI, :], Lh[LO, fs])
                nc.scalar.copy(s0l[HI, :], Ll[LO, fs])
                nc.gpsimd.tensor_copy(s1h[HI, :], Lh[HI, fs])
                nc.gpsimd.tensor_copy(s1l[HI, :], Ll[HI, fs])
                lst.append((s0h, s0l, s1h, s1l))

            # transpose full lit state; hi/lo split rides the psum copy
            rml = []
            for g in range(GPC):
                tp = pstp.tile([128, 7 * H], F32, tag="tp", name=f"tpl_{t}_{g}")
                for kk in range(7):
                    c0 = 128 * kk if kk < 6 else NL - 128
                    nc.tensor.transpose(
                        tp[:, kk * H:(kk + 1) * H],
                        lit_hf[HALF[g], c0:c0 + 128],
                        ident[HALF[g], HALF[g]],
                    )
                rm_h = work.tile([128, 7 * H], F32R, tag=f"rmlh{g}",
                                 name=f"rmlh_{t}_{g}")
                rm_l = work.tile([128, 7 * H], F32R, tag=f"rmll{g}",
                                 name=f"rmll_{t}_{g}")
                nc.scalar.copy(rm_h[:, :], tp[:, :])
                nc.vector.tensor_tensor(rm_l[:, :], tp[:, :], rm_h[:, :], op=SUB)
                rml.append((rm_h, rm_l))

            # clause agg A^T @ L: per-graph psum tiles (g0 rows HI, g1 LO) so
            # the hi/lo term groups interleave without bank conflicts
            agc0 = psag.tile([128, NC], F32, tag="ag", name=f"agc0_{t}")
            agc1 = psag.tile([128, NC], F32, tag="ag", name=f"agc1_{t}")
            agp = (agc0[LO, :], agc1[LO, :])
            for term in range(2):
                for g in range(GPC):
                    for kk in range(7):
                        MM(agp[g], rml[g][term][:, kk * H:(kk + 1) * H],
                           at_r[:, NC * (7 * g + kk):NC * (7 * g + kk + 1)],
                           start=(term == 0 and kk == 0),
                           stop=(term == 1 and kk == 6),
                           tile_position=(0, 0))

            # stack agg halves (aligned at LO)
            nc.scalar.copy(st0h[LO, :], agc0[LO, :])
            nc.vector.tensor_tensor(st0l[LO, :], agc0[LO, :], st0h[LO, :], op=SUB)
            nc.scalar.copy(st1h[LO, :], agc1[LO, :])
            nc.vector.tensor_tensor(st1l[LO, :], agc1[LO, :], st1h[LO, :], op=SUB)

            # clause gates: per graph, 2 gate-pairs, 3-term split
            cg = [[None, None], [None, None]]
            for g in range(GPC):
                wnm = "wc_1" if first else "wc_b"
                wh, wl = wsp[wnm]
                sth, stl = (st0h, st0l) if g == 0 else (st1h, st1l)
                for p in range(2):
                    ps_ = slice(p * 128, (p + 1) * 128)
                    gp = psg.tile([128, NC], F32, tag="g", name=f"cg{g}{p}_{t}")
                    MM(gp[:, :], wh[:, ps_], sth[:, :], start=True, stop=False)
                    MM(gp[:, :], wl[:, ps_], sth[:, :], start=False, stop=False)
                    MM(gp[:, :], wh[:, ps_], stl[:, :], start=False, stop=True)
                    cg[g][p] = gp

            # repack sigmoids: pair-psum -> graph-packed s tiles
            s_i = work.tile([128, NC], F32, tag="si", name=f"csi_{t}")
            s_f = work.tile([128, NC], F32, tag="sf", name=f"csf_{t}")
            s_g = work.tile([128, NC], F32, tag="sg", name=f"csg_{t}")
            s_o = work.tile([128, NC], F32, tag="so", name=f"cso_{t}")
            for g in range(GPC):
                h = HALF[g]
                nc.scalar.activation(s_i[h, :], cg[g][0][LO, :], SIG,
                                     bias=bias[h, 0:1])
                nc.scalar.activation(s_g[h, :], cg[g][1][LO, :], SIG,
                                     bias=bias[h, 2:3])
            for g in range(GPC):
                h = HALF[g]
                nc.scalar.activation(s_f[h, :], cg[g][0][HI, :], SIG,
                                     bias=bias[h, 1:2])
                nc.scalar.activation(s_o[h, :], cg[g][1][HI, :], SIG,
                                     bias=bias[h, 3:4])

            # clause LSTM pointwise (graph-packed, full height)
            cc_new = state.tile([128, NC], F32, tag="cl_c", name=f"cc_{t}")
            t1 = work.tile([128, NC], F32, tag="t1", name=f"ct1_{t}")
            nc.vector.tensor_mul(t1[:, :], s_i[:, :], s_g[:, :])
            if first:
                nc.vector.scalar_tensor_tensor(
                    cc_new[:, :], t1[:, :], 2.0, s_i[:, :], op0=MULT, op1=SUB)
            else:
                u = work.tile([128, NC], F32, tag="u", name=f"cu_{t}")
                nc.vector.scalar_tensor_tensor(
                    u[:, :], t1[:, :], 2.0, s_i[:, :], op0=MULT, op1=SUB)
                t2 = work.tile([128, NC], F32, tag="t2", name=f"ct2_{t}")
                nc.vector.tensor_mul(t2[:, :], s_f[:, :], cl_c[:, :])
                nc.vector.tensor_add(cc_new[:, :], u[:, :], t2[:, :])
            tnc = work.tile([128, NC], F32, tag="tnc", name=f"ctn_{t}")
            nc.scalar.activation(tnc[:, :], cc_new[:, :], SIG, scale=2.0)
            t3 = work.tile([128, NC], F32, tag="t3", name=f"ct3_{t}")
            nc.vector.tensor_mul(t3[:, :], s_o[:, :], tnc[:, :])
            cl_hf_new = state.tile([128, NC], F32, tag="cl_hf", name=f"chf_{t}")
            nc.vector.scalar_tensor_tensor(
                cl_hf_new[:, :], t3[:, :], 2.0, s_o[:, :], op0=MULT, op1=SUB)
            # split off the critical path (consumers are next iteration)
            Chh_new = state.tile([128, NC], F32R, tag="Chh", name=f"Chh_{t}")
            Chl_new = state.tile([128, NC], F32R, tag="Chl", name=f"Chl_{t}")
            nc.gpsimd.tensor_copy(Chh_new[:, :], cl_hf_new[:, :])
            nc.vector.tensor_tensor(Chl_new[:, :], cl_hf_new[:, :], Chh_new[:, :],
                                    op=SUB)

            # ==== lit phase ====
            # transpose full clause state; split rides the psum copy
            rmc = []
            for g in range(GPC):
                tp = pstp.tile([128, 4 * H], F32, tag="tp", name=f"tpc_{t}_{g}")
                for kk in range(4):
                    c0 = 128 * kk if kk < 3 else NC - 128
                    nc.tensor.transpose(
                        tp[:, kk * H:(kk + 1) * H],
                        cl_hf_new[HALF[g], c0:c0 + 128],
                        ident[HALF[g], HALF[g]],
                    )
                rm_h = work.tile([128, 4 * H], F32R, tag=f"rmch{g}",
                                 name=f"rmch_{t}_{g}")
                rm_l = work.tile([128, 4 * H], F32R, tag=f"rmcl{g}",
                                 name=f"rmcl_{t}_{g}")
                nc.scalar.copy(rm_h[:, :], tp[:, :])
                nc.vector.tensor_tensor(rm_l[:, :], tp[:, :], rm_h[:, :], op=SUB)
                rmc.append((rm_h, rm_l))

            Lh_new = state.tile([128, NL], F32R, tag="Lh", name=f"Lh_{t}")
            Ll_new = state.tile([128, NL], F32R, tag="Ll", name=f"Ll_{t}")
            lit_hf_new = state.tile([128, NL], F32, tag="lit_hf", name=f"lhf_{t}")
            lc_new = state.tile([128, NL], F32, tag="lit_c", name=f"lc_{t}")
            wSnm = "w_cl2_dup" if first else "w_lh_dup"
            wSh, wSl = wsp[wSnm]
            for hf in range(2):
                cs = slice(hf * CHK, (hf + 1) * CHK)
                s0h, s0l, s1h, s1l = lst[hf]
                # literal agg A @ C: per-graph psums, interleaved term groups
                agl0 = psag.tile([128, CHK], F32, tag="ag", name=f"agl0_{t}_{hf}")
                agl1 = psag.tile([128, CHK], F32, tag="ag", name=f"agl1_{t}_{hf}")
                agp = (agl0[LO, :], agl1[LO, :])
                for term in range(2):
                    for g in range(GPC):
                        for kk in range(4):
                            MM(agp[g], rmc[g][term][:, kk * H:(kk + 1) * H],
                               a_r[:, NL * (4 * g + kk) + hf * CHK:
                                   NL * (4 * g + kk) + (hf + 1) * CHK],
                               start=(term == 0 and kk == 0),
                               stop=(term == 1 and kk == 3),
                               tile_position=(0, 0))

                # lit gates: dq + rec terms first (no stack dependency -> they
                # fill the PE pipeline while stacks build)
                gps = [[None, None], [None, None]]
                for g in range(GPC):
                    hg = HALF[g]
                    for p in range(2):
                        ps_ = slice(p * 128, (p + 1) * 128)
                        gp = psg.tile([128, CHK], F32, tag="g", name=f"lg{g}{p}_{t}_{hf}")
                        dc = slice(g * NL + hf * CHK, g * NL + (hf + 1) * CHK)
                        MM(gp[:, :], wdq_h[0:2, ps_], dxr[0:2, dc],
                           start=True, stop=False)
                        MM(gp[:, :], wdq_l[0:2, ps_], dxr[0:2, dc],
                           start=False, stop=False)
                        MM(gp[:, :], wSh[hg, ps_], Lh[hg, cs], start=False,
                           stop=False, tile_position=(64 * g, 0))
                        MM(gp[:, :], wSl[hg, ps_], Lh[hg, cs], start=False,
                           stop=False, tile_position=(64 * g, 0))
                        MM(gp[:, :], wSh[hg, ps_], Ll[hg, cs], start=False,
                           stop=False, tile_position=(64 * g, 0))
                        gps[g][p] = gp

                # stack agg halves (aligned at LO)
                nc.scalar.copy(s0h[LO, :], agl0[LO, :])
                nc.vector.tensor_tensor(s0l[LO, :], agl0[LO, :], s0h[LO, :], op=SUB)
                nc.scalar.copy(s1h[LO, :], agl1[LO, :])
                nc.vector.tensor_tensor(s1l[LO, :], agl1[LO, :], s1h[LO, :], op=SUB)

                # stack-dependent gate terms
                for g in range(GPC):
                    wnm = "wl_b"
                    wh, wl = wsp[wnm]
                    sth, stl = (s0h, s0l) if g == 0 else (s1h, s1l)
                    for p in range(2):
                        ps_ = slice(p * 128, (p + 1) * 128)
                        gp = gps[g][p]
                        MM(gp[:, :], wh[:, ps_], sth[:, :], start=False, stop=False)
                        MM(gp[:, :], wl[:, ps_], sth[:, :], start=False, stop=False)
                        MM(gp[:, :], wh[:, ps_], stl[:, :], start=False, stop=True)

                s_i = work.tile([128, CHK], F32, tag="lsi", bufs=2, name=f"lsi_{t}_{hf}")
                s_f = work.tile([128, CHK], F32, tag="lsf", bufs=2, name=f"lsf_{t}_{hf}")
                s_g = work.tile([128, CHK], F32, tag="lsg", bufs=2, name=f"lsg_{t}_{hf}")
                s_o = work.tile([128, CHK], F32, tag="lso", bufs=2, name=f"lso_{t}_{hf}")
                for g in range(GPC):
                    h = HALF[g]
                    nc.scalar.activation(s_i[h, :], gps[g][0][LO, :], SIG)
                    nc.scalar.activation(s_g[h, :], gps[g][1][LO, :], SIG)
                for g in range(GPC):
                    h = HALF[g]
                    nc.scalar.activation(s_f[h, :], gps[g][0][HI, :], SIG)
                    nc.scalar.activation(s_o[h, :], gps[g][1][HI, :], SIG)

                t1 = work.tile([128, CHK], F32, tag="lt1", bufs=2, name=f"lt1_{t}_{hf}")
                nc.vector.tensor_mul(t1[:, :], s_i[:, :], s_g[:, :])
                if first:
                    nc.vector.scalar_tensor_tensor(
                        lc_new[:, cs], t1[:, :], 2.0, s_i[:, :], op0=MULT, op1=SUB)
                else:
                    u = work.tile([128, CHK], F32, tag="lu", bufs=2, name=f"lu_{t}_{hf}")
                    nc.vector.scalar_tensor_tensor(
                        u[:, :], t1[:, :], 2.0, s_i[:, :], op0=MULT, op1=SUB)
                    t2 = work.tile([128, CHK], F32, tag="lt2", bufs=2, name=f"lt2_{t}_{hf}")
                    nc.vector.tensor_mul(t2[:, :], s_f[:, :], lit_c[:, cs])
                    nc.vector.tensor_add(lc_new[:, cs], u[:, :], t2[:, :])
                tnc = work.tile([128, CHK], F32, tag="ltn", bufs=1, name=f"ltn_{t}_{hf}")
                nc.scalar.activation(tnc[:, :], lc_new[:, cs], SIG, scale=2.0)
                t3 = work.tile([128, CHK], F32, tag="lt3", bufs=2, name=f"lt3_{t}_{hf}")
                nc.vector.tensor_mul(t3[:, :], s_o[:, :], tnc[:, :])
                nc.vector.scalar_tensor_tensor(
                    lit_hf_new[:, cs], t3[:, :], 2.0, s_o[:, :], op0=MULT, op1=SUB)
                # split off the critical path (consumers are next iteration)
                nc.gpsimd.tensor_copy(Lh_new[:, cs], lit_hf_new[:, cs])
                nc.vector.tensor_tensor(Ll_new[:, cs], lit_hf_new[:, cs],
                                        Lh_new[:, cs], op=SUB)

            # PE ramp warmers: dependency-free matmuls that run inside the
            # end-of-iteration stall (PE waiting on the half-1 pointwise
            # chain) so the tensor engine re-enters the next iteration at
            # full p-state instead of paying the 2x mid-ramp on the aggs.
            if t < ITERS - 1:
                wp = pstp.tile([128, 256], F32, tag="tp", name=f"warm_{t}")
                for wi in range(40):
                    MM(wp[:, :], identr[0:1, :], dxr[0:1, 0:256],
                       start=True, stop=True)

            Lh, Ll, lit_hf = Lh_new, Ll_new, lit_hf_new
            Chh, Chl, cl_hf = Chh_new, Chl_new, cl_hf_new
            lit_c, cl_c = lc_new, cc_new

        # ---- vote head: (Lh+Ll) @ Wv, 3-term split ----
        for g in range(GPC):
            hg = HALF[g]
            for hf in range(2):
                cs = slice(hf * CHK, (hf + 1) * CHK)
                p = psg.tile([1, CHK], F32, tag="g", name=f"vps_{g}_{hf}")
                MM(p[:, :], wv_h[hg, 0:1], Lh[hg, cs], start=True, stop=False,
                   tile_position=(64 * g, 0))
                MM(p[:, :], wv_l[hg, 0:1], Lh[hg, cs], start=False, stop=False,
                   tile_position=(64 * g, 0))
                MM(p[:, :], wv_h[hg, 0:1], Ll[hg, cs], start=False, stop=True,
                   tile_position=(64 * g, 0))
                vc = work.tile([1, CHK], F32, tag="vote", bufs=1,
                               name=f"vote_{g}_{hf}")
                nc.scalar.activation(
                    vc[:, :], p[:, :], mybir.ActivationFunctionType.Identity,
                    bias=bias[0:1, 4:5],
                )
                nc.sync.dma_start(
                    out=d_out[0:1, g * NL + hf * CHK:g * NL + (hf + 1) * CHK],
                    in_=vc[:, :])

    nc.compile()
    return nc


def _fold_and_shard(inputs):
    """Host-side preprocessing: fold weights, build adjacency, shard by graph."""
    f32 = np.float32
    g = {k: np.asarray(v) for k, v in inputs.items()}

    def collapse(w1, b1, w2, b2, w3, b3):
        return w1 @ w2 @ w3, ((b1 @ w2) + b2) @ w3 + b3

    Wl, bl = collapse(g["lm1_w"], g["lm1_b"], g["lm2_w"], g["lm2_b"],
                      g["lm3_w"], g["lm3_b"])
    Wc, bc = collapse(g["cm1_w"], g["cm1_b"], g["cm2_w"], g["cm2_b"],
                      g["cm3_w"], g["cm3_b"])
    Wv, bv = collapse(g["lv1_w"], g["lv1_b"], g["lv2_w"], g["lv2_b"],
                      g["lv3_w"], g["lv3_b"])

    cu_wih, lu_wih = g["cu_wih"], g["lu_wih"]
    w_lc = (Wl @ cu_wih).astype(f32)                 # agg_c -> clause gates
    w_ch = (w_lc + g["cu_whh"]).astype(f32)          # t>=2 merged recurrent
    cbias_c = ((K + 1) * (bl @ cu_wih) + g["cu_bih"] + g["cu_bhh"]).astype(f32)
    wih_a = lu_wih[0:H].astype(f32)                  # flip -> lit gates
    w_cl2 = (Wc @ lu_wih[H:2 * H]).astype(f32)       # agg_l -> lit gates
    w_lh = (w_cl2 + g["lu_whh"]).astype(f32)         # t>=2 merged recurrent
    q_l = (bc @ lu_wih[H:2 * H]).astype(f32)         # [256]
    cbias_l = (g["lu_bih"] + g["lu_bhh"]).astype(f32)

    def gdouble(w):
        w = w.copy()
        w[:, 2 * H:3 * H] *= 2.0     # g-gate runs as sigmoid(2x+2b)
        return w

    vs = np.vstack
    wc_a = gdouble(vs([w_ch, w_lc]))
    wc_b = gdouble(vs([w_lc, w_ch]))
    wc_1 = gdouble(vs([w_lc, w_lc]))
    wl_a = gdouble(vs([wih_a, w_cl2]))
    wl_b = gdouble(vs([w_cl2, wih_a]))
    w_lh_dup = gdouble(vs([w_lh, w_lh]))
    w_cl2_dup = gdouble(vs([w_cl2, w_cl2]))
    wv_dup = vs([Wv.astype(f32), Wv.astype(f32)])

    bias_q = np.zeros((128, 5), f32)
    for x in range(4):
        scl = 2.0 if x == 2 else 1.0
        bias_q[0:64, x] = scl * cbias_c[x * H:(x + 1) * H]
        bias_q[64:128, x] = scl * cbias_c[x * H:(x + 1) * H]
    bias_q[0, 4] = bv[0]

    li_w3 = np.concatenate([g["li_w"], g["li_b"][None, :]], axis=0).astype(f32)
    ci_w3 = np.concatenate([g["ci_w"], g["ci_b"][None, :]], axis=0).astype(f32)

    # adjacency per graph from edge_index (direction-robust)
    ei = g["edge_index"].astype(np.int64)
    src, dst = ei[0], ei[1]
    src_g, dst_g = src // NPG, dst // NPG
    assert np.all(src_g == dst_g), "edges must be graph-local"
    src_l, dst_l = src % NPG, dst % NPG
    s_lit, d_lit = src_l < NL, dst_l < NL
    A_in_c = np.zeros((B, NC, NL), f32)   # clause <- literal edges
    m = (~d_lit) & s_lit
    np.add.at(A_in_c, (dst_g[m], dst_l[m] - NL, src_l[m]), 1.0)
    A_in_l = np.zeros((B, NL, NC), f32)   # literal <- clause edges
    m = d_lit & (~s_lit)
    np.add.at(A_in_l, (dst_g[m], dst_l[m], src_l[m] - NL), 1.0)
    deg_l = A_in_l.sum(axis=2)            # [B, NL]

    x = g["x"].astype(f32).reshape(B, NPG, 2)
    ones = np.ones((B, NPG, 1), f32)
    x3 = np.concatenate([x, ones], axis=2)        # [B, NPG, 3]

    shared = dict(
        wc_a=wc_a, wc_b=wc_b, wc_1=wc_1, wl_a=wl_a, wl_b=wl_b,
        w_lh_dup=w_lh_dup, w_cl2_dup=w_cl2_dup, wv_dup=wv_dup,
        li_w3=li_w3, ci_w3=ci_w3, bias_q=bias_q,
    )
    in_maps = []
    for c in range(NCORES):
        gs = slice(c * GPC, (c + 1) * GPC)
        x3c = x3[gs]                               # [GPC, NPG, 3]
        xt_lit = np.ascontiguousarray(
            x3c[:, :NL].transpose(2, 0, 1).reshape(3, GPC * NL))
        xt_cl = np.ascontiguousarray(
            x3c[:, NL:].transpose(2, 0, 1).reshape(3, GPC * NC))
        # dxr rows: (deg+1) per literal, ones; wdq rows: q, cbias_l
        # (g-gate block doubled to match the pre-doubled weights)
        dxr = np.ones((2, GPC * NL), f32)
        for gg in range(GPC):
            dxr[0, gg * NL:(gg + 1) * NL] = deg_l[c * GPC + gg] + 1.0
        wdq = np.stack([q_l, cbias_l]).astype(f32)
        wdq[:, 2 * H:3 * H] *= 2.0
        # pre-chunk adjacency into full-128-row K-chunks; the final chunk
        # overlaps the previous one with its overlap rows zeroed
        atc = np.zeros((GPC, 7, 128, NC), f32)
        ac = np.zeros((GPC, 4, 128, NL), f32)
        for gg in range(GPC):
            at_full = A_in_c[c * GPC + gg].T       # [NL, NC]
            a_full = A_in_l[c * GPC + gg].T        # [NC, NL]
            for kk in range(6):
                atc[gg, kk] = at_full[128 * kk:128 * (kk + 1)]
            atc[gg, 6, 128 - (NL - 768):] = at_full[768:]
            for kk in range(3):
                ac[gg, kk] = a_full[128 * kk:128 * (kk + 1)]
            ac[gg, 3, 128 - (NC - 384):] = a_full[384:]
        in_maps.append(dict(
            xt_lit=xt_lit, xt_cl=xt_cl, at_rm=atc, a_rm=ac,
            dxr=dxr, wdq=wdq, **shared,
        ))
    return in_maps


_LAST_RESULTS = {}


def kernel(**inputs):
    from concourse.bass_utils import run_bass_kernel_spmd

    in_maps = _fold_and_shard(inputs)
    if "nc" not in _PROGRAM_CACHE:
        _PROGRAM_CACHE["nc"] = _build_program()
    nc = _PROGRAM_CACHE["nc"]
    res = run_bass_kernel_spmd(nc, in_maps, core_ids=list(range(NCORES)))
    _LAST_RESULTS["res"] = res
    out = np.zeros((N, 1), np.float32)
    for c in range(NCORES):
        vote = res.results[c]["vote"].reshape(GPC, NL)
        for g in range(GPC):
            base = (c * GPC + g) * NPG
            out[base:base + NL, 0] = vote[g]
    return out


# revision 4
# speedup vs baseline: 1.0342x; 1.0342x over previous
"""NeuroSAT GNN message passing on 8 Trainium2 NeuronCores — v2.

Speedups over the v1 graph-data-parallel kernel:
  * All large matmuls run as fp32r (hw-rounded fp32, ~11 mantissa bits) at
    1 cycle/row instead of fp32's 4. Accuracy is restored with a hi/lo
    split: x = hi + lo with hi = round_f32r(x) (free: the producing op
    writes an f32r tile), lo = x - hi. A matmul A@B becomes
    Ah@Bh + Al@Bh + Ah@Bl (dropped lo*lo term is ~2^-24 relative).
    Aggregation matmuls need only 2 terms: the adjacency matrices are
    small integers, exact in f32r.
  * Gate matmuls pair two gates on the 128 output partitions (M=128
    instead of 64), halving streamed rows. The pair-packed PSUM is
    repacked to graph-packed tiles by the sigmoid activations themselves
    (single-input acts may cross partition offsets; 2-input DVE ops may
    not), so the LSTM pointwise stays full-height.
  * The per-literal degree bias (+ lit gate biases) is added once per
    gate-pair psum on DVE; clause gate biases ride the activation bias.
    The g-gate's tanh(x)=2*sigmoid(2x)-1 input doubling is pre-folded
    into the host-side weights/biases, keeping every activation a plain
    table sigmoid.

Layout: per core 2 graphs; feature-major state tiles [128, nodes] with
graph0 on partitions 0:64, graph1 on 64:128, kept in split (hi, lo)
f32r form. Row-major (transposed) hi/lo copies feed the aggregation
matmuls against constant f32r adjacency chunk tiles.
"""

import numpy as np

H = 64
ITERS = 24
B, NV, NC, K = 16, 400, 440, 12
NL = 2 * NV                  # literals/graph = 800
NPG = NL + NC                # nodes/graph = 1240
N = B * NPG                  # 19840
NCORES = 8
GPC = B // NCORES            # graphs per core = 2
CHK = 400                    # literal column chunk (aligned to NV flip halves)

_PROGRAM_CACHE = {}


def _build_program():
    from contextlib import ExitStack

    import concourse.bacc as bacc
    import concourse.mybir as mybir
    from concourse.masks import make_identity
    from concourse.tile import TileContext, add_dep_helper

    F32 = mybir.dt.float32
    F32R = mybir.dt.float32r
    SIG = mybir.ActivationFunctionType.Sigmoid
    MULT = mybir.AluOpType.mult
    SUB = mybir.AluOpType.subtract

    nc = bacc.Bacc(
        "TRN2", target_bir_lowering=False, debug=False, num_devices=NCORES
    )

    # ---- DRAM I/O (per-core shards; weights replicated) ----
    d_xt_lit = nc.dram_tensor("xt_lit", [3, GPC * NL], F32, kind="ExternalInput")
    d_xt_cl = nc.dram_tensor("xt_cl", [3, GPC * NC], F32, kind="ExternalInput")
    d_at = nc.dram_tensor("at_rm", [GPC, 8, 128, NC], F32, kind="ExternalInput")
    d_a = nc.dram_tensor("a_rm", [GPC, 4, 128, NL], F32, kind="ExternalInput")
    WNAMES = ("wc_a", "wc_b", "wc_1", "wl_a", "wl_b", "w_lh_dup", "w_cl2_dup")
    d_w = {nm: nc.dram_tensor(nm, [128, 256], F32, kind="ExternalInput")
           for nm in WNAMES}
    d_wv = nc.dram_tensor("wv_dup", [128, 1], F32, kind="ExternalInput")
    d_liw = nc.dram_tensor("li_w3", [3, H], F32, kind="ExternalInput")
    d_ciw = nc.dram_tensor("ci_w3", [3, H], F32, kind="ExternalInput")
    d_bias = nc.dram_tensor("bias_q", [128, 5], F32, kind="ExternalInput")
    d_dxr = nc.dram_tensor("dxr", [2, GPC * NL], F32, kind="ExternalInput")
    d_wdq = nc.dram_tensor("wdq", [2, 256], F32, kind="ExternalInput")
    d_out = nc.dram_tensor("vote", [1, GPC * NL], F32, kind="ExternalOutput")

    with TileContext(nc) as tc, ExitStack() as ctx:
        const = ctx.enter_context(tc.tile_pool(name="const", bufs=1))
        state = ctx.enter_context(tc.tile_pool(name="state", bufs=2))
        work = ctx.enter_context(tc.tile_pool(name="work", bufs=1))
        pstp = ctx.enter_context(tc.tile_pool(name="pstp", bufs=2, space="PSUM"))
        psag = ctx.enter_context(tc.tile_pool(name="psag", bufs=2, space="PSUM"))
        psg = ctx.enter_context(tc.tile_pool(name="psg", bufs=4, space="PSUM"))

        LO, HI = slice(0, 64), slice(64, 128)
        HALF = (LO, HI)

        # ---- constants ----
        ident = const.tile([128, 128], F32, name="ident")
        make_identity(nc, ident)
        identr = const.tile([128, 128], F32R, name="identr")
        nc.scalar.copy(identr[:, :], ident[:, :])

        # adjacency chunks -> f32r const tiles (integers: cvt exact)
        at_r = const.tile([128, GPC * 8 * NC], F32R, name="at_r")
        for g in range(GPC):
            for kk in range(8):
                stg = work.tile([128, NC], F32, tag="ld", bufs=1,
                                name=f"ld_at_{g}_{kk}")
                nc.sync.dma_start(out=stg[:, :], in_=d_at[g, kk])
                c0 = NC * (8 * g + kk)
                nc.scalar.copy(at_r[:, c0:c0 + NC], stg[:, :])
        a_r = const.tile([128, GPC * 4 * NL], F32R, name="a_r")
        for g in range(GPC):
            for kk in range(4):
                stg = work.tile([128, NL], F32, tag="ld", bufs=1,
                                name=f"ld_a_{g}_{kk}")
                nc.sync.dma_start(out=stg[:, :], in_=d_a[g, kk])
                c0 = NL * (4 * g + kk)
                nc.scalar.copy(a_r[:, c0:c0 + NL], stg[:, :])

        # gate weights -> (hi, lo) f32r pairs
        wsp = {}
        for nm in WNAMES:
            stg = work.tile([128, 256], F32, tag="ld", bufs=1, name=f"ldw_{nm}")
            nc.sync.dma_start(out=stg[:, :], in_=d_w[nm][:, :])
            wh = const.tile([128, 256], F32R, name=f"{nm}_h")
            wl = const.tile([128, 256], F32R, name=f"{nm}_l")
            nc.scalar.copy(wh[:, :], stg[:, :])
            nc.vector.tensor_tensor(wl[:, :], stg[:, :], wh[:, :], op=SUB)
            wsp[nm] = (wh, wl)
        stg = work.tile([128, 1], F32, tag="ld", bufs=1, name="ldw_wv")
        nc.sync.dma_start(out=stg[:, :], in_=d_wv[:, :])
        wv_h = const.tile([128, 1], F32R, name="wv_h")
        wv_l = const.tile([128, 1], F32R, name="wv_l")
        nc.scalar.copy(wv_h[:, :], stg[:, :])
        nc.vector.tensor_tensor(wv_l[:, :], stg[:, :], wv_h[:, :], op=SUB)

        def load(dram, shape, nm):
            t = const.tile(shape, F32, name=nm)
            nc.sync.dma_start(out=t[:, :], in_=dram[:, :])
            return t

        xt_lit = load(d_xt_lit, [3, GPC * NL], "xt_lit_sb")
        xt_cl = load(d_xt_cl, [3, GPC * NC], "xt_cl_sb")
        li_w = load(d_liw, [3, H], "li_w_sb")
        ci_w = load(d_ciw, [3, H], "ci_w_sb")
        bias = load(d_bias, [128, 5], "bias_sb")
        stg = work.tile([2, GPC * NL], F32, tag="ld2", bufs=1, name="ld_dxr")
        nc.sync.dma_start(out=stg[:, :], in_=d_dxr[:, :])
        dxr = const.tile([2, GPC * NL], F32R, name="dxr_sb")
        nc.scalar.copy(dxr[:, :], stg[:, :])
        stg = work.tile([2, 256], F32, tag="ld3", bufs=1, name="ld_wdq")
        nc.sync.dma_start(out=stg[:, :], in_=d_wdq[:, :])
        wdq_h = const.tile([2, 256], F32R, name="wdq_h")
        wdq_l = const.tile([2, 256], F32R, name="wdq_l")
        nc.scalar.copy(wdq_h[:, :], stg[:, :])
        nc.vector.tensor_tensor(wdq_l[:, :], stg[:, :], wdq_h[:, :], op=SUB)

        def MM(*a, **kw):
            kw.setdefault("skip_group_check", True)
            return nc.tensor.matmul(*a, **kw)

        # ---- initial node states (bias via ones row of xt) ----
        Lh = state.tile([128, NL], F32R, tag="Lh", name="Lh0")
        Ll = state.tile([128, NL], F32R, tag="Ll", name="Ll0")
        lit_hf_a = state.tile([128, CHK], F32, tag="lit_hf_a", name="lit_hf_a0")
        lit_hf_b = state.tile([128, CHK], F32, tag="lit_hf_b", name="lit_hf_b0")
        for hf in range(2):
            p = psg.tile([128, CHK], F32, tag="g", name=f"ini_{hf}")
            prev = None
            for g in range(GPC):
                mm = MM(p[HALF[g], :], li_w[0:3, :],
                        xt_lit[0:3, g * NL + hf * CHK:g * NL + (hf + 1) * CHK],
                        start=True, stop=True, tile_position=(0, 64 * g))
                if prev is not None:
                    add_dep_helper(mm.ins, prev.ins, sync=True,
                                   reason="psum half order")
                prev = mm
            cs = slice(hf * CHK, (hf + 1) * CHK)
            nc.vector.tensor_copy((lit_hf_a if hf == 0 else lit_hf_b)[:, :],
                                  p[:, :])
            nc.scalar.copy(Lh[:, cs], p[:, :])
            nc.vector.tensor_tensor(Ll[:, cs], p[:, :], Lh[:, cs], op=SUB)
        Chh = state.tile([128, NC], F32R, tag="Chh", name="Chh0")
        Chl = state.tile([128, NC], F32R, tag="Chl", name="Chl0")
        cl_hf = state.tile([128, NC], F32, tag="cl_hf", name="cl_hf0")
        pc = psg.tile([128, NC], F32, tag="g", name="ini_c")
        prev = None
        for g in range(GPC):
            mm = MM(pc[HALF[g], :], ci_w[0:3, :], xt_cl[0:3, g * NC:(g + 1) * NC],
                    start=True, stop=True, tile_position=(0, 64 * g))
            if prev is not None:
                add_dep_helper(mm.ins, prev.ins, sync=True, reason="psum half order")
            prev = mm
        nc.vector.tensor_copy(cl_hf[:, :], pc[:, :])
        nc.scalar.copy(Chh[:, :], pc[:, :])
        nc.vector.tensor_tensor(Chl[:, :], pc[:, :], Chh[:, :], op=SUB)

        lit_c = None
        cl_c = None

        for t in range(1, ITERS):
            first = t == 1

            # ==== clause phase ====
            # dependency-free copies first: clause stack state-halves and the
            # lit-phase flip halves (keeps Pool busy off the critical path)
            st0h = work.tile([128, NC], F32R, tag="st0h", name=f"st0h_{t}")
            st0l = work.tile([128, NC], F32R, tag="st0l", name=f"st0l_{t}")
            st1h = work.tile([128, NC], F32R, tag="st1h", name=f"st1h_{t}")
            st1l = work.tile([128, NC], F32R, tag="st1l", name=f"st1l_{t}")
            nc.scalar.copy(st0h[HI, :], Chh[LO, :])
            nc.scalar.copy(st0l[HI, :], Chl[LO, :])
            nc.gpsimd.tensor_copy(st1h[HI, :], Chh[HI, :])
            nc.gpsimd.tensor_copy(st1l[HI, :], Chl[HI, :])
            lst = []
            for hf in range(2):
                fs = slice((1 - hf) * CHK, (2 - hf) * CHK)
                s0h = work.tile([128, CHK], F32R, tag="s0h", bufs=2, name=f"s0h_{t}_{hf}")
                s0l = work.tile([128, CHK], F32R, tag="s0l", bufs=2, name=f"s0l_{t}_{hf}")
                s1h = work.tile([128, CHK], F32R, tag="s1h", bufs=2, name=f"s1h_{t}_{hf}")
                s1l = work.tile([128, CHK], F32R, tag="s1l", bufs=2, name=f"s1l_{t}_{hf}")
                nc.scalar.copy(s0h[HI, :], Lh[LO, fs])
                nc.scalar.copy(s0l[HI, :], Ll[LO, fs])
                nc.gpsimd.tensor_copy(s1h[HI, :], Lh[HI, fs])
                nc.gpsimd.tensor_copy(s1l[HI, :], Ll[HI, fs])
                lst.append((s0h, s0l, s1h, s1l))

            # transpose lit state per half-grid (chunks 128,128,128,16 per
            # half) so the half-0 transposes + agg terms start while the
            # half-1 pointwise of the previous lit phase is still draining
            GRID = ((0, 128), (128, 128), (256, 128), (272, 128))
            rml = [[None, None], [None, None]]   # rml[g][half] = (rm_h, rm_l)
            for x, src in ((0, lit_hf_a), (1, lit_hf_b)):
                for g in range(GPC):
                    tp = pstp.tile([128, 4 * H], F32, tag="tp",
                                   name=f"tpl_{t}_{g}_{x}")
                    for kk, (c0, sz) in enumerate(GRID):
                        nc.tensor.transpose(
                            tp[:, kk * H:(kk + 1) * H],
                            src[HALF[g], c0:c0 + sz],
                            ident[HALF[g], HALF[g]],
                        )
                    rm_h = work.tile([128, 4 * H], F32R, tag=f"rmlh{g}{x}",
                                     name=f"rmlh_{t}_{g}_{x}")
                    rm_l = work.tile([128, 4 * H], F32R, tag=f"rmll{g}{x}",
                                     name=f"rmll_{t}_{g}_{x}")
                    nc.vector.tensor_copy(rm_h[:, :], tp[:, :])
                    nc.vector.tensor_tensor(rm_l[:, :], tp[:, :], rm_h[:, :],
                                            op=SUB)
                    rml[g][x] = (rm_h, rm_l)

            # clause agg A^T @ L: per-graph psum tiles (g0 rows HI, g1 LO) so
            # the hi/lo term groups interleave without bank conflicts
            agc0 = psag.tile([128, NC], F32, tag="ag", name=f"agc0_{t}")
            agc1 = psag.tile([128, NC], F32, tag="ag", name=f"agc1_{t}")
            agp = (agc0[LO, :], agc1[LO, :])
            for x in range(2):
                for term in range(2):
                    for g in range(GPC):
                        for kk in range(4):
                            c = NC * (8 * g + 4 * x + kk)
                            MM(agp[g],
                               rml[g][x][term][:, kk * H:(kk + 1) * H],
                               at_r[:, c:c + NC],
                               start=(x == 0 and term == 0 and kk == 0),
                               stop=(x == 1 and term == 1 and kk == 3),
                               tile_position=(0, 0))

            # stack agg halves (aligned at LO)
            nc.scalar.copy(st0h[LO, :], agc0[LO, :])
            nc.vector.tensor_tensor(st0l[LO, :], agc0[LO, :], st0h[LO, :], op=SUB)
            nc.scalar.copy(st1h[LO, :], agc1[LO, :])
            nc.vector.tensor_tensor(st1l[LO, :], agc1[LO, :], st1h[LO, :], op=SUB)

            # clause gates: per graph, 2 gate-pairs, 3-term split
            cg = [[None, None], [None, None]]
            for g in range(GPC):
                wnm = "wc_1" if first else "wc_b"
                wh, wl = wsp[wnm]
                sth, stl = (st0h, st0l) if g == 0 else (st1h, st1l)
                for p in range(2):
                    ps_ = slice(p * 128, (p + 1) * 128)
                    gp = psg.tile([128, NC], F32, tag="g", name=f"cg{g}{p}_{t}")
                    MM(gp[:, :], wh[:, ps_], sth[:, :], start=True, stop=False)
                    MM(gp[:, :], wl[:, ps_], sth[:, :], start=False, stop=False)
                    MM(gp[:, :], wh[:, ps_], stl[:, :], start=False, stop=True)
                    cg[g][p] = gp

            # repack sigmoids: pair-psum -> graph-packed s tiles
            s_i = work.tile([128, NC], F32, tag="si", name=f"csi_{t}")
            s_f = work.tile([128, NC], F32, tag="sf", name=f"csf_{t}")
            s_g = work.tile([128, NC], F32, tag="sg", name=f"csg_{t}")
            s_o = work.tile([128, NC], F32, tag="so", name=f"cso_{t}")
            for g in range(GPC):
                h = HALF[g]
                nc.scalar.activation(s_i[h, :], cg[g][0][LO, :], SIG,
                                     bias=bias[h, 0:1])
                nc.scalar.activation(s_g[h, :], cg[g][1][LO, :], SIG,
                                     bias=bias[h, 2:3])
            for g in range(GPC):
                h = HALF[g]
                nc.scalar.activation(s_f[h, :], cg[g][0][HI, :], SIG,
                                     bias=bias[h, 1:2])
                nc.scalar.activation(s_o[h, :], cg[g][1][HI, :], SIG,
                                     bias=bias[h, 3:4])

            # clause LSTM pointwise (graph-packed, full height)
            cc_new = state.tile([128, NC], F32, tag="cl_c", name=f"cc_{t}")
            t1 = work.tile([128, NC], F32, tag="t1", name=f"ct1_{t}")
            nc.vector.tensor_mul(t1[:, :], s_i[:, :], s_g[:, :])
            if first:
                nc.vector.scalar_tensor_tensor(
                    cc_new[:, :], t1[:, :], 2.0, s_i[:, :], op0=MULT, op1=SUB)
            else:
                u = work.tile([128, NC], F32, tag="u", name=f"cu_{t}")
                nc.vector.scalar_tensor_tensor(
                    u[:, :], t1[:, :], 2.0, s_i[:, :], op0=MULT, op1=SUB)
                t2 = work.tile([128, NC], F32, tag="t2", name=f"ct2_{t}")
                nc.vector.tensor_mul(t2[:, :], s_f[:, :], cl_c[:, :])
                nc.vector.tensor_add(cc_new[:, :], u[:, :], t2[:, :])
            tnc = work.tile([128, NC], F32, tag="tnc", name=f"ctn_{t}")
            nc.scalar.activation(tnc[:, :], cc_new[:, :], SIG, scale=2.0)
            t3 = work.tile([128, NC], F32, tag="t3", name=f"ct3_{t}")
            nc.vector.tensor_mul(t3[:, :], s_o[:, :], tnc[:, :])
            cl_hf_new = state.tile([128, NC], F32, tag="cl_hf", name=f"chf_{t}")
            nc.vector.scalar_tensor_tensor(
                cl_hf_new[:, :], t3[:, :], 2.0, s_o[:, :], op0=MULT, op1=SUB)
            # split off the critical path (consumers are next iteration)
            Chh_new = state.tile([128, NC], F32R, tag="Chh", name=f"Chh_{t}")
            Chl_new = state.tile([128, NC], F32R, tag="Chl", name=f"Chl_{t}")
            nc.gpsimd.tensor_copy(Chh_new[:, :], cl_hf_new[:, :])
            nc.vector.tensor_tensor(Chl_new[:, :], cl_hf_new[:, :], Chh_new[:, :],
                                    op=SUB)

            # ==== lit phase ====
            # transpose full clause state; split rides the psum copy
            rmc = []
            for g in range(GPC):
                tp = pstp.tile([128, 4 * H], F32, tag="tp", name=f"tpc_{t}_{g}")
                for kk in range(4):
                    c0 = 128 * kk if kk < 3 else NC - 128
                    nc.tensor.transpose(
                        tp[:, kk * H:(kk + 1) * H],
                        cl_hf_new[HALF[g], c0:c0 + 128],
                        ident[HALF[g], HALF[g]],
                    )
                rm_h = work.tile([128, 4 * H], F32R, tag=f"rmch{g}",
                                 name=f"rmch_{t}_{g}")
                rm_l = work.tile([128, 4 * H], F32R, tag=f"rmcl{g}",
                                 name=f"rmcl_{t}_{g}")
                nc.scalar.copy(rm_h[:, :], tp[:, :])
                nc.vector.tensor_tensor(rm_l[:, :], tp[:, :], rm_h[:, :], op=SUB)
                rmc.append((rm_h, rm_l))

            Lh_new = state.tile([128, NL], F32R, tag="Lh", name=f"Lh_{t}")
            Ll_new = state.tile([128, NL], F32R, tag="Ll", name=f"Ll_{t}")
            lit_hf_na = state.tile([128, CHK], F32, tag="lit_hf_a", name=f"lhfa_{t}")
            lit_hf_nb = state.tile([128, CHK], F32, tag="lit_hf_b", name=f"lhfb_{t}")
            lc_new = state.tile([128, NL], F32, tag="lit_c", name=f"lc_{t}")
            wSnm = "w_cl2_dup" if first else "w_lh_dup"
            wSh, wSl = wsp[wSnm]
            for hf in range(2):
                cs = slice(hf * CHK, (hf + 1) * CHK)
                s0h, s0l, s1h, s1l = lst[hf]
                # literal agg A @ C: per-graph psums, interleaved term groups
                agl0 = psag.tile([128, CHK], F32, tag="ag", name=f"agl0_{t}_{hf}")
                agl1 = psag.tile([128, CHK], F32, tag="ag", name=f"agl1_{t}_{hf}")
                agp = (agl0[LO, :], agl1[LO, :])
                for term in range(2):
                    for g in range(GPC):
                        for kk in range(4):
                            MM(agp[g], rmc[g][term][:, kk * H:(kk + 1) * H],
                               a_r[:, NL * (4 * g + kk) + hf * CHK:
                                   NL * (4 * g + kk) + (hf + 1) * CHK],
                               start=(term == 0 and kk == 0),
                               stop=(term == 1 and kk == 3),
                               tile_position=(0, 0))

                # lit gates: dq + rec terms first (no stack dependency -> they
                # fill the PE pipeline while stacks build)
                gps = [[None, None], [None, None]]
                for g in range(GPC):
                    hg = HALF[g]
                    for p in range(2):
                        ps_ = slice(p * 128, (p + 1) * 128)
                        gp = psg.tile([128, CHK], F32, tag="g", name=f"lg{g}{p}_{t}_{hf}")
                        dc = slice(g * NL + hf * CHK, g * NL + (hf + 1) * CHK)
                        MM(gp[:, :], wdq_h[0:2, ps_], dxr[0:2, dc],
                           start=True, stop=False)
                        MM(gp[:, :], wdq_l[0:2, ps_], dxr[0:2, dc],
                           start=False, stop=False)
                        MM(gp[:, :], wSh[hg, ps_], Lh[hg, cs], start=False,
                           stop=False, tile_position=(64 * g, 0))
                        MM(gp[:, :], wSl[hg, ps_], Lh[hg, cs], start=False,
                           stop=False, tile_position=(64 * g, 0))
                        MM(gp[:, :], wSh[hg, ps_], Ll[hg, cs], start=False,
                           stop=False, tile_position=(64 * g, 0))
                        gps[g][p] = gp

                # stack agg halves (aligned at LO)
                nc.scalar.copy(s0h[LO, :], agl0[LO, :])
                nc.vector.tensor_tensor(s0l[LO, :], agl0[LO, :], s0h[LO, :], op=SUB)
                nc.scalar.copy(s1h[LO, :], agl1[LO, :])
                nc.vector.tensor_tensor(s1l[LO, :], agl1[LO, :], s1h[LO, :], op=SUB)

                # stack-dependent gate terms
                for g in range(GPC):
                    wnm = "wl_b"
                    wh, wl = wsp[wnm]
                    sth, stl = (s0h, s0l) if g == 0 else (s1h, s1l)
                    for p in range(2):
                        ps_ = slice(p * 128, (p + 1) * 128)
                        gp = gps[g][p]
                        MM(gp[:, :], wh[:, ps_], sth[:, :], start=False, stop=False)
                        MM(gp[:, :], wl[:, ps_], sth[:, :], start=False, stop=False)
                        MM(gp[:, :], wh[:, ps_], stl[:, :], start=False, stop=True)

                s_i = work.tile([128, CHK], F32, tag="lsi", bufs=2, name=f"lsi_{t}_{hf}")
                s_f = work.tile([128, CHK], F32, tag="lsf", bufs=2, name=f"lsf_{t}_{hf}")
                s_g = work.tile([128, CHK], F32, tag="lsg", bufs=2, name=f"lsg_{t}_{hf}")
                s_o = work.tile([128, CHK], F32, tag="lso", bufs=2, name=f"lso_{t}_{hf}")
                for g in range(GPC):
                    h = HALF[g]
                    nc.scalar.activation(s_i[h, :], gps[g][0][LO, :], SIG)
                    nc.scalar.activation(s_g[h, :], gps[g][1][LO, :], SIG)
                for g in range(GPC):
                    h = HALF[g]
                    nc.scalar.activation(s_f[h, :], gps[g][0][HI, :], SIG)
                    nc.scalar.activation(s_o[h, :], gps[g][1][HI, :], SIG)

                t1 = work.tile([128, CHK], F32, tag="lt1", bufs=2, name=f"lt1_{t}_{hf}")
                nc.vector.tensor_mul(t1[:, :], s_i[:, :], s_g[:, :])
                if first:
                    nc.vector.scalar_tensor_tensor(
                        lc_new[:, cs], t1[:, :], 2.0, s_i[:, :], op0=MULT, op1=SUB)
                else:
                    u = work.tile([128, CHK], F32, tag="lu", bufs=2, name=f"lu_{t}_{hf}")
                    nc.vector.scalar_tensor_tensor(
                        u[:, :], t1[:, :], 2.0, s_i[:, :], op0=MULT, op1=SUB)
                    t2 = work.tile([128, CHK], F32, tag="lt2", bufs=1, name=f"lt2_{t}_{hf}")
                    nc.vector.tensor_mul(t2[:, :], s_f[:, :], lit_c[:, cs])
                    nc.vector.tensor_add(lc_new[:, cs], u[:, :], t2[:, :])
                tnc = work.tile([128, CHK], F32, tag="ltn", bufs=1, name=f"ltn_{t}_{hf}")
                nc.scalar.activation(tnc[:, :], lc_new[:, cs], SIG, scale=2.0)
                t3 = work.tile([128, CHK], F32, tag="lt3", bufs=1, name=f"lt3_{t}_{hf}")
                nc.vector.tensor_mul(t3[:, :], s_o[:, :], tnc[:, :])
                lhf_x = lit_hf_na if hf == 0 else lit_hf_nb
                nc.vector.scalar_tensor_tensor(
                    lhf_x[:, :], t3[:, :], 2.0, s_o[:, :], op0=MULT, op1=SUB)
                # split off the critical path (consumers are next iteration)
                nc.gpsimd.tensor_copy(Lh_new[:, cs], lhf_x[:, :])
                nc.vector.tensor_tensor(Ll_new[:, cs], lhf_x[:, :],
                                        Lh_new[:, cs], op=SUB)

            # PE ramp warmers: dependency-free matmuls that run inside the
            # end-of-iteration stall (PE waiting on the half-1 pointwise
            # chain) so the tensor engine re-enters the next iteration at
            # full p-state instead of paying the 2x mid-ramp on the aggs.
            if t < ITERS - 1:
                wp = pstp.tile([128, 256], F32, tag="tp", name=f"warm_{t}")
                for wi in range(0):
                    MM(wp[:, :], identr[0:1, :], dxr[0:1, 0:256],
                       start=True, stop=True)

            Lh, Ll = Lh_new, Ll_new
            lit_hf_a, lit_hf_b = lit_hf_na, lit_hf_nb
            Chh, Chl, cl_hf = Chh_new, Chl_new, cl_hf_new
            lit_c, cl_c = lc_new, cc_new

        # ---- vote head: (Lh+Ll) @ Wv, 3-term split ----
        for g in range(GPC):
            hg = HALF[g]
            for hf in range(2):
                cs = slice(hf * CHK, (hf + 1) * CHK)
                p = psg.tile([1, CHK], F32, tag="g", name=f"vps_{g}_{hf}")
                MM(p[:, :], wv_h[hg, 0:1], Lh[hg, cs], start=True, stop=False,
                   tile_position=(64 * g, 0))
                MM(p[:, :], wv_l[hg, 0:1], Lh[hg, cs], start=False, stop=False,
                   tile_position=(64 * g, 0))
                MM(p[:, :], wv_h[hg, 0:1], Ll[hg, cs], start=False, stop=True,
                   tile_position=(64 * g, 0))
                vc = work.tile([1, CHK], F32, tag="vote", bufs=1,
                               name=f"vote_{g}_{hf}")
                nc.scalar.activation(
                    vc[:, :], p[:, :], mybir.ActivationFunctionType.Identity,
                    bias=bias[0:1, 4:5],
                )
                nc.sync.dma_start(
                    out=d_out[0:1, g * NL + hf * CHK:g * NL + (hf + 1) * CHK],
                    in_=vc[:, :])

    nc.compile()
    return nc


def _fold_and_shard(inputs):
    """Host-side preprocessing: fold weights, build adjacency, shard by graph."""
    f32 = np.float32
    g = {k: np.asarray(v) for k, v in inputs.items()}

    def collapse(w1, b1, w2, b2, w3, b3):
        return w1 @ w2 @ w3, ((b1 @ w2) + b2) @ w3 + b3

    Wl, bl = collapse(g["lm1_w"], g["lm1_b"], g["lm2_w"], g["lm2_b"],
                      g["lm3_w"], g["lm3_b"])
    Wc, bc = collapse(g["cm1_w"], g["cm1_b"], g["cm2_w"], g["cm2_b"],
                      g["cm3_w"], g["cm3_b"])
    Wv, bv = collapse(g["lv1_w"], g["lv1_b"], g["lv2_w"], g["lv2_b"],
                      g["lv3_w"], g["lv3_b"])

    cu_wih, lu_wih = g["cu_wih"], g["lu_wih"]
    w_lc = (Wl @ cu_wih).astype(f32)                 # agg_c -> clause gates
    w_ch = (w_lc + g["cu_whh"]).astype(f32)          # t>=2 merged recurrent
    cbias_c = ((K + 1) * (bl @ cu_wih) + g["cu_bih"] + g["cu_bhh"]).astype(f32)
    wih_a = lu_wih[0:H].astype(f32)                  # flip -> lit gates
    w_cl2 = (Wc @ lu_wih[H:2 * H]).astype(f32)       # agg_l -> lit gates
    w_lh = (w_cl2 + g["lu_whh"]).astype(f32)         # t>=2 merged recurrent
    q_l = (bc @ lu_wih[H:2 * H]).astype(f32)         # [256]
    cbias_l = (g["lu_bih"] + g["lu_bhh"]).astype(f32)

    def gdouble(w):
        w = w.copy()
        w[:, 2 * H:3 * H] *= 2.0     # g-gate runs as sigmoid(2x+2b)
        return w

    vs = np.vstack
    wc_a = gdouble(vs([w_ch, w_lc]))
    wc_b = gdouble(vs([w_lc, w_ch]))
    wc_1 = gdouble(vs([w_lc, w_lc]))
    wl_a = gdouble(vs([wih_a, w_cl2]))
    wl_b = gdouble(vs([w_cl2, wih_a]))
    w_lh_dup = gdouble(vs([w_lh, w_lh]))
    w_cl2_dup = gdouble(vs([w_cl2, w_cl2]))
    wv_dup = vs([Wv.astype(f32), Wv.astype(f32)])

    bias_q = np.zeros((128, 5), f32)
    for x in range(4):
        scl = 2.0 if x == 2 else 1.0
        bias_q[0:64, x] = scl * cbias_c[x * H:(x + 1) * H]
        bias_q[64:128, x] = scl * cbias_c[x * H:(x + 1) * H]
    bias_q[0, 4] = bv[0]

    li_w3 = np.concatenate([g["li_w"], g["li_b"][None, :]], axis=0).astype(f32)
    ci_w3 = np.concatenate([g["ci_w"], g["ci_b"][None, :]], axis=0).astype(f32)

    # adjacency per graph from edge_index (direction-robust)
    ei = g["edge_index"].astype(np.int64)
    src, dst = ei[0], ei[1]
    src_g, dst_g = src // NPG, dst // NPG
    assert np.all(src_g == dst_g), "edges must be graph-local"
    src_l, dst_l = src % NPG, dst % NPG
    s_lit, d_lit = src_l < NL, dst_l < NL
    A_in_c = np.zeros((B, NC, NL), f32)   # clause <- literal edges
    m = (~d_lit) & s_lit
    np.add.at(A_in_c, (dst_g[m], dst_l[m] - NL, src_l[m]), 1.0)
    A_in_l = np.zeros((B, NL, NC), f32)   # literal <- clause edges
    m = d_lit & (~s_lit)
    np.add.at(A_in_l, (dst_g[m], dst_l[m], src_l[m] - NL), 1.0)
    deg_l = A_in_l.sum(axis=2)            # [B, NL]

    x = g["x"].astype(f32).reshape(B, NPG, 2)
    ones = np.ones((B, NPG, 1), f32)
    x3 = np.concatenate([x, ones], axis=2)        # [B, NPG, 3]

    shared = dict(
        wc_a=wc_a, wc_b=wc_b, wc_1=wc_1, wl_a=wl_a, wl_b=wl_b,
        w_lh_dup=w_lh_dup, w_cl2_dup=w_cl2_dup, wv_dup=wv_dup,
        li_w3=li_w3, ci_w3=ci_w3, bias_q=bias_q,
    )
    in_maps = []
    for c in range(NCORES):
        gs = slice(c * GPC, (c + 1) * GPC)
        x3c = x3[gs]                               # [GPC, NPG, 3]
        xt_lit = np.ascontiguousarray(
            x3c[:, :NL].transpose(2, 0, 1).reshape(3, GPC * NL))
        xt_cl = np.ascontiguousarray(
            x3c[:, NL:].transpose(2, 0, 1).reshape(3, GPC * NC))
        # dxr rows: (deg+1) per literal, ones; wdq rows: q, cbias_l
        # (g-gate block doubled to match the pre-doubled weights)
        dxr = np.ones((2, GPC * NL), f32)
        for gg in range(GPC):
            dxr[0, gg * NL:(gg + 1) * NL] = deg_l[c * GPC + gg] + 1.0
        wdq = np.stack([q_l, cbias_l]).astype(f32)
        wdq[:, 2 * H:3 * H] *= 2.0
        # pre-chunk adjacency into full-128-row K-chunks; the final chunk
        # overlaps the previous one with its overlap rows zeroed
        atc = np.zeros((GPC, 8, 128, NC), f32)
        ac = np.zeros((GPC, 4, 128, NL), f32)
        for gg in range(GPC):
            at_full = A_in_c[c * GPC + gg].T       # [NL, NC]
            a_full = A_in_l[c * GPC + gg].T        # [NC, NL]
            for x in range(2):
                for j in range(3):
                    atc[gg, 4 * x + j] = at_full[x * 400 + 128 * j:
                                                 x * 400 + 128 * (j + 1)]
                atc[gg, 4 * x + 3, 112:128] = at_full[x * 400 + 384:
                                                      x * 400 + 400]
            for kk in range(3):
                ac[gg, kk] = a_full[128 * kk:128 * (kk + 1)]
            ac[gg, 3, 128 - (NC - 384):] = a_full[384:]
        in_maps.append(dict(
            xt_lit=xt_lit, xt_cl=xt_cl, at_rm=atc, a_rm=ac,
            dxr=dxr, wdq=wdq, **shared,
        ))
    return in_maps


_LAST_RESULTS = {}


def kernel(**inputs):
    from concourse.bass_utils import run_bass_kernel_spmd

    in_maps = _fold_and_shard(inputs)
    if "nc" not in _PROGRAM_CACHE:
        _PROGRAM_CACHE["nc"] = _build_program()
    nc = _PROGRAM_CACHE["nc"]
    res = run_bass_kernel_spmd(nc, in_maps, core_ids=list(range(NCORES)))
    _LAST_RESULTS["res"] = res
    out = np.zeros((N, 1), np.float32)
    for c in range(NCORES):
        vote = res.results[c]["vote"].reshape(GPC, NL)
        for g in range(GPC):
            base = (c * GPC + g) * NPG
            out[base:base + NL, 0] = vote[g]
    return out


# revision 6
# speedup vs baseline: 1.0462x; 1.0116x over previous
"""NeuroSAT GNN message passing on 8 Trainium2 NeuronCores — v2.

Speedups over the v1 graph-data-parallel kernel:
  * All large matmuls run as fp32r (hw-rounded fp32, ~11 mantissa bits) at
    1 cycle/row instead of fp32's 4. Accuracy is restored with a hi/lo
    split: x = hi + lo with hi = round_f32r(x) (free: the producing op
    writes an f32r tile), lo = x - hi. A matmul A@B becomes
    Ah@Bh + Al@Bh + Ah@Bl (dropped lo*lo term is ~2^-24 relative).
    Aggregation matmuls need only 2 terms: the adjacency matrices are
    small integers, exact in f32r.
  * Gate matmuls pair two gates on the 128 output partitions (M=128
    instead of 64), halving streamed rows. The pair-packed PSUM is
    repacked to graph-packed tiles by the sigmoid activations themselves
    (single-input acts may cross partition offsets; 2-input DVE ops may
    not), so the LSTM pointwise stays full-height.
  * The per-literal degree bias (+ lit gate biases) is added once per
    gate-pair psum on DVE; clause gate biases ride the activation bias.
    The g-gate's tanh(x)=2*sigmoid(2x)-1 input doubling is pre-folded
    into the host-side weights/biases, keeping every activation a plain
    table sigmoid.

Layout: per core 2 graphs; feature-major state tiles [128, nodes] with
graph0 on partitions 0:64, graph1 on 64:128, kept in split (hi, lo)
f32r form. Row-major (transposed) hi/lo copies feed the aggregation
matmuls against constant f32r adjacency chunk tiles.
"""

import numpy as np

H = 64
ITERS = 24
B, NV, NC, K = 16, 400, 440, 12
NL = 2 * NV                  # literals/graph = 800
NPG = NL + NC                # nodes/graph = 1240
N = B * NPG                  # 19840
NCORES = 8
GPC = B // NCORES            # graphs per core = 2
CHK = 400                    # literal column chunk (aligned to NV flip halves)

_PROGRAM_CACHE = {}


def _build_program():
    from contextlib import ExitStack

    import concourse.bacc as bacc
    import concourse.mybir as mybir
    from concourse.masks import make_identity
    from concourse.tile import TileContext, add_dep_helper

    F32 = mybir.dt.float32
    F32R = mybir.dt.float32r
    SIG = mybir.ActivationFunctionType.Sigmoid
    MULT = mybir.AluOpType.mult
    SUB = mybir.AluOpType.subtract

    nc = bacc.Bacc(
        "TRN2", target_bir_lowering=False, debug=False, num_devices=NCORES
    )

    # ---- DRAM I/O (per-core shards; weights replicated) ----
    d_xt_lit = nc.dram_tensor("xt_lit", [3, GPC * NL], F32, kind="ExternalInput")
    d_xt_cl = nc.dram_tensor("xt_cl", [3, GPC * NC], F32, kind="ExternalInput")
    d_at = nc.dram_tensor("at_rm", [GPC, 8, 128, NC], F32, kind="ExternalInput")
    d_a = nc.dram_tensor("a_rm", [GPC, 4, 128, NL], F32, kind="ExternalInput")
    WNAMES = ("wc_a", "wc_b", "wc_1", "wl_a", "wl_b", "w_lh_dup", "w_cl2_dup")
    d_w = {nm: nc.dram_tensor(nm, [128, 256], F32, kind="ExternalInput")
           for nm in WNAMES}
    d_wv = nc.dram_tensor("wv_dup", [128, 1], F32, kind="ExternalInput")
    d_liw = nc.dram_tensor("li_w3", [3, H], F32, kind="ExternalInput")
    d_ciw = nc.dram_tensor("ci_w3", [3, H], F32, kind="ExternalInput")
    d_bias = nc.dram_tensor("bias_q", [128, 5], F32, kind="ExternalInput")
    d_dxr = nc.dram_tensor("dxr", [2, GPC * NL], F32, kind="ExternalInput")
    d_wdq = nc.dram_tensor("wdq", [2, 256], F32, kind="ExternalInput")
    d_out = nc.dram_tensor("vote", [1, GPC * NL], F32, kind="ExternalOutput")

    with TileContext(nc) as tc, ExitStack() as ctx:
        const = ctx.enter_context(tc.tile_pool(name="const", bufs=1))
        state = ctx.enter_context(tc.tile_pool(name="state", bufs=2))
        work = ctx.enter_context(tc.tile_pool(name="work", bufs=1))
        pstp = ctx.enter_context(tc.tile_pool(name="pstp", bufs=2, space="PSUM"))
        psag = ctx.enter_context(tc.tile_pool(name="psag", bufs=2, space="PSUM"))
        psg = ctx.enter_context(tc.tile_pool(name="psg", bufs=4, space="PSUM"))

        LO, HI = slice(0, 64), slice(64, 128)
        HALF = (LO, HI)

        # ---- constants ----
        ident = const.tile([128, 128], F32, name="ident")
        make_identity(nc, ident)
        identr = const.tile([128, 128], F32R, name="identr")
        nc.scalar.copy(identr[:, :], ident[:, :])

        # adjacency chunks -> f32r const tiles (integers: cvt exact)
        at_r = const.tile([128, GPC * 8 * NC], F32R, name="at_r")
        for g in range(GPC):
            for kk in range(8):
                stg = work.tile([128, NC], F32, tag="ld", bufs=1,
                                name=f"ld_at_{g}_{kk}")
                nc.sync.dma_start(out=stg[:, :], in_=d_at[g, kk])
                c0 = NC * (8 * g + kk)
                nc.scalar.copy(at_r[:, c0:c0 + NC], stg[:, :])
        a_r = const.tile([128, GPC * 4 * NL], F32R, name="a_r")
        for g in range(GPC):
            for kk in range(4):
                stg = work.tile([128, NL], F32, tag="ld", bufs=1,
                                name=f"ld_a_{g}_{kk}")
                nc.sync.dma_start(out=stg[:, :], in_=d_a[g, kk])
                c0 = NL * (4 * g + kk)
                nc.scalar.copy(a_r[:, c0:c0 + NL], stg[:, :])

        # gate weights -> (hi, lo) f32r pairs
        wsp = {}
        for nm in WNAMES:
            stg = work.tile([128, 256], F32, tag="ld", bufs=1, name=f"ldw_{nm}")
            nc.sync.dma_start(out=stg[:, :], in_=d_w[nm][:, :])
            wh = const.tile([128, 256], F32R, name=f"{nm}_h")
            wl = const.tile([128, 256], F32R, name=f"{nm}_l")
            nc.scalar.copy(wh[:, :], stg[:, :])
            nc.vector.tensor_tensor(wl[:, :], stg[:, :], wh[:, :], op=SUB)
            wsp[nm] = (wh, wl)
        stg = work.tile([128, 1], F32, tag="ld", bufs=1, name="ldw_wv")
        nc.sync.dma_start(out=stg[:, :], in_=d_wv[:, :])
        wv_h = const.tile([128, 1], F32R, name="wv_h")
        wv_l = const.tile([128, 1], F32R, name="wv_l")
        nc.scalar.copy(wv_h[:, :], stg[:, :])
        nc.vector.tensor_tensor(wv_l[:, :], stg[:, :], wv_h[:, :], op=SUB)

        def load(dram, shape, nm):
            t = const.tile(shape, F32, name=nm)
            nc.sync.dma_start(out=t[:, :], in_=dram[:, :])
            return t

        xt_lit = load(d_xt_lit, [3, GPC * NL], "xt_lit_sb")
        xt_cl = load(d_xt_cl, [3, GPC * NC], "xt_cl_sb")
        li_w = load(d_liw, [3, H], "li_w_sb")
        ci_w = load(d_ciw, [3, H], "ci_w_sb")
        bias = load(d_bias, [128, 5], "bias_sb")
        stg = work.tile([2, GPC * NL], F32, tag="ld2", bufs=1, name="ld_dxr")
        nc.sync.dma_start(out=stg[:, :], in_=d_dxr[:, :])
        dxr = const.tile([2, GPC * NL], F32R, name="dxr_sb")
        nc.scalar.copy(dxr[:, :], stg[:, :])
        stg = work.tile([2, 256], F32, tag="ld3", bufs=1, name="ld_wdq")
        nc.sync.dma_start(out=stg[:, :], in_=d_wdq[:, :])
        wdq_h = const.tile([2, 256], F32R, name="wdq_h")
        wdq_l = const.tile([2, 256], F32R, name="wdq_l")
        nc.scalar.copy(wdq_h[:, :], stg[:, :])
        nc.vector.tensor_tensor(wdq_l[:, :], stg[:, :], wdq_h[:, :], op=SUB)

        def MM(*a, **kw):
            kw.setdefault("skip_group_check", True)
            return nc.tensor.matmul(*a, **kw)

        # ---- initial node states (bias via ones row of xt) ----
        Lh = state.tile([128, NL], F32R, tag="Lh", name="Lh0")
        Ll = state.tile([128, NL], F32R, tag="Ll", name="Ll0")
        lit_hf_a = state.tile([128, CHK], F32, tag="lit_hf_a", name="lit_hf_a0")
        lit_hf_b = state.tile([128, CHK], F32, tag="lit_hf_b", name="lit_hf_b0")
        for hf in range(2):
            p = psg.tile([128, CHK], F32, tag="g", name=f"ini_{hf}")
            prev = None
            for g in range(GPC):
                mm = MM(p[HALF[g], :], li_w[0:3, :],
                        xt_lit[0:3, g * NL + hf * CHK:g * NL + (hf + 1) * CHK],
                        start=True, stop=True, tile_position=(0, 64 * g))
                if prev is not None:
                    add_dep_helper(mm.ins, prev.ins, sync=True,
                                   reason="psum half order")
                prev = mm
            cs = slice(hf * CHK, (hf + 1) * CHK)
            nc.vector.tensor_copy((lit_hf_a if hf == 0 else lit_hf_b)[:, :],
                                  p[:, :])
            nc.scalar.copy(Lh[:, cs], p[:, :])
            nc.vector.tensor_tensor(Ll[:, cs], p[:, :], Lh[:, cs], op=SUB)
        Chh = state.tile([128, NC], F32R, tag="Chh", name="Chh0")
        Chl = state.tile([128, NC], F32R, tag="Chl", name="Chl0")
        cl_hf = state.tile([128, NC], F32, tag="cl_hf", name="cl_hf0")
        pc = psg.tile([128, NC], F32, tag="g", name="ini_c")
        prev = None
        for g in range(GPC):
            mm = MM(pc[HALF[g], :], ci_w[0:3, :], xt_cl[0:3, g * NC:(g + 1) * NC],
                    start=True, stop=True, tile_position=(0, 64 * g))
            if prev is not None:
                add_dep_helper(mm.ins, prev.ins, sync=True, reason="psum half order")
            prev = mm
        nc.vector.tensor_copy(cl_hf[:, :], pc[:, :])
        nc.scalar.copy(Chh[:, :], pc[:, :])
        nc.vector.tensor_tensor(Chl[:, :], pc[:, :], Chh[:, :], op=SUB)

        lit_c = None
        cl_c = None

        for t in range(1, ITERS):
            first = t == 1

            # ==== clause phase ====
            # dependency-free copies first: clause stack state-halves and the
            # lit-phase flip halves (keeps Pool busy off the critical path)
            st0h = work.tile([128, NC], F32R, tag="st0h", name=f"st0h_{t}")
            st0l = work.tile([128, NC], F32R, tag="st0l", name=f"st0l_{t}")
            st1h = work.tile([128, NC], F32R, tag="st1h", name=f"st1h_{t}")
            st1l = work.tile([128, NC], F32R, tag="st1l", name=f"st1l_{t}")
            nc.scalar.copy(st0h[HI, :], Chh[LO, :])
            nc.scalar.copy(st0l[HI, :], Chl[LO, :])
            nc.gpsimd.tensor_copy(st1h[HI, :], Chh[HI, :])
            nc.gpsimd.tensor_copy(st1l[HI, :], Chl[HI, :])
            lst = []
            for hf in range(2):
                fs = slice((1 - hf) * CHK, (2 - hf) * CHK)
                s0h = work.tile([128, CHK], F32R, tag="s0h", bufs=2, name=f"s0h_{t}_{hf}")
                s0l = work.tile([128, CHK], F32R, tag="s0l", bufs=2, name=f"s0l_{t}_{hf}")
                s1h = work.tile([128, CHK], F32R, tag="s1h", bufs=2, name=f"s1h_{t}_{hf}")
                s1l = work.tile([128, CHK], F32R, tag="s1l", bufs=2, name=f"s1l_{t}_{hf}")
                nc.scalar.copy(s0h[HI, :], Lh[LO, fs])
                nc.scalar.copy(s0l[HI, :], Ll[LO, fs])
                nc.gpsimd.tensor_copy(s1h[HI, :], Lh[HI, fs])
                nc.gpsimd.tensor_copy(s1l[HI, :], Ll[HI, fs])
                lst.append((s0h, s0l, s1h, s1l))

            # transpose lit state per half-grid (chunks 128,128,128,16 per
            # half) so the half-0 transposes + agg terms start while the
            # half-1 pointwise of the previous lit phase is still draining
            GRID = ((0, 128), (128, 128), (256, 128), (272, 128))
            rml = [[None, None], [None, None]]   # rml[g][half] = (rm_h, rm_l)
            for x, src in ((0, lit_hf_a), (1, lit_hf_b)):
                for g in range(GPC):
                    tp = pstp.tile([128, 4 * H], F32, tag="tp",
                                   name=f"tpl_{t}_{g}_{x}")
                    for kk, (c0, sz) in enumerate(GRID):
                        nc.tensor.transpose(
                            tp[:, kk * H:(kk + 1) * H],
                            src[HALF[g], c0:c0 + sz],
                            ident[HALF[g], HALF[g]],
                        )
                    rm_h = work.tile([128, 4 * H], F32R, tag=f"rmlh{g}{x}",
                                     name=f"rmlh_{t}_{g}_{x}")
                    rm_l = work.tile([128, 4 * H], F32R, tag=f"rmll{g}{x}",
                                     name=f"rmll_{t}_{g}_{x}")
                    nc.vector.tensor_copy(rm_h[:, :], tp[:, :])
                    nc.vector.tensor_tensor(rm_l[:, :], tp[:, :], rm_h[:, :],
                                            op=SUB)
                    rml[g][x] = (rm_h, rm_l)

            # clause agg A^T @ L: per-graph psum tiles (g0 rows HI, g1 LO) so
            # the hi/lo term groups interleave without bank conflicts
            agc0 = psag.tile([128, NC], F32, tag="ag", name=f"agc0_{t}")
            agc1 = psag.tile([128, NC], F32, tag="ag", name=f"agc1_{t}")
            agp = (agc0[LO, :], agc1[LO, :])
            for x in range(2):
                for term in range(2):
                    for g in range(GPC):
                        for kk in range(4):
                            c = NC * (8 * g + 4 * x + kk)
                            MM(agp[g],
                               rml[g][x][term][:, kk * H:(kk + 1) * H],
                               at_r[:, c:c + NC],
                               start=(x == 0 and term == 0 and kk == 0),
                               stop=(x == 1 and term == 1 and kk == 3),
                               tile_position=(0, 0))

            # stack agg halves (aligned at LO)
            nc.scalar.copy(st0h[LO, :], agc0[LO, :])
            nc.vector.tensor_tensor(st0l[LO, :], agc0[LO, :], st0h[LO, :], op=SUB)
            nc.scalar.copy(st1h[LO, :], agc1[LO, :])
            nc.vector.tensor_tensor(st1l[LO, :], agc1[LO, :], st1h[LO, :], op=SUB)

            # clause gates: per graph, 2 gate-pairs, 3-term split
            cg = [[None, None], [None, None]]
            for g in range(GPC):
                wnm = "wc_1" if first else "wc_b"
                wh, wl = wsp[wnm]
                sth, stl = (st0h, st0l) if g == 0 else (st1h, st1l)
                for p in range(2):
                    ps_ = slice(p * 128, (p + 1) * 128)
                    gp = psg.tile([128, NC], F32, tag="g", name=f"cg{g}{p}_{t}")
                    MM(gp[:, :], wh[:, ps_], sth[:, :], start=True, stop=False)
                    MM(gp[:, :], wl[:, ps_], sth[:, :], start=False, stop=False)
                    MM(gp[:, :], wh[:, ps_], stl[:, :], start=False, stop=True)
                    cg[g][p] = gp

            # repack sigmoids: pair-psum -> graph-packed s tiles
            s_i = work.tile([128, NC], F32, tag="si", name=f"csi_{t}")
            s_f = work.tile([128, NC], F32, tag="sf", name=f"csf_{t}")
            s_g = work.tile([128, NC], F32, tag="sg", name=f"csg_{t}")
            s_o = work.tile([128, NC], F32, tag="so", name=f"cso_{t}")
            for g in range(GPC):
                h = HALF[g]
                nc.scalar.activation(s_i[h, :], cg[g][0][LO, :], SIG,
                                     bias=bias[h, 0:1])
                nc.scalar.activation(s_g[h, :], cg[g][1][LO, :], SIG,
                                     bias=bias[h, 2:3])
            for g in range(GPC):
                h = HALF[g]
                nc.scalar.activation(s_f[h, :], cg[g][0][HI, :], SIG,
                                     bias=bias[h, 1:2])
                nc.scalar.activation(s_o[h, :], cg[g][1][HI, :], SIG,
                                     bias=bias[h, 3:4])

            # clause LSTM pointwise (graph-packed, full height)
            cc_new = state.tile([128, NC], F32, tag="cl_c", name=f"cc_{t}")
            t1 = work.tile([128, NC], F32, tag="t1", name=f"ct1_{t}")
            # pointwise chain column-split across DVE (lo cols) and Pool (hi)
            CSP = ((nc.vector, slice(0, 220)), (nc.gpsimd, slice(220, NC)))
            for eng, q in CSP:
                eng.tensor_mul(t1[:, q], s_i[:, q], s_g[:, q])
            if first:
                nc.vector.scalar_tensor_tensor(
                    cc_new[:, :], t1[:, :], 2.0, s_i[:, :], op0=MULT, op1=SUB)
            else:
                u = work.tile([128, NC], F32, tag="u", name=f"cu_{t}")
                t2 = work.tile([128, NC], F32, tag="t2", name=f"ct2_{t}")
                nc.vector.scalar_tensor_tensor(
                    u[:, :], t1[:, :], 2.0, s_i[:, :], op0=MULT, op1=SUB)
                for eng, q in CSP:
                    eng.tensor_mul(t2[:, q], s_f[:, q], cl_c[:, q])
                for eng, q in CSP:
                    eng.tensor_add(cc_new[:, q], u[:, q], t2[:, q])
            tnc = work.tile([128, NC], F32, tag="tnc", name=f"ctn_{t}")
            nc.scalar.activation(tnc[:, :], cc_new[:, :], SIG, scale=2.0)
            t3 = work.tile([128, NC], F32, tag="t3", name=f"ct3_{t}")
            cl_hf_new = state.tile([128, NC], F32, tag="cl_hf", name=f"chf_{t}")
            for eng, q in CSP:
                eng.tensor_mul(t3[:, q], s_o[:, q], tnc[:, q])
            nc.vector.scalar_tensor_tensor(
                cl_hf_new[:, :], t3[:, :], 2.0, s_o[:, :], op0=MULT, op1=SUB)
            # split off the critical path (consumers are next iteration)
            Chh_new = state.tile([128, NC], F32R, tag="Chh", name=f"Chh_{t}")
            Chl_new = state.tile([128, NC], F32R, tag="Chl", name=f"Chl_{t}")
            nc.gpsimd.tensor_copy(Chh_new[:, :], cl_hf_new[:, :])
            nc.vector.tensor_tensor(Chl_new[:, :], cl_hf_new[:, :], Chh_new[:, :],
                                    op=SUB)

            # ==== lit phase ====
            # transpose full clause state; split rides the psum copy
            rmc = []
            for g in range(GPC):
                tp = pstp.tile([128, 4 * H], F32, tag="tp", name=f"tpc_{t}_{g}")
                for kk in range(4):
                    c0 = 128 * kk if kk < 3 else NC - 128
                    nc.tensor.transpose(
                        tp[:, kk * H:(kk + 1) * H],
                        cl_hf_new[HALF[g], c0:c0 + 128],
                        ident[HALF[g], HALF[g]],
                    )
                rm_h = work.tile([128, 4 * H], F32R, tag=f"rmch{g}",
                                 name=f"rmch_{t}_{g}")
                rm_l = work.tile([128, 4 * H], F32R, tag=f"rmcl{g}",
                                 name=f"rmcl_{t}_{g}")
                nc.scalar.copy(rm_h[:, :], tp[:, :])
                nc.vector.tensor_tensor(rm_l[:, :], tp[:, :], rm_h[:, :], op=SUB)
                rmc.append((rm_h, rm_l))

            Lh_new = state.tile([128, NL], F32R, tag="Lh", name=f"Lh_{t}")
            Ll_new = state.tile([128, NL], F32R, tag="Ll", name=f"Ll_{t}")
            lit_hf_na = state.tile([128, CHK], F32, tag="lit_hf_a", name=f"lhfa_{t}")
            lit_hf_nb = state.tile([128, CHK], F32, tag="lit_hf_b", name=f"lhfb_{t}")
            lc_new = state.tile([128, NL], F32, tag="lit_c", name=f"lc_{t}")
            wSnm = "w_cl2_dup" if first else "w_lh_dup"
            wSh, wSl = wsp[wSnm]
            for hf in range(2):
                cs = slice(hf * CHK, (hf + 1) * CHK)
                s0h, s0l, s1h, s1l = lst[hf]
                # literal agg A @ C: per-graph psums, interleaved term groups
                agl0 = psag.tile([128, CHK], F32, tag="ag", name=f"agl0_{t}_{hf}")
                agl1 = psag.tile([128, CHK], F32, tag="ag", name=f"agl1_{t}_{hf}")
                agp = (agl0[LO, :], agl1[LO, :])
                for term in range(2):
                    for g in range(GPC):
                        for kk in range(4):
                            MM(agp[g], rmc[g][term][:, kk * H:(kk + 1) * H],
                               a_r[:, NL * (4 * g + kk) + hf * CHK:
                                   NL * (4 * g + kk) + (hf + 1) * CHK],
                               start=(term == 0 and kk == 0),
                               stop=(term == 1 and kk == 3),
                               tile_position=(0, 0))

                # lit gates: dq + rec terms first (no stack dependency -> they
                # fill the PE pipeline while stacks build)
                gps = [[None, None], [None, None]]
                for g in range(GPC):
                    hg = HALF[g]
                    for p in range(2):
                        ps_ = slice(p * 128, (p + 1) * 128)
                        gp = psg.tile([128, CHK], F32, tag="g", name=f"lg{g}{p}_{t}_{hf}")
                        dc = slice(g * NL + hf * CHK, g * NL + (hf + 1) * CHK)
                        MM(gp[:, :], wdq_h[0:2, ps_], dxr[0:2, dc],
                           start=True, stop=False)
                        MM(gp[:, :], wdq_l[0:2, ps_], dxr[0:2, dc],
                           start=False, stop=False)
                        MM(gp[:, :], wSh[hg, ps_], Lh[hg, cs], start=False,
                           stop=False, tile_position=(64 * g, 0))
                        MM(gp[:, :], wSl[hg, ps_], Lh[hg, cs], start=False,
                           stop=False, tile_position=(64 * g, 0))
                        MM(gp[:, :], wSh[hg, ps_], Ll[hg, cs], start=False,
                           stop=False, tile_position=(64 * g, 0))
                        gps[g][p] = gp

                # stack agg halves (aligned at LO)
                nc.scalar.copy(s0h[LO, :], agl0[LO, :])
                nc.vector.tensor_tensor(s0l[LO, :], agl0[LO, :], s0h[LO, :], op=SUB)
                nc.scalar.copy(s1h[LO, :], agl1[LO, :])
                nc.vector.tensor_tensor(s1l[LO, :], agl1[LO, :], s1h[LO, :], op=SUB)

                # stack-dependent gate terms
                for g in range(GPC):
                    wnm = "wl_b"
                    wh, wl = wsp[wnm]
                    sth, stl = (s0h, s0l) if g == 0 else (s1h, s1l)
                    for p in range(2):
                        ps_ = slice(p * 128, (p + 1) * 128)
                        gp = gps[g][p]
                        MM(gp[:, :], wh[:, ps_], sth[:, :], start=False, stop=False)
                        MM(gp[:, :], wl[:, ps_], sth[:, :], start=False, stop=False)
                        MM(gp[:, :], wh[:, ps_], stl[:, :], start=False, stop=True)

                s_i = work.tile([128, CHK], F32, tag="lsi", bufs=2, name=f"lsi_{t}_{hf}")
                s_f = work.tile([128, CHK], F32, tag="lsf", bufs=2, name=f"lsf_{t}_{hf}")
                s_g = work.tile([128, CHK], F32, tag="lsg", bufs=2, name=f"lsg_{t}_{hf}")
                s_o = work.tile([128, CHK], F32, tag="lso", bufs=2, name=f"lso_{t}_{hf}")
                for g in range(GPC):
                    h = HALF[g]
                    nc.scalar.activation(s_i[h, :], gps[g][0][LO, :], SIG)
                    nc.scalar.activation(s_g[h, :], gps[g][1][LO, :], SIG)
                for g in range(GPC):
                    h = HALF[g]
                    nc.scalar.activation(s_f[h, :], gps[g][0][HI, :], SIG)
                    nc.scalar.activation(s_o[h, :], gps[g][1][HI, :], SIG)

                t1 = work.tile([128, CHK], F32, tag="lt1", bufs=2, name=f"lt1_{t}_{hf}")
                LSP = ((nc.vector, slice(0, 200)), (nc.gpsimd, slice(200, CHK)))
                for eng, q in LSP:
                    eng.tensor_mul(t1[:, q], s_i[:, q], s_g[:, q])
                cq = [slice(cs.start, cs.start + 200),
                      slice(cs.start + 200, cs.stop)]
                if first:
                    nc.vector.scalar_tensor_tensor(
                        lc_new[:, cs], t1[:, :], 2.0, s_i[:, :],
                        op0=MULT, op1=SUB)
                else:
                    u = work.tile([128, CHK], F32, tag="lu", bufs=2, name=f"lu_{t}_{hf}")
                    t2 = work.tile([128, CHK], F32, tag="lt2", bufs=1, name=f"lt2_{t}_{hf}")
                    nc.vector.scalar_tensor_tensor(
                        u[:, :], t1[:, :], 2.0, s_i[:, :], op0=MULT, op1=SUB)
                    for (eng, q), c2 in zip(LSP, cq):
                        eng.tensor_mul(t2[:, q], s_f[:, q], lit_c[:, c2])
                    for (eng, q), c2 in zip(LSP, cq):
                        eng.tensor_add(lc_new[:, c2], u[:, q], t2[:, q])
                tnc = work.tile([128, CHK], F32, tag="ltn", bufs=1, name=f"ltn_{t}_{hf}")
                nc.scalar.activation(tnc[:, :], lc_new[:, cs], SIG, scale=2.0)
                t3 = work.tile([128, CHK], F32, tag="lt3", bufs=1, name=f"lt3_{t}_{hf}")
                lhf_x = lit_hf_na if hf == 0 else lit_hf_nb
                for eng, q in LSP:
                    eng.tensor_mul(t3[:, q], s_o[:, q], tnc[:, q])
                nc.vector.scalar_tensor_tensor(
                    lhf_x[:, :], t3[:, :], 2.0, s_o[:, :], op0=MULT, op1=SUB)
                # split off the critical path (consumers are next iteration)
                nc.gpsimd.tensor_copy(Lh_new[:, cs], lhf_x[:, :])
                nc.vector.tensor_tensor(Ll_new[:, cs], lhf_x[:, :],
                                        Lh_new[:, cs], op=SUB)

            Lh, Ll = Lh_new, Ll_new
            lit_hf_a, lit_hf_b = lit_hf_na, lit_hf_nb
            Chh, Chl, cl_hf = Chh_new, Chl_new, cl_hf_new
            lit_c, cl_c = lc_new, cc_new

        # ---- vote head: (Lh+Ll) @ Wv, 3-term split ----
        for g in range(GPC):
            hg = HALF[g]
            for hf in range(2):
                cs = slice(hf * CHK, (hf + 1) * CHK)
                p = psg.tile([1, CHK], F32, tag="g", name=f"vps_{g}_{hf}")
                MM(p[:, :], wv_h[hg, 0:1], Lh[hg, cs], start=True, stop=False,
                   tile_position=(64 * g, 0))
                MM(p[:, :], wv_l[hg, 0:1], Lh[hg, cs], start=False, stop=False,
                   tile_position=(64 * g, 0))
                MM(p[:, :], wv_h[hg, 0:1], Ll[hg, cs], start=False, stop=True,
                   tile_position=(64 * g, 0))
                vc = work.tile([1, CHK], F32, tag="vote", bufs=1,
                               name=f"vote_{g}_{hf}")
                nc.scalar.activation(
                    vc[:, :], p[:, :], mybir.ActivationFunctionType.Identity,
                    bias=bias[0:1, 4:5],
                )
                nc.sync.dma_start(
                    out=d_out[0:1, g * NL + hf * CHK:g * NL + (hf + 1) * CHK],
                    in_=vc[:, :])

    nc.compile()
    return nc


def _fold_and_shard(inputs):
    """Host-side preprocessing: fold weights, build adjacency, shard by graph."""
    f32 = np.float32
    g = {k: np.asarray(v) for k, v in inputs.items()}

    def collapse(w1, b1, w2, b2, w3, b3):
        return w1 @ w2 @ w3, ((b1 @ w2) + b2) @ w3 + b3

    Wl, bl = collapse(g["lm1_w"], g["lm1_b"], g["lm2_w"], g["lm2_b"],
                      g["lm3_w"], g["lm3_b"])
    Wc, bc = collapse(g["cm1_w"], g["cm1_b"], g["cm2_w"], g["cm2_b"],
                      g["cm3_w"], g["cm3_b"])
    Wv, bv = collapse(g["lv1_w"], g["lv1_b"], g["lv2_w"], g["lv2_b"],
                      g["lv3_w"], g["lv3_b"])

    cu_wih, lu_wih = g["cu_wih"], g["lu_wih"]
    w_lc = (Wl @ cu_wih).astype(f32)                 # agg_c -> clause gates
    w_ch = (w_lc + g["cu_whh"]).astype(f32)          # t>=2 merged recurrent
    cbias_c = ((K + 1) * (bl @ cu_wih) + g["cu_bih"] + g["cu_bhh"]).astype(f32)
    wih_a = lu_wih[0:H].astype(f32)                  # flip -> lit gates
    w_cl2 = (Wc @ lu_wih[H:2 * H]).astype(f32)       # agg_l -> lit gates
    w_lh = (w_cl2 + g["lu_whh"]).astype(f32)         # t>=2 merged recurrent
    q_l = (bc @ lu_wih[H:2 * H]).astype(f32)         # [256]
    cbias_l = (g["lu_bih"] + g["lu_bhh"]).astype(f32)

    def gdouble(w):
        w = w.copy()
        w[:, 2 * H:3 * H] *= 2.0     # g-gate runs as sigmoid(2x+2b)
        return w

    vs = np.vstack
    wc_a = gdouble(vs([w_ch, w_lc]))
    wc_b = gdouble(vs([w_lc, w_ch]))
    wc_1 = gdouble(vs([w_lc, w_lc]))
    wl_a = gdouble(vs([wih_a, w_cl2]))
    wl_b = gdouble(vs([w_cl2, wih_a]))
    w_lh_dup = gdouble(vs([w_lh, w_lh]))
    w_cl2_dup = gdouble(vs([w_cl2, w_cl2]))
    wv_dup = vs([Wv.astype(f32), Wv.astype(f32)])

    bias_q = np.zeros((128, 5), f32)
    for x in range(4):
        scl = 2.0 if x == 2 else 1.0
        bias_q[0:64, x] = scl * cbias_c[x * H:(x + 1) * H]
        bias_q[64:128, x] = scl * cbias_c[x * H:(x + 1) * H]
    bias_q[0, 4] = bv[0]

    li_w3 = np.concatenate([g["li_w"], g["li_b"][None, :]], axis=0).astype(f32)
    ci_w3 = np.concatenate([g["ci_w"], g["ci_b"][None, :]], axis=0).astype(f32)

    # adjacency per graph from edge_index (direction-robust)
    ei = g["edge_index"].astype(np.int64)
    src, dst = ei[0], ei[1]
    src_g, dst_g = src // NPG, dst // NPG
    assert np.all(src_g == dst_g), "edges must be graph-local"
    src_l, dst_l = src % NPG, dst % NPG
    s_lit, d_lit = src_l < NL, dst_l < NL
    A_in_c = np.zeros((B, NC, NL), f32)   # clause <- literal edges
    m = (~d_lit) & s_lit
    np.add.at(A_in_c, (dst_g[m], dst_l[m] - NL, src_l[m]), 1.0)
    A_in_l = np.zeros((B, NL, NC), f32)   # literal <- clause edges
    m = d_lit & (~s_lit)
    np.add.at(A_in_l, (dst_g[m], dst_l[m], src_l[m] - NL), 1.0)
    deg_l = A_in_l.sum(axis=2)            # [B, NL]

    x = g["x"].astype(f32).reshape(B, NPG, 2)
    ones = np.ones((B, NPG, 1), f32)
    x3 = np.concatenate([x, ones], axis=2)        # [B, NPG, 3]

    shared = dict(
        wc_a=wc_a, wc_b=wc_b, wc_1=wc_1, wl_a=wl_a, wl_b=wl_b,
        w_lh_dup=w_lh_dup, w_cl2_dup=w_cl2_dup, wv_dup=wv_dup,
        li_w3=li_w3, ci_w3=ci_w3, bias_q=bias_q,
    )
    in_maps = []
    for c in range(NCORES):
        gs = slice(c * GPC, (c + 1) * GPC)
        x3c = x3[gs]                               # [GPC, NPG, 3]
        xt_lit = np.ascontiguousarray(
            x3c[:, :NL].transpose(2, 0, 1).reshape(3, GPC * NL))
        xt_cl = np.ascontiguousarray(
            x3c[:, NL:].transpose(2, 0, 1).reshape(3, GPC * NC))
        # dxr rows: (deg+1) per literal, ones; wdq rows: q, cbias_l
        # (g-gate block doubled to match the pre-doubled weights)
        dxr = np.ones((2, GPC * NL), f32)
        for gg in range(GPC):
            dxr[0, gg * NL:(gg + 1) * NL] = deg_l[c * GPC + gg] + 1.0
        wdq = np.stack([q_l, cbias_l]).astype(f32)
        wdq[:, 2 * H:3 * H] *= 2.0
        # pre-chunk adjacency into full-128-row K-chunks; the final chunk
        # overlaps the previous one with its overlap rows zeroed
        atc = np.zeros((GPC, 8, 128, NC), f32)
        ac = np.zeros((GPC, 4, 128, NL), f32)
        for gg in range(GPC):
            at_full = A_in_c[c * GPC + gg].T       # [NL, NC]
            a_full = A_in_l[c * GPC + gg].T        # [NC, NL]
            for x in range(2):
                for j in range(3):
                    atc[gg, 4 * x + j] = at_full[x * 400 + 128 * j:
                                                 x * 400 + 128 * (j + 1)]
                atc[gg, 4 * x + 3, 112:128] = at_full[x * 400 + 384:
                                                      x * 400 + 400]
            for kk in range(3):
                ac[gg, kk] = a_full[128 * kk:128 * (kk + 1)]
            ac[gg, 3, 128 - (NC - 384):] = a_full[384:]
        in_maps.append(dict(
            xt_lit=xt_lit, xt_cl=xt_cl, at_rm=atc, a_rm=ac,
            dxr=dxr, wdq=wdq, **shared,
        ))
    return in_maps


_LAST_RESULTS = {}


def kernel(**inputs):
    from concourse.bass_utils import run_bass_kernel_spmd

    in_maps = _fold_and_shard(inputs)
    if "nc" not in _PROGRAM_CACHE:
        _PROGRAM_CACHE["nc"] = _build_program()
    nc = _PROGRAM_CACHE["nc"]
    res = run_bass_kernel_spmd(nc, in_maps, core_ids=list(range(NCORES)))
    _LAST_RESULTS["res"] = res
    out = np.zeros((N, 1), np.float32)
    for c in range(NCORES):
        vote = res.results[c]["vote"].reshape(GPC, NL)
        for g in range(GPC):
            base = (c * GPC + g) * NPG
            out[base:base + NL, 0] = vote[g]
    return out


# revision 7
# speedup vs baseline: 1.0595x; 1.0127x over previous
"""NeuroSAT GNN message passing on 8 Trainium2 NeuronCores — v2.

Speedups over the v1 graph-data-parallel kernel:
  * All large matmuls run as fp32r (hw-rounded fp32, ~11 mantissa bits) at
    1 cycle/row instead of fp32's 4. Accuracy is restored with a hi/lo
    split: x = hi + lo with hi = round_f32r(x) (free: the producing op
    writes an f32r tile), lo = x - hi. A matmul A@B becomes
    Ah@Bh + Al@Bh + Ah@Bl (dropped lo*lo term is ~2^-24 relative).
    Aggregation matmuls need only 2 terms: the adjacency matrices are
    small integers, exact in f32r.
  * Gate matmuls pair two gates on the 128 output partitions (M=128
    instead of 64), halving streamed rows. The pair-packed PSUM is
    repacked to graph-packed tiles by the sigmoid activations themselves
    (single-input acts may cross partition offsets; 2-input DVE ops may
    not), so the LSTM pointwise stays full-height.
  * The per-literal degree bias (+ lit gate biases) is added once per
    gate-pair psum on DVE; clause gate biases ride the activation bias.
    The g-gate's tanh(x)=2*sigmoid(2x)-1 input doubling is pre-folded
    into the host-side weights/biases, keeping every activation a plain
    table sigmoid.

Layout: per core 2 graphs; feature-major state tiles [128, nodes] with
graph0 on partitions 0:64, graph1 on 64:128, kept in split (hi, lo)
f32r form. Row-major (transposed) hi/lo copies feed the aggregation
matmuls against constant f32r adjacency chunk tiles.
"""

import numpy as np

H = 64
ITERS = 24
B, NV, NC, K = 16, 400, 440, 12
NL = 2 * NV                  # literals/graph = 800
NPG = NL + NC                # nodes/graph = 1240
N = B * NPG                  # 19840
NCORES = 8
GPC = B // NCORES            # graphs per core = 2
CHK = 400                    # literal column chunk (aligned to NV flip halves)

_PROGRAM_CACHE = {}


def _build_program():
    from contextlib import ExitStack

    import concourse.bacc as bacc
    import concourse.mybir as mybir
    from concourse.masks import make_identity
    from concourse.tile import TileContext, add_dep_helper

    F32 = mybir.dt.float32
    F32R = mybir.dt.float32r
    SIG = mybir.ActivationFunctionType.Sigmoid
    MULT = mybir.AluOpType.mult
    SUB = mybir.AluOpType.subtract
    ADD = mybir.AluOpType.add

    nc = bacc.Bacc(
        "TRN2", target_bir_lowering=False, debug=False, num_devices=NCORES
    )

    # ---- DRAM I/O (per-core shards; weights replicated) ----
    d_xt_lit = nc.dram_tensor("xt_lit", [3, GPC * NL], F32, kind="ExternalInput")
    d_xt_cl = nc.dram_tensor("xt_cl", [3, GPC * NC], F32, kind="ExternalInput")
    d_at = nc.dram_tensor("at_rm", [GPC, 8, 128, NC], F32, kind="ExternalInput")
    d_a = nc.dram_tensor("a_rm", [GPC, 4, 128, NL], F32, kind="ExternalInput")
    WNAMES = ("wc_a", "wc_b", "wc_1", "wl_a", "wl_b", "w_lh_dup", "w_cl2_dup")
    d_w = {nm: nc.dram_tensor(nm, [128, 256], F32, kind="ExternalInput")
           for nm in WNAMES}
    d_wv = nc.dram_tensor("wv_dup", [128, 1], F32, kind="ExternalInput")
    d_liw = nc.dram_tensor("li_w3", [3, H], F32, kind="ExternalInput")
    d_ciw = nc.dram_tensor("ci_w3", [3, H], F32, kind="ExternalInput")
    d_bias = nc.dram_tensor("bias_q", [128, 5], F32, kind="ExternalInput")
    d_dxr = nc.dram_tensor("dxr", [2, GPC * NL], F32, kind="ExternalInput")
    d_wdq = nc.dram_tensor("wdq", [2, 256], F32, kind="ExternalInput")
    d_out = nc.dram_tensor("vote", [1, GPC * NL], F32, kind="ExternalOutput")

    with TileContext(nc) as tc, ExitStack() as ctx:
        const = ctx.enter_context(tc.tile_pool(name="const", bufs=1))
        state = ctx.enter_context(tc.tile_pool(name="state", bufs=2))
        work = ctx.enter_context(tc.tile_pool(name="work", bufs=1))
        pstp = ctx.enter_context(tc.tile_pool(name="pstp", bufs=2, space="PSUM"))
        psag = ctx.enter_context(tc.tile_pool(name="psag", bufs=2, space="PSUM"))
        psg = ctx.enter_context(tc.tile_pool(name="psg", bufs=4, space="PSUM"))

        LO, HI = slice(0, 64), slice(64, 128)
        HALF = (LO, HI)

        # ---- constants ----
        ident = const.tile([128, 128], F32, name="ident")
        make_identity(nc, ident)
        identr = const.tile([128, 128], F32R, name="identr")
        nc.scalar.copy(identr[:, :], ident[:, :])

        # adjacency chunks -> f32r const tiles (integers: cvt exact)
        at_r = const.tile([128, GPC * 8 * NC], F32R, name="at_r")
        for g in range(GPC):
            for kk in range(8):
                stg = work.tile([128, NC], F32, tag="ld", bufs=1,
                                name=f"ld_at_{g}_{kk}")
                nc.sync.dma_start(out=stg[:, :], in_=d_at[g, kk])
                c0 = NC * (8 * g + kk)
                nc.scalar.copy(at_r[:, c0:c0 + NC], stg[:, :])
        a_r = const.tile([128, GPC * 4 * NL], F32R, name="a_r")
        for g in range(GPC):
            for kk in range(4):
                stg = work.tile([128, NL], F32, tag="ld", bufs=1,
                                name=f"ld_a_{g}_{kk}")
                nc.sync.dma_start(out=stg[:, :], in_=d_a[g, kk])
                c0 = NL * (4 * g + kk)
                nc.scalar.copy(a_r[:, c0:c0 + NL], stg[:, :])

        # gate weights -> (hi, lo) f32r pairs
        wsp = {}
        for nm in WNAMES:
            stg = work.tile([128, 256], F32, tag="ld", bufs=1, name=f"ldw_{nm}")
            nc.sync.dma_start(out=stg[:, :], in_=d_w[nm][:, :])
            wh = const.tile([128, 256], F32R, name=f"{nm}_h")
            wl = const.tile([128, 256], F32R, name=f"{nm}_l")
            nc.scalar.copy(wh[:, :], stg[:, :])
            nc.vector.tensor_tensor(wl[:, :], stg[:, :], wh[:, :], op=SUB)
            wsp[nm] = (wh, wl)
        stg = work.tile([128, 1], F32, tag="ld", bufs=1, name="ldw_wv")
        nc.sync.dma_start(out=stg[:, :], in_=d_wv[:, :])
        wv_h = const.tile([128, 1], F32R, name="wv_h")
        wv_l = const.tile([128, 1], F32R, name="wv_l")
        nc.scalar.copy(wv_h[:, :], stg[:, :])
        nc.vector.tensor_tensor(wv_l[:, :], stg[:, :], wv_h[:, :], op=SUB)

        def load(dram, shape, nm):
            t = const.tile(shape, F32, name=nm)
            nc.sync.dma_start(out=t[:, :], in_=dram[:, :])
            return t

        xt_lit = load(d_xt_lit, [3, GPC * NL], "xt_lit_sb")
        xt_cl = load(d_xt_cl, [3, GPC * NC], "xt_cl_sb")
        li_w = load(d_liw, [3, H], "li_w_sb")
        ci_w = load(d_ciw, [3, H], "ci_w_sb")
        bias = load(d_bias, [128, 5], "bias_sb")
        stg = work.tile([2, GPC * NL], F32, tag="ld2", bufs=1, name="ld_dxr")
        nc.sync.dma_start(out=stg[:, :], in_=d_dxr[:, :])
        dxr = const.tile([2, GPC * NL], F32R, name="dxr_sb")
        nc.scalar.copy(dxr[:, :], stg[:, :])
        stg = work.tile([2, 256], F32, tag="ld3", bufs=1, name="ld_wdq")
        nc.sync.dma_start(out=stg[:, :], in_=d_wdq[:, :])
        wdq_h = const.tile([2, 256], F32R, name="wdq_h")
        wdq_l = const.tile([2, 256], F32R, name="wdq_l")
        nc.scalar.copy(wdq_h[:, :], stg[:, :])
        nc.vector.tensor_tensor(wdq_l[:, :], stg[:, :], wdq_h[:, :], op=SUB)

        def MM(*a, **kw):
            kw.setdefault("skip_group_check", True)
            return nc.tensor.matmul(*a, **kw)

        # ---- initial node states (bias via ones row of xt) ----
        Lh = state.tile([128, NL], F32R, tag="Lh", name="Lh0")
        Ll = state.tile([128, NL], F32R, tag="Ll", name="Ll0")
        lit_hf_a = state.tile([128, CHK], F32, tag="lit_hf_a", name="lit_hf_a0")
        lit_hf_b = state.tile([128, CHK], F32, tag="lit_hf_b", name="lit_hf_b0")
        for hf in range(2):
            p = psg.tile([128, CHK], F32, tag="g", name=f"ini_{hf}")
            prev = None
            for g in range(GPC):
                mm = MM(p[HALF[g], :], li_w[0:3, :],
                        xt_lit[0:3, g * NL + hf * CHK:g * NL + (hf + 1) * CHK],
                        start=True, stop=True, tile_position=(0, 64 * g))
                if prev is not None:
                    add_dep_helper(mm.ins, prev.ins, sync=True,
                                   reason="psum half order")
                prev = mm
            cs = slice(hf * CHK, (hf + 1) * CHK)
            nc.vector.tensor_copy((lit_hf_a if hf == 0 else lit_hf_b)[:, :],
                                  p[:, :])
            nc.scalar.copy(Lh[:, cs], p[:, :])
            nc.vector.tensor_tensor(Ll[:, cs], p[:, :], Lh[:, cs], op=SUB)
        Chh = state.tile([128, NC], F32R, tag="Chh", name="Chh0")
        Chl = state.tile([128, NC], F32R, tag="Chl", name="Chl0")
        cl_hf = state.tile([128, NC], F32, tag="cl_hf", name="cl_hf0")
        pc = psg.tile([128, NC], F32, tag="g", name="ini_c")
        prev = None
        for g in range(GPC):
            mm = MM(pc[HALF[g], :], ci_w[0:3, :], xt_cl[0:3, g * NC:(g + 1) * NC],
                    start=True, stop=True, tile_position=(0, 64 * g))
            if prev is not None:
                add_dep_helper(mm.ins, prev.ins, sync=True, reason="psum half order")
            prev = mm
        nc.vector.tensor_copy(cl_hf[:, :], pc[:, :])
        nc.scalar.copy(Chh[:, :], pc[:, :])
        nc.vector.tensor_tensor(Chl[:, :], pc[:, :], Chh[:, :], op=SUB)

        lit_c = None
        cl_c = None

        for t in range(1, ITERS):
            first = t == 1

            # ==== clause phase ====
            # dependency-free copies first: clause stack state-halves and the
            # lit-phase flip halves (keeps Pool busy off the critical path)
            st0h = work.tile([128, NC], F32R, tag="st0h", name=f"st0h_{t}")
            st0l = work.tile([128, NC], F32R, tag="st0l", name=f"st0l_{t}")
            st1h = work.tile([128, NC], F32R, tag="st1h", name=f"st1h_{t}")
            st1l = work.tile([128, NC], F32R, tag="st1l", name=f"st1l_{t}")
            nc.scalar.copy(st0h[HI, :], Chh[LO, :])
            nc.scalar.copy(st0l[HI, :], Chl[LO, :])
            nc.gpsimd.tensor_copy(st1h[HI, :], Chh[HI, :])
            nc.gpsimd.tensor_copy(st1l[HI, :], Chl[HI, :])
            lst = []
            for hf in range(2):
                fs = slice((1 - hf) * CHK, (2 - hf) * CHK)
                s0h = work.tile([128, CHK], F32R, tag="s0h", bufs=2, name=f"s0h_{t}_{hf}")
                s0l = work.tile([128, CHK], F32R, tag="s0l", bufs=2, name=f"s0l_{t}_{hf}")
                s1h = work.tile([128, CHK], F32R, tag="s1h", bufs=2, name=f"s1h_{t}_{hf}")
                s1l = work.tile([128, CHK], F32R, tag="s1l", bufs=2, name=f"s1l_{t}_{hf}")
                nc.scalar.copy(s0h[HI, :], Lh[LO, fs])
                nc.scalar.copy(s0l[HI, :], Ll[LO, fs])
                nc.gpsimd.tensor_copy(s1h[HI, :], Lh[HI, fs])
                nc.gpsimd.tensor_copy(s1l[HI, :], Ll[HI, fs])
                lst.append((s0h, s0l, s1h, s1l))

            # transpose lit state per half-grid (chunks 128,128,128,16 per
            # half) so the half-0 transposes + agg terms start while the
            # half-1 pointwise of the previous lit phase is still draining
            GRID = ((0, 128), (128, 128), (256, 128), (272, 128))
            rml = [[None, None], [None, None]]   # rml[g][half] = (rm_h, rm_l)
            for x, src in ((0, lit_hf_a), (1, lit_hf_b)):
                for g in range(GPC):
                    tp = pstp.tile([128, 4 * H], F32, tag="tp",
                                   name=f"tpl_{t}_{g}_{x}")
                    for kk, (c0, sz) in enumerate(GRID):
                        nc.tensor.transpose(
                            tp[:, kk * H:(kk + 1) * H],
                            src[HALF[g], c0:c0 + sz],
                            ident[HALF[g], HALF[g]],
                        )
                    rm_h = work.tile([128, 4 * H], F32R, tag=f"rmlh{g}{x}",
                                     name=f"rmlh_{t}_{g}_{x}")
                    rm_l = work.tile([128, 4 * H], F32R, tag=f"rmll{g}{x}",
                                     name=f"rmll_{t}_{g}_{x}")
                    nc.vector.tensor_copy(rm_h[:, :], tp[:, :])
                    nc.vector.tensor_tensor(rm_l[:, :], tp[:, :], rm_h[:, :],
                                            op=SUB)
                    rml[g][x] = (rm_h, rm_l)

            # clause agg A^T @ L: per-graph psum tiles (g0 rows HI, g1 LO) so
            # the hi/lo term groups interleave without bank conflicts
            agc0 = psag.tile([128, NC], F32, tag="ag", name=f"agc0_{t}")
            agc1 = psag.tile([128, NC], F32, tag="ag", name=f"agc1_{t}")
            agp = (agc0[LO, :], agc1[LO, :])
            for x in range(2):
                for term in range(2):
                    for g in range(GPC):
                        for kk in range(4):
                            c = NC * (8 * g + 4 * x + kk)
                            MM(agp[g],
                               rml[g][x][term][:, kk * H:(kk + 1) * H],
                               at_r[:, c:c + NC],
                               start=(x == 0 and term == 0 and kk == 0),
                               stop=(x == 1 and term == 1 and kk == 3),
                               tile_position=(0, 0))

            # stack agg halves (aligned at LO)
            nc.scalar.copy(st0h[LO, :], agc0[LO, :])
            nc.vector.tensor_tensor(st0l[LO, :], agc0[LO, :], st0h[LO, :], op=SUB)
            nc.scalar.copy(st1h[LO, :], agc1[LO, :])
            nc.vector.tensor_tensor(st1l[LO, :], agc1[LO, :], st1h[LO, :], op=SUB)

            # clause gates: per graph, 2 gate-pairs, 3-term split
            cg = [[None, None], [None, None]]
            for g in range(GPC):
                wnm = "wc_1" if first else "wc_b"
                wh, wl = wsp[wnm]
                sth, stl = (st0h, st0l) if g == 0 else (st1h, st1l)
                for p in range(2):
                    ps_ = slice(p * 128, (p + 1) * 128)
                    gp = psg.tile([128, NC], F32, tag="g", name=f"cg{g}{p}_{t}")
                    MM(gp[:, :], wh[:, ps_], sth[:, :], start=True, stop=False)
                    MM(gp[:, :], wl[:, ps_], sth[:, :], start=False, stop=False)
                    MM(gp[:, :], wh[:, ps_], stl[:, :], start=False, stop=True)
                    cg[g][p] = gp

            # repack sigmoids: pair-psum -> graph-packed s tiles
            s_i = work.tile([128, NC], F32, tag="si", name=f"csi_{t}")
            s_f = work.tile([128, NC], F32, tag="sf", name=f"csf_{t}")
            s_g = work.tile([128, NC], F32, tag="sg", name=f"csg_{t}")
            s_o = work.tile([128, NC], F32, tag="so", name=f"cso_{t}")
            for g in range(GPC):
                h = HALF[g]
                nc.scalar.activation(s_i[h, :], cg[g][0][LO, :], SIG,
                                     bias=bias[h, 0:1])
                nc.scalar.activation(s_g[h, :], cg[g][1][LO, :], SIG,
                                     bias=bias[h, 2:3])
            for g in range(GPC):
                h = HALF[g]
                nc.scalar.activation(s_f[h, :], cg[g][0][HI, :], SIG,
                                     bias=bias[h, 1:2])
                nc.scalar.activation(s_o[h, :], cg[g][1][HI, :], SIG,
                                     bias=bias[h, 3:4])

            # clause LSTM pointwise (graph-packed, full height)
            cc_new = state.tile([128, NC], F32, tag="cl_c", name=f"cc_{t}")
            # pointwise chain column-split across DVE (lo cols) and Pool (hi);
            # tanh realized as tg = 2*sigmoid-1 first so every link is a
            # splittable tensor_scalar/tensor_tensor (no DVE-only stt)
            CSP = ((nc.vector, slice(0, 220)), (nc.gpsimd, slice(220, NC)))
            tg = work.tile([128, NC], F32, tag="t1", name=f"ctg_{t}")
            for eng, q in CSP:
                eng.tensor_scalar(tg[:, q], s_g[:, q], 2.0, -1.0,
                                  op0=MULT, op1=ADD)
            if first:
                for eng, q in CSP:
                    eng.tensor_mul(cc_new[:, q], s_i[:, q], tg[:, q])
            else:
                u = work.tile([128, NC], F32, tag="u", name=f"cu_{t}")
                t2 = work.tile([128, NC], F32, tag="t2", name=f"ct2_{t}")
                for eng, q in CSP:
                    eng.tensor_mul(u[:, q], s_i[:, q], tg[:, q])
                for eng, q in CSP:
                    eng.tensor_mul(t2[:, q], s_f[:, q], cl_c[:, q])
                for eng, q in CSP:
                    eng.tensor_add(cc_new[:, q], u[:, q], t2[:, q])
            tnc = work.tile([128, NC], F32, tag="tnc", name=f"ctn_{t}")
            nc.scalar.activation(tnc[:, :], cc_new[:, :], SIG, scale=2.0)
            th = work.tile([128, NC], F32, tag="t3", name=f"cth_{t}")
            cl_hf_new = state.tile([128, NC], F32, tag="cl_hf", name=f"chf_{t}")
            for eng, q in CSP:
                eng.tensor_scalar(th[:, q], tnc[:, q], 2.0, -1.0,
                                  op0=MULT, op1=ADD)
            for eng, q in CSP:
                eng.tensor_mul(cl_hf_new[:, q], s_o[:, q], th[:, q])
            # split off the critical path (consumers are next iteration)
            Chh_new = state.tile([128, NC], F32R, tag="Chh", name=f"Chh_{t}")
            Chl_new = state.tile([128, NC], F32R, tag="Chl", name=f"Chl_{t}")
            nc.gpsimd.tensor_copy(Chh_new[:, :], cl_hf_new[:, :])
            nc.vector.tensor_tensor(Chl_new[:, :], cl_hf_new[:, :], Chh_new[:, :],
                                    op=SUB)

            # ==== lit phase ====
            # transpose full clause state; split rides the psum copy
            rmc = []
            for g in range(GPC):
                tp = pstp.tile([128, 4 * H], F32, tag="tp", name=f"tpc_{t}_{g}")
                for kk in range(4):
                    c0 = 128 * kk if kk < 3 else NC - 128
                    nc.tensor.transpose(
                        tp[:, kk * H:(kk + 1) * H],
                        cl_hf_new[HALF[g], c0:c0 + 128],
                        ident[HALF[g], HALF[g]],
                    )
                rm_h = work.tile([128, 4 * H], F32R, tag=f"rmch{g}",
                                 name=f"rmch_{t}_{g}")
                rm_l = work.tile([128, 4 * H], F32R, tag=f"rmcl{g}",
                                 name=f"rmcl_{t}_{g}")
                nc.scalar.copy(rm_h[:, :], tp[:, :])
                nc.vector.tensor_tensor(rm_l[:, :], tp[:, :], rm_h[:, :], op=SUB)
                rmc.append((rm_h, rm_l))

            Lh_new = state.tile([128, NL], F32R, tag="Lh", name=f"Lh_{t}")
            Ll_new = state.tile([128, NL], F32R, tag="Ll", name=f"Ll_{t}")
            lit_hf_na = state.tile([128, CHK], F32, tag="lit_hf_a", name=f"lhfa_{t}")
            lit_hf_nb = state.tile([128, CHK], F32, tag="lit_hf_b", name=f"lhfb_{t}")
            lc_new = state.tile([128, NL], F32, tag="lit_c", name=f"lc_{t}")
            wSnm = "w_cl2_dup" if first else "w_lh_dup"
            wSh, wSl = wsp[wSnm]
            for hf in range(2):
                cs = slice(hf * CHK, (hf + 1) * CHK)
                s0h, s0l, s1h, s1l = lst[hf]
                # literal agg A @ C: per-graph psums, interleaved term groups
                agl0 = psag.tile([128, CHK], F32, tag="ag", name=f"agl0_{t}_{hf}")
                agl1 = psag.tile([128, CHK], F32, tag="ag", name=f"agl1_{t}_{hf}")
                agp = (agl0[LO, :], agl1[LO, :])
                for term in range(2):
                    for g in range(GPC):
                        for kk in range(4):
                            MM(agp[g], rmc[g][term][:, kk * H:(kk + 1) * H],
                               a_r[:, NL * (4 * g + kk) + hf * CHK:
                                   NL * (4 * g + kk) + (hf + 1) * CHK],
                               start=(term == 0 and kk == 0),
                               stop=(term == 1 and kk == 3),
                               tile_position=(0, 0))

                # lit gates: dq + rec terms first (no stack dependency -> they
                # fill the PE pipeline while stacks build)
                gps = [[None, None], [None, None]]
                for g in range(GPC):
                    hg = HALF[g]
                    for p in range(2):
                        ps_ = slice(p * 128, (p + 1) * 128)
                        gp = psg.tile([128, CHK], F32, tag="g", name=f"lg{g}{p}_{t}_{hf}")
                        dc = slice(g * NL + hf * CHK, g * NL + (hf + 1) * CHK)
                        MM(gp[:, :], wdq_h[0:2, ps_], dxr[0:2, dc],
                           start=True, stop=False)
                        MM(gp[:, :], wdq_l[0:2, ps_], dxr[0:2, dc],
                           start=False, stop=False)
                        MM(gp[:, :], wSh[hg, ps_], Lh[hg, cs], start=False,
                           stop=False, tile_position=(64 * g, 0))
                        MM(gp[:, :], wSl[hg, ps_], Lh[hg, cs], start=False,
                           stop=False, tile_position=(64 * g, 0))
                        MM(gp[:, :], wSh[hg, ps_], Ll[hg, cs], start=False,
                           stop=False, tile_position=(64 * g, 0))
                        gps[g][p] = gp

                # stack agg halves (aligned at LO)
                nc.scalar.copy(s0h[LO, :], agl0[LO, :])
                nc.vector.tensor_tensor(s0l[LO, :], agl0[LO, :], s0h[LO, :], op=SUB)
                nc.scalar.copy(s1h[LO, :], agl1[LO, :])
                nc.vector.tensor_tensor(s1l[LO, :], agl1[LO, :], s1h[LO, :], op=SUB)

                # stack-dependent gate terms
                for g in range(GPC):
                    wnm = "wl_b"
                    wh, wl = wsp[wnm]
                    sth, stl = (s0h, s0l) if g == 0 else (s1h, s1l)
                    for p in range(2):
                        ps_ = slice(p * 128, (p + 1) * 128)
                        gp = gps[g][p]
                        MM(gp[:, :], wh[:, ps_], sth[:, :], start=False, stop=False)
                        MM(gp[:, :], wl[:, ps_], sth[:, :], start=False, stop=False)
                        MM(gp[:, :], wh[:, ps_], stl[:, :], start=False, stop=True)

                s_i = work.tile([128, CHK], F32, tag="lsi", bufs=2, name=f"lsi_{t}_{hf}")
                s_f = work.tile([128, CHK], F32, tag="lsf", bufs=2, name=f"lsf_{t}_{hf}")
                s_g = work.tile([128, CHK], F32, tag="lsg", bufs=2, name=f"lsg_{t}_{hf}")
                s_o = work.tile([128, CHK], F32, tag="lso", bufs=2, name=f"lso_{t}_{hf}")
                for g in range(GPC):
                    h = HALF[g]
                    nc.scalar.activation(s_i[h, :], gps[g][0][LO, :], SIG)
                    nc.scalar.activation(s_g[h, :], gps[g][1][LO, :], SIG)
                for g in range(GPC):
                    h = HALF[g]
                    nc.scalar.activation(s_f[h, :], gps[g][0][HI, :], SIG)
                    nc.scalar.activation(s_o[h, :], gps[g][1][HI, :], SIG)

                tg = work.tile([128, CHK], F32, tag="lt1", bufs=2, name=f"ltg_{t}_{hf}")
                LSP = ((nc.vector, slice(0, 200)), (nc.gpsimd, slice(200, CHK)))
                cq = [slice(cs.start, cs.start + 200),
                      slice(cs.start + 200, cs.stop)]
                for eng, q in LSP:
                    eng.tensor_scalar(tg[:, q], s_g[:, q], 2.0, -1.0,
                                      op0=MULT, op1=ADD)
                if first:
                    for (eng, q), c2 in zip(LSP, cq):
                        eng.tensor_mul(lc_new[:, c2], s_i[:, q], tg[:, q])
                else:
                    u = work.tile([128, CHK], F32, tag="lu", bufs=2, name=f"lu_{t}_{hf}")
                    t2 = work.tile([128, CHK], F32, tag="lt2", bufs=1, name=f"lt2_{t}_{hf}")
                    for eng, q in LSP:
                        eng.tensor_mul(u[:, q], s_i[:, q], tg[:, q])
                    for (eng, q), c2 in zip(LSP, cq):
                        eng.tensor_mul(t2[:, q], s_f[:, q], lit_c[:, c2])
                    for (eng, q), c2 in zip(LSP, cq):
                        eng.tensor_add(lc_new[:, c2], u[:, q], t2[:, q])
                tnc = work.tile([128, CHK], F32, tag="ltn", bufs=1, name=f"ltn_{t}_{hf}")
                nc.scalar.activation(tnc[:, :], lc_new[:, cs], SIG, scale=2.0)
                th = work.tile([128, CHK], F32, tag="lt3", bufs=1, name=f"lth_{t}_{hf}")
                lhf_x = lit_hf_na if hf == 0 else lit_hf_nb
                for eng, q in LSP:
                    eng.tensor_scalar(th[:, q], tnc[:, q], 2.0, -1.0,
                                      op0=MULT, op1=ADD)
                for eng, q in LSP:
                    eng.tensor_mul(lhf_x[:, q], s_o[:, q], th[:, q])
                # split off the critical path (consumers are next iteration)
                nc.gpsimd.tensor_copy(Lh_new[:, cs], lhf_x[:, :])
                nc.vector.tensor_tensor(Ll_new[:, cs], lhf_x[:, :],
                                        Lh_new[:, cs], op=SUB)

            Lh, Ll = Lh_new, Ll_new
            lit_hf_a, lit_hf_b = lit_hf_na, lit_hf_nb
            Chh, Chl, cl_hf = Chh_new, Chl_new, cl_hf_new
            lit_c, cl_c = lc_new, cc_new

        # ---- vote head: (Lh+Ll) @ Wv, 3-term split ----
        for g in range(GPC):
            hg = HALF[g]
            for hf in range(2):
                cs = slice(hf * CHK, (hf + 1) * CHK)
                p = psg.tile([1, CHK], F32, tag="g", name=f"vps_{g}_{hf}")
                MM(p[:, :], wv_h[hg, 0:1], Lh[hg, cs], start=True, stop=False,
                   tile_position=(64 * g, 0))
                MM(p[:, :], wv_l[hg, 0:1], Lh[hg, cs], start=False, stop=False,
                   tile_position=(64 * g, 0))
                MM(p[:, :], wv_h[hg, 0:1], Ll[hg, cs], start=False, stop=True,
                   tile_position=(64 * g, 0))
                vc = work.tile([1, CHK], F32, tag="vote", bufs=1,
                               name=f"vote_{g}_{hf}")
                nc.scalar.activation(
                    vc[:, :], p[:, :], mybir.ActivationFunctionType.Identity,
                    bias=bias[0:1, 4:5],
                )
                nc.sync.dma_start(
                    out=d_out[0:1, g * NL + hf * CHK:g * NL + (hf + 1) * CHK],
                    in_=vc[:, :])

    nc.compile()
    return nc


def _fold_and_shard(inputs):
    """Host-side preprocessing: fold weights, build adjacency, shard by graph."""
    f32 = np.float32
    g = {k: np.asarray(v) for k, v in inputs.items()}

    def collapse(w1, b1, w2, b2, w3, b3):
        return w1 @ w2 @ w3, ((b1 @ w2) + b2) @ w3 + b3

    Wl, bl = collapse(g["lm1_w"], g["lm1_b"], g["lm2_w"], g["lm2_b"],
                      g["lm3_w"], g["lm3_b"])
    Wc, bc = collapse(g["cm1_w"], g["cm1_b"], g["cm2_w"], g["cm2_b"],
                      g["cm3_w"], g["cm3_b"])
    Wv, bv = collapse(g["lv1_w"], g["lv1_b"], g["lv2_w"], g["lv2_b"],
                      g["lv3_w"], g["lv3_b"])

    cu_wih, lu_wih = g["cu_wih"], g["lu_wih"]
    w_lc = (Wl @ cu_wih).astype(f32)                 # agg_c -> clause gates
    w_ch = (w_lc + g["cu_whh"]).astype(f32)          # t>=2 merged recurrent
    cbias_c = ((K + 1) * (bl @ cu_wih) + g["cu_bih"] + g["cu_bhh"]).astype(f32)
    wih_a = lu_wih[0:H].astype(f32)                  # flip -> lit gates
    w_cl2 = (Wc @ lu_wih[H:2 * H]).astype(f32)       # agg_l -> lit gates
    w_lh = (w_cl2 + g["lu_whh"]).astype(f32)         # t>=2 merged recurrent
    q_l = (bc @ lu_wih[H:2 * H]).astype(f32)         # [256]
    cbias_l = (g["lu_bih"] + g["lu_bhh"]).astype(f32)

    def gdouble(w):
        w = w.copy()
        w[:, 2 * H:3 * H] *= 2.0     # g-gate runs as sigmoid(2x+2b)
        return w

    vs = np.vstack
    wc_a = gdouble(vs([w_ch, w_lc]))
    wc_b = gdouble(vs([w_lc, w_ch]))
    wc_1 = gdouble(vs([w_lc, w_lc]))
    wl_a = gdouble(vs([wih_a, w_cl2]))
    wl_b = gdouble(vs([w_cl2, wih_a]))
    w_lh_dup = gdouble(vs([w_lh, w_lh]))
    w_cl2_dup = gdouble(vs([w_cl2, w_cl2]))
    wv_dup = vs([Wv.astype(f32), Wv.astype(f32)])

    bias_q = np.zeros((128, 5), f32)
    for x in range(4):
        scl = 2.0 if x == 2 else 1.0
        bias_q[0:64, x] = scl * cbias_c[x * H:(x + 1) * H]
        bias_q[64:128, x] = scl * cbias_c[x * H:(x + 1) * H]
    bias_q[0, 4] = bv[0]

    li_w3 = np.concatenate([g["li_w"], g["li_b"][None, :]], axis=0).astype(f32)
    ci_w3 = np.concatenate([g["ci_w"], g["ci_b"][None, :]], axis=0).astype(f32)

    # adjacency per graph from edge_index (direction-robust)
    ei = g["edge_index"].astype(np.int64)
    src, dst = ei[0], ei[1]
    src_g, dst_g = src // NPG, dst // NPG
    assert np.all(src_g == dst_g), "edges must be graph-local"
    src_l, dst_l = src % NPG, dst % NPG
    s_lit, d_lit = src_l < NL, dst_l < NL
    A_in_c = np.zeros((B, NC, NL), f32)   # clause <- literal edges
    m = (~d_lit) & s_lit
    np.add.at(A_in_c, (dst_g[m], dst_l[m] - NL, src_l[m]), 1.0)
    A_in_l = np.zeros((B, NL, NC), f32)   # literal <- clause edges
    m = d_lit & (~s_lit)
    np.add.at(A_in_l, (dst_g[m], dst_l[m], src_l[m] - NL), 1.0)
    deg_l = A_in_l.sum(axis=2)            # [B, NL]

    x = g["x"].astype(f32).reshape(B, NPG, 2)
    ones = np.ones((B, NPG, 1), f32)
    x3 = np.concatenate([x, ones], axis=2)        # [B, NPG, 3]

    shared = dict(
        wc_a=wc_a, wc_b=wc_b, wc_1=wc_1, wl_a=wl_a, wl_b=wl_b,
        w_lh_dup=w_lh_dup, w_cl2_dup=w_cl2_dup, wv_dup=wv_dup,
        li_w3=li_w3, ci_w3=ci_w3, bias_q=bias_q,
    )
    in_maps = []
    for c in range(NCORES):
        gs = slice(c * GPC, (c + 1) * GPC)
        x3c = x3[gs]                               # [GPC, NPG, 3]
        xt_lit = np.ascontiguousarray(
            x3c[:, :NL].transpose(2, 0, 1).reshape(3, GPC * NL))
        xt_cl = np.ascontiguousarray(
            x3c[:, NL:].transpose(2, 0, 1).reshape(3, GPC * NC))
        # dxr rows: (deg+1) per literal, ones; wdq rows: q, cbias_l
        # (g-gate block doubled to match the pre-doubled weights)
        dxr = np.ones((2, GPC * NL), f32)
        for gg in range(GPC):
            dxr[0, gg * NL:(gg + 1) * NL] = deg_l[c * GPC + gg] + 1.0
        wdq = np.stack([q_l, cbias_l]).astype(f32)
        wdq[:, 2 * H:3 * H] *= 2.0
        # pre-chunk adjacency into full-128-row K-chunks; the final chunk
        # overlaps the previous one with its overlap rows zeroed
        atc = np.zeros((GPC, 8, 128, NC), f32)
        ac = np.zeros((GPC, 4, 128, NL), f32)
        for gg in range(GPC):
            at_full = A_in_c[c * GPC + gg].T       # [NL, NC]
            a_full = A_in_l[c * GPC + gg].T        # [NC, NL]
            for x in range(2):
                for j in range(3):
                    atc[gg, 4 * x + j] = at_full[x * 400 + 128 * j:
                                                 x * 400 + 128 * (j + 1)]
                atc[gg, 4 * x + 3, 112:128] = at_full[x * 400 + 384:
                                                      x * 400 + 400]
            for kk in range(3):
                ac[gg, kk] = a_full[128 * kk:128 * (kk + 1)]
            ac[gg, 3, 128 - (NC - 384):] = a_full[384:]
        in_maps.append(dict(
            xt_lit=xt_lit, xt_cl=xt_cl, at_rm=atc, a_rm=ac,
            dxr=dxr, wdq=wdq, **shared,
        ))
    return in_maps


_LAST_RESULTS = {}


def kernel(**inputs):
    from concourse.bass_utils import run_bass_kernel_spmd

    in_maps = _fold_and_shard(inputs)
    if "nc" not in _PROGRAM_CACHE:
        _PROGRAM_CACHE["nc"] = _build_program()
    nc = _PROGRAM_CACHE["nc"]
    res = run_bass_kernel_spmd(nc, in_maps, core_ids=list(range(NCORES)))
    _LAST_RESULTS["res"] = res
    out = np.zeros((N, 1), np.float32)
    for c in range(NCORES):
        vote = res.results[c]["vote"].reshape(GPC, NL)
        for g in range(GPC):
            base = (c * GPC + g) * NPG
            out[base:base + NL, 0] = vote[g]
    return out


# revision 8
# speedup vs baseline: 1.0669x; 1.0070x over previous
"""NeuroSAT GNN message passing on 8 Trainium2 NeuronCores — v2.

Speedups over the v1 graph-data-parallel kernel:
  * All large matmuls run as fp32r (hw-rounded fp32, ~11 mantissa bits) at
    1 cycle/row instead of fp32's 4. Accuracy is restored with a hi/lo
    split: x = hi + lo with hi = round_f32r(x) (free: the producing op
    writes an f32r tile), lo = x - hi. A matmul A@B becomes
    Ah@Bh + Al@Bh + Ah@Bl (dropped lo*lo term is ~2^-24 relative).
    Aggregation matmuls need only 2 terms: the adjacency matrices are
    small integers, exact in f32r.
  * Gate matmuls pair two gates on the 128 output partitions (M=128
    instead of 64), halving streamed rows. The pair-packed PSUM is
    repacked to graph-packed tiles by the sigmoid activations themselves
    (single-input acts may cross partition offsets; 2-input DVE ops may
    not), so the LSTM pointwise stays full-height.
  * The per-literal degree bias (+ lit gate biases) is added once per
    gate-pair psum on DVE; clause gate biases ride the activation bias.
    The g-gate's tanh(x)=2*sigmoid(2x)-1 input doubling is pre-folded
    into the host-side weights/biases, keeping every activation a plain
    table sigmoid.

Layout: per core 2 graphs; feature-major state tiles [128, nodes] with
graph0 on partitions 0:64, graph1 on 64:128, kept in split (hi, lo)
f32r form. Row-major (transposed) hi/lo copies feed the aggregation
matmuls against constant f32r adjacency chunk tiles.
"""

import numpy as np

H = 64
ITERS = 24
B, NV, NC, K = 16, 400, 440, 12
NL = 2 * NV                  # literals/graph = 800
NPG = NL + NC                # nodes/graph = 1240
N = B * NPG                  # 19840
NCORES = 8
GPC = B // NCORES            # graphs per core = 2
CHK = 400                    # literal column chunk (aligned to NV flip halves)

_PROGRAM_CACHE = {}


def _build_program():
    from contextlib import ExitStack

    import concourse.bacc as bacc
    import concourse.mybir as mybir
    from concourse.masks import make_identity
    from concourse.tile import TileContext, add_dep_helper

    F32 = mybir.dt.float32
    F32R = mybir.dt.float32r
    SIG = mybir.ActivationFunctionType.Sigmoid
    MULT = mybir.AluOpType.mult
    SUB = mybir.AluOpType.subtract
    ADD = mybir.AluOpType.add

    nc = bacc.Bacc(
        "TRN2", target_bir_lowering=False, debug=False, num_devices=NCORES
    )

    # ---- DRAM I/O (per-core shards; weights replicated) ----
    d_xt_lit = nc.dram_tensor("xt_lit", [3, GPC * NL], F32, kind="ExternalInput")
    d_xt_cl = nc.dram_tensor("xt_cl", [3, GPC * NC], F32, kind="ExternalInput")
    d_at = nc.dram_tensor("at_rm", [GPC, 8, 128, NC], F32, kind="ExternalInput")
    d_a = nc.dram_tensor("a_rm", [GPC, 4, 128, NL], F32, kind="ExternalInput")
    WNAMES = ("wc_a", "wc_b", "wc_1", "wl_a", "wl_b", "w_lh_dup", "w_cl2_dup")
    d_w = {nm: nc.dram_tensor(nm, [128, 256], F32, kind="ExternalInput")
           for nm in WNAMES}
    d_wv = nc.dram_tensor("wv_dup", [128, 1], F32, kind="ExternalInput")
    d_liw = nc.dram_tensor("li_w3", [3, H], F32, kind="ExternalInput")
    d_ciw = nc.dram_tensor("ci_w3", [3, H], F32, kind="ExternalInput")
    d_bias = nc.dram_tensor("bias_q", [128, 5], F32, kind="ExternalInput")
    d_dxr = nc.dram_tensor("dxr", [2, GPC * NL], F32, kind="ExternalInput")
    d_wdq = nc.dram_tensor("wdq", [2, 256], F32, kind="ExternalInput")
    d_out = nc.dram_tensor("vote", [1, GPC * NL], F32, kind="ExternalOutput")

    with TileContext(nc) as tc, ExitStack() as ctx:
        const = ctx.enter_context(tc.tile_pool(name="const", bufs=1))
        state = ctx.enter_context(tc.tile_pool(name="state", bufs=2))
        work = ctx.enter_context(tc.tile_pool(name="work", bufs=1))
        pstp = ctx.enter_context(tc.tile_pool(name="pstp", bufs=2, space="PSUM"))
        psag = ctx.enter_context(tc.tile_pool(name="psag", bufs=2, space="PSUM"))
        psg = ctx.enter_context(tc.tile_pool(name="psg", bufs=4, space="PSUM"))

        LO, HI = slice(0, 64), slice(64, 128)
        HALF = (LO, HI)

        # ---- constants ----
        ident = const.tile([128, 128], F32, name="ident")
        make_identity(nc, ident)
        identr = const.tile([128, 128], F32R, name="identr")
        nc.scalar.copy(identr[:, :], ident[:, :])

        # adjacency chunks -> f32r const tiles (integers: cvt exact)
        at_r = const.tile([128, GPC * 8 * NC], F32R, name="at_r")
        for g in range(GPC):
            for kk in range(8):
                stg = work.tile([128, NC], F32, tag="ld", bufs=1,
                                name=f"ld_at_{g}_{kk}")
                nc.sync.dma_start(out=stg[:, :], in_=d_at[g, kk])
                c0 = NC * (8 * g + kk)
                nc.scalar.copy(at_r[:, c0:c0 + NC], stg[:, :])
        a_r = const.tile([128, GPC * 4 * NL], F32R, name="a_r")
        for g in range(GPC):
            for kk in range(4):
                stg = work.tile([128, NL], F32, tag="ld", bufs=1,
                                name=f"ld_a_{g}_{kk}")
                nc.sync.dma_start(out=stg[:, :], in_=d_a[g, kk])
                c0 = NL * (4 * g + kk)
                nc.scalar.copy(a_r[:, c0:c0 + NL], stg[:, :])

        # gate weights -> (hi, lo) f32r pairs
        wsp = {}
        for nm in WNAMES:
            stg = work.tile([128, 256], F32, tag="ld", bufs=1, name=f"ldw_{nm}")
            nc.sync.dma_start(out=stg[:, :], in_=d_w[nm][:, :])
            wh = const.tile([128, 256], F32R, name=f"{nm}_h")
            wl = const.tile([128, 256], F32R, name=f"{nm}_l")
            nc.scalar.copy(wh[:, :], stg[:, :])
            nc.vector.tensor_tensor(wl[:, :], stg[:, :], wh[:, :], op=SUB)
            wsp[nm] = (wh, wl)
        stg = work.tile([128, 1], F32, tag="ld", bufs=1, name="ldw_wv")
        nc.sync.dma_start(out=stg[:, :], in_=d_wv[:, :])
        wv_h = const.tile([128, 1], F32R, name="wv_h")
        wv_l = const.tile([128, 1], F32R, name="wv_l")
        nc.scalar.copy(wv_h[:, :], stg[:, :])
        nc.vector.tensor_tensor(wv_l[:, :], stg[:, :], wv_h[:, :], op=SUB)

        def load(dram, shape, nm):
            t = const.tile(shape, F32, name=nm)
            nc.sync.dma_start(out=t[:, :], in_=dram[:, :])
            return t

        xt_lit = load(d_xt_lit, [3, GPC * NL], "xt_lit_sb")
        xt_cl = load(d_xt_cl, [3, GPC * NC], "xt_cl_sb")
        li_w = load(d_liw, [3, H], "li_w_sb")
        ci_w = load(d_ciw, [3, H], "ci_w_sb")
        bias = load(d_bias, [128, 5], "bias_sb")
        stg = work.tile([2, GPC * NL], F32, tag="ld2", bufs=1, name="ld_dxr")
        nc.sync.dma_start(out=stg[:, :], in_=d_dxr[:, :])
        dxr = const.tile([2, GPC * NL], F32R, name="dxr_sb")
        nc.scalar.copy(dxr[:, :], stg[:, :])
        stg = work.tile([2, 256], F32, tag="ld3", bufs=1, name="ld_wdq")
        nc.sync.dma_start(out=stg[:, :], in_=d_wdq[:, :])
        wdq_h = const.tile([2, 256], F32R, name="wdq_h")
        wdq_l = const.tile([2, 256], F32R, name="wdq_l")
        nc.scalar.copy(wdq_h[:, :], stg[:, :])
        nc.vector.tensor_tensor(wdq_l[:, :], stg[:, :], wdq_h[:, :], op=SUB)

        def MM(*a, **kw):
            kw.setdefault("skip_group_check", True)
            return nc.tensor.matmul(*a, **kw)

        # ---- initial node states (bias via ones row of xt) ----
        Lh = state.tile([128, NL], F32R, tag="Lh", name="Lh0")
        Ll = state.tile([128, NL], F32R, tag="Ll", name="Ll0")
        lit_hf_a = state.tile([128, CHK], F32, tag="lit_hf_a", name="lit_hf_a0")
        lit_hf_b = state.tile([128, CHK], F32, tag="lit_hf_b", name="lit_hf_b0")
        for hf in range(2):
            p = psg.tile([128, CHK], F32, tag="g", name=f"ini_{hf}")
            prev = None
            for g in range(GPC):
                mm = MM(p[HALF[g], :], li_w[0:3, :],
                        xt_lit[0:3, g * NL + hf * CHK:g * NL + (hf + 1) * CHK],
                        start=True, stop=True, tile_position=(0, 64 * g))
                if prev is not None:
                    add_dep_helper(mm.ins, prev.ins, sync=True,
                                   reason="psum half order")
                prev = mm
            cs = slice(hf * CHK, (hf + 1) * CHK)
            nc.vector.tensor_copy((lit_hf_a if hf == 0 else lit_hf_b)[:, :],
                                  p[:, :])
            nc.scalar.copy(Lh[:, cs], p[:, :])
            nc.vector.tensor_tensor(Ll[:, cs], p[:, :], Lh[:, cs], op=SUB)
        Chh = state.tile([128, NC], F32R, tag="Chh", name="Chh0")
        Chl = state.tile([128, NC], F32R, tag="Chl", name="Chl0")
        cl_hf = state.tile([128, NC], F32, tag="cl_hf", name="cl_hf0")
        pc = psg.tile([128, NC], F32, tag="g", name="ini_c")
        prev = None
        for g in range(GPC):
            mm = MM(pc[HALF[g], :], ci_w[0:3, :], xt_cl[0:3, g * NC:(g + 1) * NC],
                    start=True, stop=True, tile_position=(0, 64 * g))
            if prev is not None:
                add_dep_helper(mm.ins, prev.ins, sync=True, reason="psum half order")
            prev = mm
        nc.vector.tensor_copy(cl_hf[:, :], pc[:, :])
        nc.scalar.copy(Chh[:, :], pc[:, :])
        nc.vector.tensor_tensor(Chl[:, :], pc[:, :], Chh[:, :], op=SUB)

        lit_c = None
        cl_c = None

        for t in range(1, ITERS):
            first = t == 1

            # ==== clause phase ====
            # dependency-free copies first: clause stack state-halves and the
            # lit-phase flip halves (keeps Pool busy off the critical path)
            st0h = work.tile([128, NC], F32R, tag="st0h", name=f"st0h_{t}")
            st0l = work.tile([128, NC], F32R, tag="st0l", name=f"st0l_{t}")
            st1h = work.tile([128, NC], F32R, tag="st1h", name=f"st1h_{t}")
            st1l = work.tile([128, NC], F32R, tag="st1l", name=f"st1l_{t}")
            nc.scalar.copy(st0h[HI, :], Chh[LO, :])
            nc.scalar.copy(st0l[HI, :], Chl[LO, :])
            nc.gpsimd.tensor_copy(st1h[HI, :], Chh[HI, :])
            nc.gpsimd.tensor_copy(st1l[HI, :], Chl[HI, :])
            lst = []
            for hf in range(2):
                fs = slice((1 - hf) * CHK, (2 - hf) * CHK)
                s0h = work.tile([128, CHK], F32R, tag="s0h", bufs=2, name=f"s0h_{t}_{hf}")
                s0l = work.tile([128, CHK], F32R, tag="s0l", bufs=2, name=f"s0l_{t}_{hf}")
                s1h = work.tile([128, CHK], F32R, tag="s1h", bufs=2, name=f"s1h_{t}_{hf}")
                s1l = work.tile([128, CHK], F32R, tag="s1l", bufs=2, name=f"s1l_{t}_{hf}")
                nc.scalar.copy(s0h[HI, :], Lh[LO, fs])
                nc.scalar.copy(s0l[HI, :], Ll[LO, fs])
                nc.gpsimd.tensor_copy(s1h[HI, :], Lh[HI, fs])
                nc.gpsimd.tensor_copy(s1l[HI, :], Ll[HI, fs])
                lst.append((s0h, s0l, s1h, s1l))

            # transpose lit state per half-grid (chunks 128,128,128,16 per
            # half) so the half-0 transposes + agg terms start while the
            # half-1 pointwise of the previous lit phase is still draining
            GRID = ((0, 128), (128, 128), (256, 128), (272, 128))
            rml = [[None, None], [None, None]]   # rml[g][half] = (rm_h, rm_l)
            for x, src in ((0, lit_hf_a), (1, lit_hf_b)):
                for g in range(GPC):
                    tp = pstp.tile([128, 4 * H], F32, tag="tp",
                                   name=f"tpl_{t}_{g}_{x}")
                    for kk, (c0, sz) in enumerate(GRID):
                        nc.tensor.transpose(
                            tp[:, kk * H:(kk + 1) * H],
                            src[HALF[g], c0:c0 + sz],
                            ident[HALF[g], HALF[g]],
                        )
                    rm_h = work.tile([128, 4 * H], F32R, tag=f"rmlh{g}{x}",
                                     name=f"rmlh_{t}_{g}_{x}")
                    rm_l = work.tile([128, 4 * H], F32R, tag=f"rmll{g}{x}",
                                     name=f"rmll_{t}_{g}_{x}")
                    nc.vector.tensor_copy(rm_h[:, :], tp[:, :])
                    nc.vector.tensor_tensor(rm_l[:, :], tp[:, :], rm_h[:, :],
                                            op=SUB)
                    rml[g][x] = (rm_h, rm_l)

            # clause agg A^T @ L: per-graph psum tiles (g0 rows HI, g1 LO) so
            # the hi/lo term groups interleave without bank conflicts
            agc0 = psag.tile([128, NC], F32, tag="ag", name=f"agc0_{t}")
            agc1 = psag.tile([128, NC], F32, tag="ag", name=f"agc1_{t}")
            agp = (agc0[LO, :], agc1[LO, :])
            for x in range(2):
                for term in range(2):
                    for g in range(GPC):
                        for kk in range(4):
                            c = NC * (8 * g + 4 * x + kk)
                            MM(agp[g],
                               rml[g][x][term][:, kk * H:(kk + 1) * H],
                               at_r[:, c:c + NC],
                               start=(x == 0 and term == 0 and kk == 0),
                               stop=(x == 1 and term == 1 and kk == 3),
                               tile_position=(0, 0))

            # stack agg halves (aligned at LO)
            nc.scalar.copy(st0h[LO, :], agc0[LO, :])
            nc.vector.tensor_tensor(st0l[LO, :], agc0[LO, :], st0h[LO, :], op=SUB)
            nc.scalar.copy(st1h[LO, :], agc1[LO, :])
            nc.vector.tensor_tensor(st1l[LO, :], agc1[LO, :], st1h[LO, :], op=SUB)

            # clause gates: per graph, 2 gate-pairs, 3-term split
            cg = [[None, None], [None, None]]
            for g in range(GPC):
                wnm = "wc_1" if first else "wc_b"
                wh, wl = wsp[wnm]
                sth, stl = (st0h, st0l) if g == 0 else (st1h, st1l)
                for p in range(2):
                    ps_ = slice(p * 128, (p + 1) * 128)
                    gp = psg.tile([128, NC], F32, tag="g", name=f"cg{g}{p}_{t}")
                    MM(gp[:, :], wh[:, ps_], sth[:, :], start=True, stop=False)
                    MM(gp[:, :], wl[:, ps_], sth[:, :], start=False, stop=False)
                    MM(gp[:, :], wh[:, ps_], stl[:, :], start=False, stop=True)
                    cg[g][p] = gp

            # repack sigmoids: pair-psum -> graph-packed s tiles
            s_i = work.tile([128, NC], F32, tag="si", name=f"csi_{t}")
            s_f = work.tile([128, NC], F32, tag="sf", name=f"csf_{t}")
            s_g = work.tile([128, NC], F32, tag="sg", name=f"csg_{t}")
            s_o = work.tile([128, NC], F32, tag="so", name=f"cso_{t}")
            for g in range(GPC):
                h = HALF[g]
                nc.scalar.activation(s_i[h, :], cg[g][0][LO, :], SIG,
                                     bias=bias[h, 0:1])
                nc.scalar.activation(s_g[h, :], cg[g][1][LO, :], SIG,
                                     bias=bias[h, 2:3])
            for g in range(GPC):
                h = HALF[g]
                nc.scalar.activation(s_f[h, :], cg[g][0][HI, :], SIG,
                                     bias=bias[h, 1:2])
                nc.scalar.activation(s_o[h, :], cg[g][1][HI, :], SIG,
                                     bias=bias[h, 3:4])

            # clause LSTM pointwise (graph-packed, full height)
            cc_new = state.tile([128, NC], F32, tag="cl_c", name=f"cc_{t}")
            # pointwise chain column-split across DVE (lo cols) and Pool (hi);
            # tanh realized as tg = 2*sigmoid-1 first so every link is a
            # splittable tensor_scalar/tensor_tensor (no DVE-only stt)
            CSP = ((nc.vector, slice(0, 220)), (nc.gpsimd, slice(220, NC)))
            tg = work.tile([128, NC], F32, tag="t1", name=f"ctg_{t}")
            for eng, q in CSP:
                eng.tensor_scalar(tg[:, q], s_g[:, q], 2.0, -1.0,
                                  op0=MULT, op1=ADD)
            if first:
                for eng, q in CSP:
                    eng.tensor_mul(cc_new[:, q], s_i[:, q], tg[:, q])
            else:
                u = work.tile([128, NC], F32, tag="u", name=f"cu_{t}")
                t2 = work.tile([128, NC], F32, tag="t2", name=f"ct2_{t}")
                for eng, q in CSP:
                    eng.tensor_mul(u[:, q], s_i[:, q], tg[:, q])
                for eng, q in CSP:
                    eng.tensor_mul(t2[:, q], s_f[:, q], cl_c[:, q])
                for eng, q in CSP:
                    eng.tensor_add(cc_new[:, q], u[:, q], t2[:, q])
            tnc = work.tile([128, NC], F32, tag="tnc", name=f"ctn_{t}")
            nc.scalar.activation(tnc[:, 0:220], cc_new[:, 0:220], SIG, scale=2.0)
            nc.scalar.activation(tnc[:, 220:NC], cc_new[:, 220:NC], SIG, scale=2.0)
            th = work.tile([128, NC], F32, tag="t3", name=f"cth_{t}")
            cl_hf_new = state.tile([128, NC], F32, tag="cl_hf", name=f"chf_{t}")
            for eng, q in CSP:
                eng.tensor_scalar(th[:, q], tnc[:, q], 2.0, -1.0,
                                  op0=MULT, op1=ADD)
            for eng, q in CSP:
                eng.tensor_mul(cl_hf_new[:, q], s_o[:, q], th[:, q])
            # split off the critical path (consumers are next iteration)
            Chh_new = state.tile([128, NC], F32R, tag="Chh", name=f"Chh_{t}")
            Chl_new = state.tile([128, NC], F32R, tag="Chl", name=f"Chl_{t}")
            nc.gpsimd.tensor_copy(Chh_new[:, :], cl_hf_new[:, :])
            nc.vector.tensor_tensor(Chl_new[:, :], cl_hf_new[:, :], Chh_new[:, :],
                                    op=SUB)

            # ==== lit phase ====
            # transpose full clause state; split rides the psum copy
            rmc = []
            for g in range(GPC):
                tp = pstp.tile([128, 4 * H], F32, tag="tp", name=f"tpc_{t}_{g}")
                for kk in range(4):
                    c0 = 128 * kk if kk < 3 else NC - 128
                    nc.tensor.transpose(
                        tp[:, kk * H:(kk + 1) * H],
                        cl_hf_new[HALF[g], c0:c0 + 128],
                        ident[HALF[g], HALF[g]],
                    )
                rm_h = work.tile([128, 4 * H], F32R, tag=f"rmch{g}",
                                 name=f"rmch_{t}_{g}")
                rm_l = work.tile([128, 4 * H], F32R, tag=f"rmcl{g}",
                                 name=f"rmcl_{t}_{g}")
                nc.scalar.copy(rm_h[:, :], tp[:, :])
                nc.vector.tensor_tensor(rm_l[:, :], tp[:, :], rm_h[:, :], op=SUB)
                rmc.append((rm_h, rm_l))

            Lh_new = state.tile([128, NL], F32R, tag="Lh", name=f"Lh_{t}")
            Ll_new = state.tile([128, NL], F32R, tag="Ll", name=f"Ll_{t}")
            lit_hf_na = state.tile([128, CHK], F32, tag="lit_hf_a", name=f"lhfa_{t}")
            lit_hf_nb = state.tile([128, CHK], F32, tag="lit_hf_b", name=f"lhfb_{t}")
            lc_new = state.tile([128, NL], F32, tag="lit_c", name=f"lc_{t}")
            wSnm = "w_cl2_dup" if first else "w_lh_dup"
            wSh, wSl = wsp[wSnm]
            for hf in range(2):
                cs = slice(hf * CHK, (hf + 1) * CHK)
                s0h, s0l, s1h, s1l = lst[hf]
                # literal agg A @ C: per-graph psums, interleaved term groups
                agl0 = psag.tile([128, CHK], F32, tag="ag", name=f"agl0_{t}_{hf}")
                agl1 = psag.tile([128, CHK], F32, tag="ag", name=f"agl1_{t}_{hf}")
                agp = (agl0[LO, :], agl1[LO, :])
                for term in range(2):
                    for g in range(GPC):
                        for kk in range(4):
                            MM(agp[g], rmc[g][term][:, kk * H:(kk + 1) * H],
                               a_r[:, NL * (4 * g + kk) + hf * CHK:
                                   NL * (4 * g + kk) + (hf + 1) * CHK],
                               start=(term == 0 and kk == 0),
                               stop=(term == 1 and kk == 3),
                               tile_position=(0, 0))

                # lit gates: dq + rec terms first (no stack dependency -> they
                # fill the PE pipeline while stacks build)
                gps = [[None, None], [None, None]]
                for g in range(GPC):
                    hg = HALF[g]
                    for p in range(2):
                        ps_ = slice(p * 128, (p + 1) * 128)
                        gp = psg.tile([128, CHK], F32, tag="g", name=f"lg{g}{p}_{t}_{hf}")
                        dc = slice(g * NL + hf * CHK, g * NL + (hf + 1) * CHK)
                        MM(gp[:, :], wdq_h[0:2, ps_], dxr[0:2, dc],
                           start=True, stop=False)
                        MM(gp[:, :], wdq_l[0:2, ps_], dxr[0:2, dc],
                           start=False, stop=False)
                        MM(gp[:, :], wSh[hg, ps_], Lh[hg, cs], start=False,
                           stop=False, tile_position=(64 * g, 0))
                        MM(gp[:, :], wSl[hg, ps_], Lh[hg, cs], start=False,
                           stop=False, tile_position=(64 * g, 0))
                        MM(gp[:, :], wSh[hg, ps_], Ll[hg, cs], start=False,
                           stop=False, tile_position=(64 * g, 0))
                        gps[g][p] = gp

                # stack agg halves (aligned at LO)
                nc.scalar.copy(s0h[LO, :], agl0[LO, :])
                nc.vector.tensor_tensor(s0l[LO, :], agl0[LO, :], s0h[LO, :], op=SUB)
                nc.scalar.copy(s1h[LO, :], agl1[LO, :])
                nc.vector.tensor_tensor(s1l[LO, :], agl1[LO, :], s1h[LO, :], op=SUB)

                # stack-dependent gate terms
                for g in range(GPC):
                    wnm = "wl_b"
                    wh, wl = wsp[wnm]
                    sth, stl = (s0h, s0l) if g == 0 else (s1h, s1l)
                    for p in range(2):
                        ps_ = slice(p * 128, (p + 1) * 128)
                        gp = gps[g][p]
                        MM(gp[:, :], wh[:, ps_], sth[:, :], start=False, stop=False)
                        MM(gp[:, :], wl[:, ps_], sth[:, :], start=False, stop=False)
                        MM(gp[:, :], wh[:, ps_], stl[:, :], start=False, stop=True)

                s_i = work.tile([128, CHK], F32, tag="lsi", bufs=2, name=f"lsi_{t}_{hf}")
                s_f = work.tile([128, CHK], F32, tag="lsf", bufs=2, name=f"lsf_{t}_{hf}")
                s_g = work.tile([128, CHK], F32, tag="lsg", bufs=2, name=f"lsg_{t}_{hf}")
                s_o = work.tile([128, CHK], F32, tag="lso", bufs=2, name=f"lso_{t}_{hf}")
                for g in range(GPC):
                    h = HALF[g]
                    nc.scalar.activation(s_i[h, :], gps[g][0][LO, :], SIG)
                    nc.scalar.activation(s_g[h, :], gps[g][1][LO, :], SIG)
                for g in range(GPC):
                    h = HALF[g]
                    nc.scalar.activation(s_f[h, :], gps[g][0][HI, :], SIG)
                    nc.scalar.activation(s_o[h, :], gps[g][1][HI, :], SIG)

                tg = work.tile([128, CHK], F32, tag="lt1", bufs=2, name=f"ltg_{t}_{hf}")
                LSP = ((nc.vector, slice(0, 200)), (nc.gpsimd, slice(200, CHK)))
                cq = [slice(cs.start, cs.start + 200),
                      slice(cs.start + 200, cs.stop)]
                for eng, q in LSP:
                    eng.tensor_scalar(tg[:, q], s_g[:, q], 2.0, -1.0,
                                      op0=MULT, op1=ADD)
                if first:
                    for (eng, q), c2 in zip(LSP, cq):
                        eng.tensor_mul(lc_new[:, c2], s_i[:, q], tg[:, q])
                else:
                    u = work.tile([128, CHK], F32, tag="lu", bufs=2, name=f"lu_{t}_{hf}")
                    t2 = work.tile([128, CHK], F32, tag="lt2", bufs=1, name=f"lt2_{t}_{hf}")
                    for eng, q in LSP:
                        eng.tensor_mul(u[:, q], s_i[:, q], tg[:, q])
                    for (eng, q), c2 in zip(LSP, cq):
                        eng.tensor_mul(t2[:, q], s_f[:, q], lit_c[:, c2])
                    for (eng, q), c2 in zip(LSP, cq):
                        eng.tensor_add(lc_new[:, c2], u[:, q], t2[:, q])
                tnc = work.tile([128, CHK], F32, tag="ltn", bufs=1, name=f"ltn_{t}_{hf}")
                nc.scalar.activation(tnc[:, 0:200],
                                     lc_new[:, cs.start:cs.start + 200],
                                     SIG, scale=2.0)
                nc.scalar.activation(tnc[:, 200:CHK],
                                     lc_new[:, cs.start + 200:cs.stop],
                                     SIG, scale=2.0)
                th = work.tile([128, CHK], F32, tag="lt3", bufs=1, name=f"lth_{t}_{hf}")
                lhf_x = lit_hf_na if hf == 0 else lit_hf_nb
                for eng, q in LSP:
                    eng.tensor_scalar(th[:, q], tnc[:, q], 2.0, -1.0,
                                      op0=MULT, op1=ADD)
                for eng, q in LSP:
                    eng.tensor_mul(lhf_x[:, q], s_o[:, q], th[:, q])
                # split off the critical path (consumers are next iteration)
                nc.gpsimd.tensor_copy(Lh_new[:, cs], lhf_x[:, :])
                nc.vector.tensor_tensor(Ll_new[:, cs], lhf_x[:, :],
                                        Lh_new[:, cs], op=SUB)

            Lh, Ll = Lh_new, Ll_new
            lit_hf_a, lit_hf_b = lit_hf_na, lit_hf_nb
            Chh, Chl, cl_hf = Chh_new, Chl_new, cl_hf_new
            lit_c, cl_c = lc_new, cc_new

        # ---- vote head: (Lh+Ll) @ Wv, 3-term split ----
        for g in range(GPC):
            hg = HALF[g]
            for hf in range(2):
                cs = slice(hf * CHK, (hf + 1) * CHK)
                p = psg.tile([1, CHK], F32, tag="g", name=f"vps_{g}_{hf}")
                MM(p[:, :], wv_h[hg, 0:1], Lh[hg, cs], start=True, stop=False,
                   tile_position=(64 * g, 0))
                MM(p[:, :], wv_l[hg, 0:1], Lh[hg, cs], start=False, stop=False,
                   tile_position=(64 * g, 0))
                MM(p[:, :], wv_h[hg, 0:1], Ll[hg, cs], start=False, stop=True,
                   tile_position=(64 * g, 0))
                vc = work.tile([1, CHK], F32, tag="vote", bufs=1,
                               name=f"vote_{g}_{hf}")
                nc.scalar.activation(
                    vc[:, :], p[:, :], mybir.ActivationFunctionType.Identity,
                    bias=bias[0:1, 4:5],
                )
                nc.sync.dma_start(
                    out=d_out[0:1, g * NL + hf * CHK:g * NL + (hf + 1) * CHK],
                    in_=vc[:, :])

    nc.compile()
    return nc


def _fold_and_shard(inputs):
    """Host-side preprocessing: fold weights, build adjacency, shard by graph."""
    f32 = np.float32
    g = {k: np.asarray(v) for k, v in inputs.items()}

    def collapse(w1, b1, w2, b2, w3, b3):
        return w1 @ w2 @ w3, ((b1 @ w2) + b2) @ w3 + b3

    Wl, bl = collapse(g["lm1_w"], g["lm1_b"], g["lm2_w"], g["lm2_b"],
                      g["lm3_w"], g["lm3_b"])
    Wc, bc = collapse(g["cm1_w"], g["cm1_b"], g["cm2_w"], g["cm2_b"],
                      g["cm3_w"], g["cm3_b"])
    Wv, bv = collapse(g["lv1_w"], g["lv1_b"], g["lv2_w"], g["lv2_b"],
                      g["lv3_w"], g["lv3_b"])

    cu_wih, lu_wih = g["cu_wih"], g["lu_wih"]
    w_lc = (Wl @ cu_wih).astype(f32)                 # agg_c -> clause gates
    w_ch = (w_lc + g["cu_whh"]).astype(f32)          # t>=2 merged recurrent
    cbias_c = ((K + 1) * (bl @ cu_wih) + g["cu_bih"] + g["cu_bhh"]).astype(f32)
    wih_a = lu_wih[0:H].astype(f32)                  # flip -> lit gates
    w_cl2 = (Wc @ lu_wih[H:2 * H]).astype(f32)       # agg_l -> lit gates
    w_lh = (w_cl2 + g["lu_whh"]).astype(f32)         # t>=2 merged recurrent
    q_l = (bc @ lu_wih[H:2 * H]).astype(f32)         # [256]
    cbias_l = (g["lu_bih"] + g["lu_bhh"]).astype(f32)

    def gdouble(w):
        w = w.copy()
        w[:, 2 * H:3 * H] *= 2.0     # g-gate runs as sigmoid(2x+2b)
        return w

    vs = np.vstack
    wc_a = gdouble(vs([w_ch, w_lc]))
    wc_b = gdouble(vs([w_lc, w_ch]))
    wc_1 = gdouble(vs([w_lc, w_lc]))
    wl_a = gdouble(vs([wih_a, w_cl2]))
    wl_b = gdouble(vs([w_cl2, wih_a]))
    w_lh_dup = gdouble(vs([w_lh, w_lh]))
    w_cl2_dup = gdouble(vs([w_cl2, w_cl2]))
    wv_dup = vs([Wv.astype(f32), Wv.astype(f32)])

    bias_q = np.zeros((128, 5), f32)
    for x in range(4):
        scl = 2.0 if x == 2 else 1.0
        bias_q[0:64, x] = scl * cbias_c[x * H:(x + 1) * H]
        bias_q[64:128, x] = scl * cbias_c[x * H:(x + 1) * H]
    bias_q[0, 4] = bv[0]

    li_w3 = np.concatenate([g["li_w"], g["li_b"][None, :]], axis=0).astype(f32)
    ci_w3 = np.concatenate([g["ci_w"], g["ci_b"][None, :]], axis=0).astype(f32)

    # adjacency per graph from edge_index (direction-robust)
    ei = g["edge_index"].astype(np.int64)
    src, dst = ei[0], ei[1]
    src_g, dst_g = src // NPG, dst // NPG
    assert np.all(src_g == dst_g), "edges must be graph-local"
    src_l, dst_l = src % NPG, dst % NPG
    s_lit, d_lit = src_l < NL, dst_l < NL
    A_in_c = np.zeros((B, NC, NL), f32)   # clause <- literal edges
    m = (~d_lit) & s_lit
    np.add.at(A_in_c, (dst_g[m], dst_l[m] - NL, src_l[m]), 1.0)
    A_in_l = np.zeros((B, NL, NC), f32)   # literal <- clause edges
    m = d_lit & (~s_lit)
    np.add.at(A_in_l, (dst_g[m], dst_l[m], src_l[m] - NL), 1.0)
    deg_l = A_in_l.sum(axis=2)            # [B, NL]

    x = g["x"].astype(f32).reshape(B, NPG, 2)
    ones = np.ones((B, NPG, 1), f32)
    x3 = np.concatenate([x, ones], axis=2)        # [B, NPG, 3]

    shared = dict(
        wc_a=wc_a, wc_b=wc_b, wc_1=wc_1, wl_a=wl_a, wl_b=wl_b,
        w_lh_dup=w_lh_dup, w_cl2_dup=w_cl2_dup, wv_dup=wv_dup,
        li_w3=li_w3, ci_w3=ci_w3, bias_q=bias_q,
    )
    in_maps = []
    for c in range(NCORES):
        gs = slice(c * GPC, (c + 1) * GPC)
        x3c = x3[gs]                               # [GPC, NPG, 3]
        xt_lit = np.ascontiguousarray(
            x3c[:, :NL].transpose(2, 0, 1).reshape(3, GPC * NL))
        xt_cl = np.ascontiguousarray(
            x3c[:, NL:].transpose(2, 0, 1).reshape(3, GPC * NC))
        # dxr rows: (deg+1) per literal, ones; wdq rows: q, cbias_l
        # (g-gate block doubled to match the pre-doubled weights)
        dxr = np.ones((2, GPC * NL), f32)
        for gg in range(GPC):
            dxr[0, gg * NL:(gg + 1) * NL] = deg_l[c * GPC + gg] + 1.0
        wdq = np.stack([q_l, cbias_l]).astype(f32)
        wdq[:, 2 * H:3 * H] *= 2.0
        # pre-chunk adjacency into full-128-row K-chunks; the final chunk
        # overlaps the previous one with its overlap rows zeroed
        atc = np.zeros((GPC, 8, 128, NC), f32)
        ac = np.zeros((GPC, 4, 128, NL), f32)
        for gg in range(GPC):
            at_full = A_in_c[c * GPC + gg].T       # [NL, NC]
            a_full = A_in_l[c * GPC + gg].T        # [NC, NL]
            for x in range(2):
                for j in range(3):
                    atc[gg, 4 * x + j] = at_full[x * 400 + 128 * j:
                                                 x * 400 + 128 * (j + 1)]
                atc[gg, 4 * x + 3, 112:128] = at_full[x * 400 + 384:
                                                      x * 400 + 400]
            for kk in range(3):
                ac[gg, kk] = a_full[128 * kk:128 * (kk + 1)]
            ac[gg, 3, 128 - (NC - 384):] = a_full[384:]
        in_maps.append(dict(
            xt_lit=xt_lit, xt_cl=xt_cl, at_rm=atc, a_rm=ac,
            dxr=dxr, wdq=wdq, **shared,
        ))
    return in_maps


_LAST_RESULTS = {}


def kernel(**inputs):
    from concourse.bass_utils import run_bass_kernel_spmd

    in_maps = _fold_and_shard(inputs)
    if "nc" not in _PROGRAM_CACHE:
        _PROGRAM_CACHE["nc"] = _build_program()
    nc = _PROGRAM_CACHE["nc"]
    res = run_bass_kernel_spmd(nc, in_maps, core_ids=list(range(NCORES)))
    _LAST_RESULTS["res"] = res
    out = np.zeros((N, 1), np.float32)
    for c in range(NCORES):
        vote = res.results[c]["vote"].reshape(GPC, NL)
        for g in range(GPC):
            base = (c * GPC + g) * NPG
            out[base:base + NL, 0] = vote[g]
    return out


# revision 9
# speedup vs baseline: 1.0738x; 1.0064x over previous
"""NeuroSAT GNN message passing on 8 Trainium2 NeuronCores — v2.

Speedups over the v1 graph-data-parallel kernel:
  * All large matmuls run as fp32r (hw-rounded fp32, ~11 mantissa bits) at
    1 cycle/row instead of fp32's 4. Accuracy is restored with a hi/lo
    split: x = hi + lo with hi = round_f32r(x) (free: the producing op
    writes an f32r tile), lo = x - hi. A matmul A@B becomes
    Ah@Bh + Al@Bh + Ah@Bl (dropped lo*lo term is ~2^-24 relative).
    Aggregation matmuls need only 2 terms: the adjacency matrices are
    small integers, exact in f32r.
  * Gate matmuls pair two gates on the 128 output partitions (M=128
    instead of 64), halving streamed rows. The pair-packed PSUM is
    repacked to graph-packed tiles by the sigmoid activations themselves
    (single-input acts may cross partition offsets; 2-input DVE ops may
    not), so the LSTM pointwise stays full-height.
  * The per-literal degree bias (+ lit gate biases) is added once per
    gate-pair psum on DVE; clause gate biases ride the activation bias.
    The g-gate's tanh(x)=2*sigmoid(2x)-1 input doubling is pre-folded
    into the host-side weights/biases, keeping every activation a plain
    table sigmoid.

Layout: per core 2 graphs; feature-major state tiles [128, nodes] with
graph0 on partitions 0:64, graph1 on 64:128, kept in split (hi, lo)
f32r form. Row-major (transposed) hi/lo copies feed the aggregation
matmuls against constant f32r adjacency chunk tiles.
"""

import numpy as np

H = 64
ITERS = 24
B, NV, NC, K = 16, 400, 440, 12
NL = 2 * NV                  # literals/graph = 800
NPG = NL + NC                # nodes/graph = 1240
N = B * NPG                  # 19840
NCORES = 8
GPC = B // NCORES            # graphs per core = 2
CHK = 400                    # literal column chunk (aligned to NV flip halves)

_PROGRAM_CACHE = {}


def _build_program():
    from contextlib import ExitStack

    import concourse.bacc as bacc
    import concourse.mybir as mybir
    from concourse.masks import make_identity
    from concourse.tile import TileContext, add_dep_helper

    F32 = mybir.dt.float32
    F32R = mybir.dt.float32r
    SIG = mybir.ActivationFunctionType.Sigmoid
    MULT = mybir.AluOpType.mult
    SUB = mybir.AluOpType.subtract
    ADD = mybir.AluOpType.add

    nc = bacc.Bacc(
        "TRN2", target_bir_lowering=False, debug=False, num_devices=NCORES
    )

    # ---- DRAM I/O (per-core shards; weights replicated) ----
    d_xt_lit = nc.dram_tensor("xt_lit", [3, GPC * NL], F32, kind="ExternalInput")
    d_xt_cl = nc.dram_tensor("xt_cl", [3, GPC * NC], F32, kind="ExternalInput")
    d_at = nc.dram_tensor("at_rm", [GPC, 8, 128, NC], F32, kind="ExternalInput")
    d_a = nc.dram_tensor("a_rm", [GPC, 4, 128, NL], F32, kind="ExternalInput")
    WNAMES = ("wc_a", "wc_b", "wc_1", "wl_a", "wl_b", "w_lh_dup", "w_cl2_dup")
    d_w = {nm: nc.dram_tensor(nm, [128, 256], F32, kind="ExternalInput")
           for nm in WNAMES}
    d_wv = nc.dram_tensor("wv_dup", [128, 1], F32, kind="ExternalInput")
    d_liw = nc.dram_tensor("li_w3", [3, H], F32, kind="ExternalInput")
    d_ciw = nc.dram_tensor("ci_w3", [3, H], F32, kind="ExternalInput")
    d_bias = nc.dram_tensor("bias_q", [128, 5], F32, kind="ExternalInput")
    d_dxr = nc.dram_tensor("dxr", [2, GPC * NL], F32, kind="ExternalInput")
    d_wdq = nc.dram_tensor("wdq", [2, 256], F32, kind="ExternalInput")
    d_out = nc.dram_tensor("vote", [1, GPC * NL], F32, kind="ExternalOutput")

    with TileContext(nc) as tc, ExitStack() as ctx:
        const = ctx.enter_context(tc.tile_pool(name="const", bufs=1))
        state = ctx.enter_context(tc.tile_pool(name="state", bufs=2))
        work = ctx.enter_context(tc.tile_pool(name="work", bufs=1))
        pstp = ctx.enter_context(tc.tile_pool(name="pstp", bufs=2, space="PSUM"))
        psag = ctx.enter_context(tc.tile_pool(name="psag", bufs=2, space="PSUM"))
        psg = ctx.enter_context(tc.tile_pool(name="psg", bufs=4, space="PSUM"))

        LO, HI = slice(0, 64), slice(64, 128)
        HALF = (LO, HI)

        # ---- constants ----
        ident = const.tile([128, 128], F32, name="ident")
        make_identity(nc, ident)
        identr = const.tile([128, 128], F32R, name="identr")
        nc.scalar.copy(identr[:, :], ident[:, :])

        # adjacency chunks -> f32r const tiles (integers: cvt exact)
        at_r = const.tile([128, GPC * 8 * NC], F32R, name="at_r")
        for g in range(GPC):
            for kk in range(8):
                stg = work.tile([128, NC], F32, tag="ld", bufs=1,
                                name=f"ld_at_{g}_{kk}")
                nc.sync.dma_start(out=stg[:, :], in_=d_at[g, kk])
                c0 = NC * (8 * g + kk)
                nc.scalar.copy(at_r[:, c0:c0 + NC], stg[:, :])
        a_r = const.tile([128, GPC * 4 * NL], F32R, name="a_r")
        for g in range(GPC):
            for kk in range(4):
                stg = work.tile([128, NL], F32, tag="ld", bufs=1,
                                name=f"ld_a_{g}_{kk}")
                nc.sync.dma_start(out=stg[:, :], in_=d_a[g, kk])
                c0 = NL * (4 * g + kk)
                nc.scalar.copy(a_r[:, c0:c0 + NL], stg[:, :])

        # gate weights -> (hi, lo) f32r pairs
        wsp = {}
        for nm in WNAMES:
            stg = work.tile([128, 256], F32, tag="ld", bufs=1, name=f"ldw_{nm}")
            nc.sync.dma_start(out=stg[:, :], in_=d_w[nm][:, :])
            wh = const.tile([128, 256], F32R, name=f"{nm}_h")
            wl = const.tile([128, 256], F32R, name=f"{nm}_l")
            nc.scalar.copy(wh[:, :], stg[:, :])
            nc.vector.tensor_tensor(wl[:, :], stg[:, :], wh[:, :], op=SUB)
            wsp[nm] = (wh, wl)
        stg = work.tile([128, 1], F32, tag="ld", bufs=1, name="ldw_wv")
        nc.sync.dma_start(out=stg[:, :], in_=d_wv[:, :])
        wv_h = const.tile([128, 1], F32R, name="wv_h")
        wv_l = const.tile([128, 1], F32R, name="wv_l")
        nc.scalar.copy(wv_h[:, :], stg[:, :])
        nc.vector.tensor_tensor(wv_l[:, :], stg[:, :], wv_h[:, :], op=SUB)

        def load(dram, shape, nm):
            t = const.tile(shape, F32, name=nm)
            nc.sync.dma_start(out=t[:, :], in_=dram[:, :])
            return t

        xt_lit = load(d_xt_lit, [3, GPC * NL], "xt_lit_sb")
        xt_cl = load(d_xt_cl, [3, GPC * NC], "xt_cl_sb")
        li_w = load(d_liw, [3, H], "li_w_sb")
        ci_w = load(d_ciw, [3, H], "ci_w_sb")
        bias = load(d_bias, [128, 5], "bias_sb")
        stg = work.tile([2, GPC * NL], F32, tag="ld2", bufs=1, name="ld_dxr")
        nc.sync.dma_start(out=stg[:, :], in_=d_dxr[:, :])
        dxr = const.tile([2, GPC * NL], F32R, name="dxr_sb")
        nc.scalar.copy(dxr[:, :], stg[:, :])
        stg = work.tile([2, 256], F32, tag="ld3", bufs=1, name="ld_wdq")
        nc.sync.dma_start(out=stg[:, :], in_=d_wdq[:, :])
        wdq_h = const.tile([2, 256], F32R, name="wdq_h")
        wdq_l = const.tile([2, 256], F32R, name="wdq_l")
        nc.scalar.copy(wdq_h[:, :], stg[:, :])
        nc.vector.tensor_tensor(wdq_l[:, :], stg[:, :], wdq_h[:, :], op=SUB)

        def MM(*a, **kw):
            kw.setdefault("skip_group_check", True)
            return nc.tensor.matmul(*a, **kw)

        # ---- initial node states (bias via ones row of xt) ----
        Lh = state.tile([128, NL], F32R, tag="Lh", name="Lh0")
        Ll = state.tile([128, NL], F32R, tag="Ll", name="Ll0")
        lit_hf_a = state.tile([128, CHK], F32, tag="lit_hf_a", name="lit_hf_a0")
        lit_hf_b = state.tile([128, CHK], F32, tag="lit_hf_b", name="lit_hf_b0")
        for hf in range(2):
            p = psg.tile([128, CHK], F32, tag="g", name=f"ini_{hf}")
            prev = None
            for g in range(GPC):
                mm = MM(p[HALF[g], :], li_w[0:3, :],
                        xt_lit[0:3, g * NL + hf * CHK:g * NL + (hf + 1) * CHK],
                        start=True, stop=True, tile_position=(0, 64 * g))
                if prev is not None:
                    add_dep_helper(mm.ins, prev.ins, sync=True,
                                   reason="psum half order")
                prev = mm
            cs = slice(hf * CHK, (hf + 1) * CHK)
            nc.vector.tensor_copy((lit_hf_a if hf == 0 else lit_hf_b)[:, :],
                                  p[:, :])
            nc.scalar.copy(Lh[:, cs], p[:, :])
            nc.vector.tensor_tensor(Ll[:, cs], p[:, :], Lh[:, cs], op=SUB)
        Chh = state.tile([128, NC], F32R, tag="Chh", name="Chh0")
        Chl = state.tile([128, NC], F32R, tag="Chl", name="Chl0")
        cl_hf = state.tile([128, NC], F32, tag="cl_hf", name="cl_hf0")
        pc = psg.tile([128, NC], F32, tag="g", name="ini_c")
        prev = None
        for g in range(GPC):
            mm = MM(pc[HALF[g], :], ci_w[0:3, :], xt_cl[0:3, g * NC:(g + 1) * NC],
                    start=True, stop=True, tile_position=(0, 64 * g))
            if prev is not None:
                add_dep_helper(mm.ins, prev.ins, sync=True, reason="psum half order")
            prev = mm
        nc.vector.tensor_copy(cl_hf[:, :], pc[:, :])
        nc.scalar.copy(Chh[:, :], pc[:, :])
        nc.vector.tensor_tensor(Chl[:, :], pc[:, :], Chh[:, :], op=SUB)

        lit_c = None
        cl_c = None

        for t in range(1, ITERS):
            first = t == 1

            # ==== clause phase ====
            # dependency-free copies first: clause stack state-halves and the
            # lit-phase flip halves (keeps Pool busy off the critical path)
            st0h = work.tile([128, NC], F32R, tag="st0h", name=f"st0h_{t}")
            st0l = work.tile([128, NC], F32R, tag="st0l", name=f"st0l_{t}")
            st1h = work.tile([128, NC], F32R, tag="st1h", name=f"st1h_{t}")
            st1l = work.tile([128, NC], F32R, tag="st1l", name=f"st1l_{t}")
            nc.scalar.copy(st0h[HI, :], Chh[LO, :])
            nc.scalar.copy(st0l[HI, :], Chl[LO, :])
            nc.gpsimd.tensor_copy(st1h[HI, :], Chh[HI, :])
            nc.gpsimd.tensor_copy(st1l[HI, :], Chl[HI, :])
            lst = []
            for hf in range(2):
                fs = slice((1 - hf) * CHK, (2 - hf) * CHK)
                s0h = work.tile([128, CHK], F32R, tag="s0h", bufs=2, name=f"s0h_{t}_{hf}")
                s0l = work.tile([128, CHK], F32R, tag="s0l", bufs=2, name=f"s0l_{t}_{hf}")
                s1h = work.tile([128, CHK], F32R, tag="s1h", bufs=2, name=f"s1h_{t}_{hf}")
                s1l = work.tile([128, CHK], F32R, tag="s1l", bufs=2, name=f"s1l_{t}_{hf}")
                nc.scalar.copy(s0h[HI, :], Lh[LO, fs])
                nc.scalar.copy(s0l[HI, :], Ll[LO, fs])
                nc.gpsimd.tensor_copy(s1h[HI, :], Lh[HI, fs])
                nc.gpsimd.tensor_copy(s1l[HI, :], Ll[HI, fs])
                lst.append((s0h, s0l, s1h, s1l))

            # transpose lit state per half-grid (chunks 128,128,128,16 per
            # half) so the half-0 transposes + agg terms start while the
            # half-1 pointwise of the previous lit phase is still draining
            GRID = ((0, 128), (128, 128), (256, 128), (272, 128))
            rml = [[None, None], [None, None]]   # rml[g][half] = (rm_h, rm_l)
            for x, src in ((0, lit_hf_a), (1, lit_hf_b)):
                for g in range(GPC):
                    tp = pstp.tile([128, 4 * H], F32, tag="tp",
                                   name=f"tpl_{t}_{g}_{x}")
                    for kk, (c0, sz) in enumerate(GRID):
                        nc.tensor.transpose(
                            tp[:, kk * H:(kk + 1) * H],
                            src[HALF[g], c0:c0 + sz],
                            ident[HALF[g], HALF[g]],
                        )
                    rm_h = work.tile([128, 4 * H], F32R, tag=f"rmlh{g}{x}",
                                     name=f"rmlh_{t}_{g}_{x}")
                    rm_l = work.tile([128, 4 * H], F32R, tag=f"rmll{g}{x}",
                                     name=f"rmll_{t}_{g}_{x}")
                    nc.vector.tensor_copy(rm_h[:, :], tp[:, :])
                    nc.vector.tensor_tensor(rm_l[:, :], tp[:, :], rm_h[:, :],
                                            op=SUB)
                    rml[g][x] = (rm_h, rm_l)

            # clause agg A^T @ L: per-graph psum tiles (g0 rows HI, g1 LO) so
            # the hi/lo term groups interleave without bank conflicts
            agc0 = psag.tile([128, NC], F32, tag="ag", name=f"agc0_{t}")
            agc1 = psag.tile([128, NC], F32, tag="ag", name=f"agc1_{t}")
            agp = (agc0[LO, :], agc1[LO, :])
            for x in range(2):
                for term in range(2):
                    for g in range(GPC):
                        for kk in range(4):
                            c = NC * (8 * g + 4 * x + kk)
                            MM(agp[g],
                               rml[g][x][term][:, kk * H:(kk + 1) * H],
                               at_r[:, c:c + NC],
                               start=(x == 0 and term == 0 and kk == 0),
                               stop=(x == 1 and term == 1 and kk == 3),
                               tile_position=(0, 0))

            # stack agg halves (aligned at LO)
            nc.scalar.copy(st0h[LO, :], agc0[LO, :])
            nc.vector.tensor_tensor(st0l[LO, :], agc0[LO, :], st0h[LO, :], op=SUB)
            nc.scalar.copy(st1h[LO, :], agc1[LO, :])
            nc.vector.tensor_tensor(st1l[LO, :], agc1[LO, :], st1h[LO, :], op=SUB)

            # clause gates: per graph, 2 gate-pairs, 3-term split
            cg = [[None, None], [None, None]]
            for g in range(GPC):
                wnm = "wc_1" if first else "wc_b"
                wh, wl = wsp[wnm]
                sth, stl = (st0h, st0l) if g == 0 else (st1h, st1l)
                for p in range(2):
                    ps_ = slice(p * 128, (p + 1) * 128)
                    gp = psg.tile([128, NC], F32, tag="g", name=f"cg{g}{p}_{t}")
                    MM(gp[:, :], wh[:, ps_], sth[:, :], start=True, stop=False)
                    MM(gp[:, :], wl[:, ps_], sth[:, :], start=False, stop=False)
                    MM(gp[:, :], wh[:, ps_], stl[:, :], start=False, stop=True)
                    cg[g][p] = gp

            # repack sigmoids: pair-psum -> graph-packed s tiles
            s_i = work.tile([128, NC], F32, tag="si", name=f"csi_{t}")
            s_f = work.tile([128, NC], F32, tag="sf", name=f"csf_{t}")
            s_g = work.tile([128, NC], F32, tag="sg", name=f"csg_{t}")
            s_o = work.tile([128, NC], F32, tag="so", name=f"cso_{t}")
            nc.scalar.activation(s_i[LO, :], cg[0][0][LO, :], SIG,
                                 bias=bias[LO, 0:1])
            nc.scalar.activation(s_f[LO, :], cg[0][0][HI, :], SIG,
                                 bias=bias[LO, 1:2])
            nc.scalar.activation(s_g[LO, :], cg[0][1][LO, :], SIG,
                                 bias=bias[LO, 2:3])
            nc.scalar.activation(s_i[HI, :], cg[1][0][LO, :], SIG,
                                 bias=bias[HI, 0:1])
            nc.scalar.activation(s_f[HI, :], cg[1][0][HI, :], SIG,
                                 bias=bias[HI, 1:2])
            nc.scalar.activation(s_g[HI, :], cg[1][1][LO, :], SIG,
                                 bias=bias[HI, 2:3])
            nc.scalar.activation(s_o[LO, :], cg[0][1][HI, :], SIG,
                                 bias=bias[LO, 3:4])
            nc.scalar.activation(s_o[HI, :], cg[1][1][HI, :], SIG,
                                 bias=bias[HI, 3:4])

            # clause LSTM pointwise (graph-packed, full height)
            cc_new = state.tile([128, NC], F32, tag="cl_c", name=f"cc_{t}")
            # pointwise chain column-split across DVE (lo cols) and Pool (hi);
            # tanh realized as tg = 2*sigmoid-1 first so every link is a
            # splittable tensor_scalar/tensor_tensor (no DVE-only stt)
            CSP = ((nc.vector, slice(0, 220)), (nc.gpsimd, slice(220, NC)))
            tg = work.tile([128, NC], F32, tag="t1", name=f"ctg_{t}")
            for eng, q in CSP:
                eng.tensor_scalar(tg[:, q], s_g[:, q], 2.0, -1.0,
                                  op0=MULT, op1=ADD)
            if first:
                for eng, q in CSP:
                    eng.tensor_mul(cc_new[:, q], s_i[:, q], tg[:, q])
            else:
                u = work.tile([128, NC], F32, tag="u", name=f"cu_{t}")
                t2 = work.tile([128, NC], F32, tag="t2", name=f"ct2_{t}")
                for eng, q in CSP:
                    eng.tensor_mul(u[:, q], s_i[:, q], tg[:, q])
                for eng, q in CSP:
                    eng.tensor_mul(t2[:, q], s_f[:, q], cl_c[:, q])
                for eng, q in CSP:
                    eng.tensor_add(cc_new[:, q], u[:, q], t2[:, q])
            tnc = work.tile([128, NC], F32, tag="tnc", name=f"ctn_{t}")
            nc.scalar.activation(tnc[:, 0:220], cc_new[:, 0:220], SIG, scale=2.0)
            nc.scalar.activation(tnc[:, 220:NC], cc_new[:, 220:NC], SIG, scale=2.0)
            th = work.tile([128, NC], F32, tag="t3", name=f"cth_{t}")
            cl_hf_new = state.tile([128, NC], F32, tag="cl_hf", name=f"chf_{t}")
            for eng, q in CSP:
                eng.tensor_scalar(th[:, q], tnc[:, q], 2.0, -1.0,
                                  op0=MULT, op1=ADD)
            for eng, q in CSP:
                eng.tensor_mul(cl_hf_new[:, q], s_o[:, q], th[:, q])
            # split off the critical path (consumers are next iteration)
            Chh_new = state.tile([128, NC], F32R, tag="Chh", name=f"Chh_{t}")
            Chl_new = state.tile([128, NC], F32R, tag="Chl", name=f"Chl_{t}")
            nc.gpsimd.tensor_copy(Chh_new[:, :], cl_hf_new[:, :])
            nc.vector.tensor_tensor(Chl_new[:, :], cl_hf_new[:, :], Chh_new[:, :],
                                    op=SUB)

            # ==== lit phase ====
            # transpose full clause state; split rides the psum copy
            rmc = []
            for g in range(GPC):
                tp = pstp.tile([128, 4 * H], F32, tag="tp", name=f"tpc_{t}_{g}")
                for kk in range(4):
                    c0 = 128 * kk if kk < 3 else NC - 128
                    nc.tensor.transpose(
                        tp[:, kk * H:(kk + 1) * H],
                        cl_hf_new[HALF[g], c0:c0 + 128],
                        ident[HALF[g], HALF[g]],
                    )
                rm_h = work.tile([128, 4 * H], F32R, tag=f"rmch{g}",
                                 name=f"rmch_{t}_{g}")
                rm_l = work.tile([128, 4 * H], F32R, tag=f"rmcl{g}",
                                 name=f"rmcl_{t}_{g}")
                nc.scalar.copy(rm_h[:, :], tp[:, :])
                nc.vector.tensor_tensor(rm_l[:, :], tp[:, :], rm_h[:, :], op=SUB)
                rmc.append((rm_h, rm_l))

            Lh_new = state.tile([128, NL], F32R, tag="Lh", name=f"Lh_{t}")
            Ll_new = state.tile([128, NL], F32R, tag="Ll", name=f"Ll_{t}")
            lit_hf_na = state.tile([128, CHK], F32, tag="lit_hf_a", name=f"lhfa_{t}")
            lit_hf_nb = state.tile([128, CHK], F32, tag="lit_hf_b", name=f"lhfb_{t}")
            lc_new = state.tile([128, NL], F32, tag="lit_c", name=f"lc_{t}")
            wSnm = "w_cl2_dup" if first else "w_lh_dup"
            wSh, wSl = wsp[wSnm]
            for hf in range(2):
                cs = slice(hf * CHK, (hf + 1) * CHK)
                s0h, s0l, s1h, s1l = lst[hf]
                # literal agg A @ C: per-graph psums, interleaved term groups
                agl0 = psag.tile([128, CHK], F32, tag="ag", name=f"agl0_{t}_{hf}")
                agl1 = psag.tile([128, CHK], F32, tag="ag", name=f"agl1_{t}_{hf}")
                agp = (agl0[LO, :], agl1[LO, :])
                for term in range(2):
                    for g in range(GPC):
                        for kk in range(4):
                            MM(agp[g], rmc[g][term][:, kk * H:(kk + 1) * H],
                               a_r[:, NL * (4 * g + kk) + hf * CHK:
                                   NL * (4 * g + kk) + (hf + 1) * CHK],
                               start=(term == 0 and kk == 0),
                               stop=(term == 1 and kk == 3),
                               tile_position=(0, 0))

                # lit gates: dq + rec terms first (no stack dependency -> they
                # fill the PE pipeline while stacks build)
                gps = [[None, None], [None, None]]
                for g in range(GPC):
                    hg = HALF[g]
                    for p in range(2):
                        ps_ = slice(p * 128, (p + 1) * 128)
                        gp = psg.tile([128, CHK], F32, tag="g", name=f"lg{g}{p}_{t}_{hf}")
                        dc = slice(g * NL + hf * CHK, g * NL + (hf + 1) * CHK)
                        MM(gp[:, :], wdq_h[0:2, ps_], dxr[0:2, dc],
                           start=True, stop=False)
                        MM(gp[:, :], wdq_l[0:2, ps_], dxr[0:2, dc],
                           start=False, stop=False)
                        MM(gp[:, :], wSh[hg, ps_], Lh[hg, cs], start=False,
                           stop=False, tile_position=(64 * g, 0))
                        MM(gp[:, :], wSl[hg, ps_], Lh[hg, cs], start=False,
                           stop=False, tile_position=(64 * g, 0))
                        MM(gp[:, :], wSh[hg, ps_], Ll[hg, cs], start=False,
                           stop=False, tile_position=(64 * g, 0))
                        gps[g][p] = gp

                # stack agg halves (aligned at LO)
                nc.scalar.copy(s0h[LO, :], agl0[LO, :])
                nc.vector.tensor_tensor(s0l[LO, :], agl0[LO, :], s0h[LO, :], op=SUB)
                nc.scalar.copy(s1h[LO, :], agl1[LO, :])
                nc.vector.tensor_tensor(s1l[LO, :], agl1[LO, :], s1h[LO, :], op=SUB)

                # stack-dependent gate terms
                for g in range(GPC):
                    wnm = "wl_b"
                    wh, wl = wsp[wnm]
                    sth, stl = (s0h, s0l) if g == 0 else (s1h, s1l)
                    for p in range(2):
                        ps_ = slice(p * 128, (p + 1) * 128)
                        gp = gps[g][p]
                        MM(gp[:, :], wh[:, ps_], sth[:, :], start=False, stop=False)
                        MM(gp[:, :], wl[:, ps_], sth[:, :], start=False, stop=False)
                        MM(gp[:, :], wh[:, ps_], stl[:, :], start=False, stop=True)

                s_i = work.tile([128, CHK], F32, tag="lsi", bufs=2, name=f"lsi_{t}_{hf}")
                s_f = work.tile([128, CHK], F32, tag="lsf", bufs=2, name=f"lsf_{t}_{hf}")
                s_g = work.tile([128, CHK], F32, tag="lsg", bufs=2, name=f"lsg_{t}_{hf}")
                s_o = work.tile([128, CHK], F32, tag="lso", bufs=2, name=f"lso_{t}_{hf}")
                nc.scalar.activation(s_i[LO, :], gps[0][0][LO, :], SIG)
                nc.scalar.activation(s_f[LO, :], gps[0][0][HI, :], SIG)
                nc.scalar.activation(s_g[LO, :], gps[0][1][LO, :], SIG)
                nc.scalar.activation(s_i[HI, :], gps[1][0][LO, :], SIG)
                nc.scalar.activation(s_f[HI, :], gps[1][0][HI, :], SIG)
                nc.scalar.activation(s_g[HI, :], gps[1][1][LO, :], SIG)
                nc.scalar.activation(s_o[LO, :], gps[0][1][HI, :], SIG)
                nc.scalar.activation(s_o[HI, :], gps[1][1][HI, :], SIG)

                tg = work.tile([128, CHK], F32, tag="lt1", bufs=2, name=f"ltg_{t}_{hf}")
                LSP = ((nc.vector, slice(0, 200)), (nc.gpsimd, slice(200, CHK)))
                cq = [slice(cs.start, cs.start + 200),
                      slice(cs.start + 200, cs.stop)]
                for eng, q in LSP:
                    eng.tensor_scalar(tg[:, q], s_g[:, q], 2.0, -1.0,
                                      op0=MULT, op1=ADD)
                if first:
                    for (eng, q), c2 in zip(LSP, cq):
                        eng.tensor_mul(lc_new[:, c2], s_i[:, q], tg[:, q])
                else:
                    u = work.tile([128, CHK], F32, tag="lu", bufs=2, name=f"lu_{t}_{hf}")
                    t2 = work.tile([128, CHK], F32, tag="lt2", bufs=1, name=f"lt2_{t}_{hf}")
                    for eng, q in LSP:
                        eng.tensor_mul(u[:, q], s_i[:, q], tg[:, q])
                    for (eng, q), c2 in zip(LSP, cq):
                        eng.tensor_mul(t2[:, q], s_f[:, q], lit_c[:, c2])
                    for (eng, q), c2 in zip(LSP, cq):
                        eng.tensor_add(lc_new[:, c2], u[:, q], t2[:, q])
                tnc = work.tile([128, CHK], F32, tag="ltn", bufs=1, name=f"ltn_{t}_{hf}")
                nc.scalar.activation(tnc[:, 0:200],
                                     lc_new[:, cs.start:cs.start + 200],
                                     SIG, scale=2.0)
                nc.scalar.activation(tnc[:, 200:CHK],
                                     lc_new[:, cs.start + 200:cs.stop],
                                     SIG, scale=2.0)
                th = work.tile([128, CHK], F32, tag="lt3", bufs=1, name=f"lth_{t}_{hf}")
                lhf_x = lit_hf_na if hf == 0 else lit_hf_nb
                for eng, q in LSP:
                    eng.tensor_scalar(th[:, q], tnc[:, q], 2.0, -1.0,
                                      op0=MULT, op1=ADD)
                for eng, q in LSP:
                    eng.tensor_mul(lhf_x[:, q], s_o[:, q], th[:, q])
                # split off the critical path (consumers are next iteration)
                nc.gpsimd.tensor_copy(Lh_new[:, cs], lhf_x[:, :])
                nc.vector.tensor_tensor(Ll_new[:, cs], lhf_x[:, :],
                                        Lh_new[:, cs], op=SUB)

            Lh, Ll = Lh_new, Ll_new
            lit_hf_a, lit_hf_b = lit_hf_na, lit_hf_nb
            Chh, Chl, cl_hf = Chh_new, Chl_new, cl_hf_new
            lit_c, cl_c = lc_new, cc_new

        # ---- vote head: (Lh+Ll) @ Wv, 3-term split ----
        for g in range(GPC):
            hg = HALF[g]
            for hf in range(2):
                cs = slice(hf * CHK, (hf + 1) * CHK)
                p = psg.tile([1, CHK], F32, tag="g", name=f"vps_{g}_{hf}")
                MM(p[:, :], wv_h[hg, 0:1], Lh[hg, cs], start=True, stop=False,
                   tile_position=(64 * g, 0))
                MM(p[:, :], wv_l[hg, 0:1], Lh[hg, cs], start=False, stop=False,
                   tile_position=(64 * g, 0))
                MM(p[:, :], wv_h[hg, 0:1], Ll[hg, cs], start=False, stop=True,
                   tile_position=(64 * g, 0))
                vc = work.tile([1, CHK], F32, tag="vote", bufs=1,
                               name=f"vote_{g}_{hf}")
                nc.scalar.activation(
                    vc[:, :], p[:, :], mybir.ActivationFunctionType.Identity,
                    bias=bias[0:1, 4:5],
                )
                nc.sync.dma_start(
                    out=d_out[0:1, g * NL + hf * CHK:g * NL + (hf + 1) * CHK],
                    in_=vc[:, :])

    nc.compile()
    return nc


def _fold_and_shard(inputs):
    """Host-side preprocessing: fold weights, build adjacency, shard by graph."""
    f32 = np.float32
    g = {k: np.asarray(v) for k, v in inputs.items()}

    def collapse(w1, b1, w2, b2, w3, b3):
        return w1 @ w2 @ w3, ((b1 @ w2) + b2) @ w3 + b3

    Wl, bl = collapse(g["lm1_w"], g["lm1_b"], g["lm2_w"], g["lm2_b"],
                      g["lm3_w"], g["lm3_b"])
    Wc, bc = collapse(g["cm1_w"], g["cm1_b"], g["cm2_w"], g["cm2_b"],
                      g["cm3_w"], g["cm3_b"])
    Wv, bv = collapse(g["lv1_w"], g["lv1_b"], g["lv2_w"], g["lv2_b"],
                      g["lv3_w"], g["lv3_b"])

    cu_wih, lu_wih = g["cu_wih"], g["lu_wih"]
    w_lc = (Wl @ cu_wih).astype(f32)                 # agg_c -> clause gates
    w_ch = (w_lc + g["cu_whh"]).astype(f32)          # t>=2 merged recurrent
    cbias_c = ((K + 1) * (bl @ cu_wih) + g["cu_bih"] + g["cu_bhh"]).astype(f32)
    wih_a = lu_wih[0:H].astype(f32)                  # flip -> lit gates
    w_cl2 = (Wc @ lu_wih[H:2 * H]).astype(f32)       # agg_l -> lit gates
    w_lh = (w_cl2 + g["lu_whh"]).astype(f32)         # t>=2 merged recurrent
    q_l = (bc @ lu_wih[H:2 * H]).astype(f32)         # [256]
    cbias_l = (g["lu_bih"] + g["lu_bhh"]).astype(f32)

    def gdouble(w):
        w = w.copy()
        w[:, 2 * H:3 * H] *= 2.0     # g-gate runs as sigmoid(2x+2b)
        return w

    vs = np.vstack
    wc_a = gdouble(vs([w_ch, w_lc]))
    wc_b = gdouble(vs([w_lc, w_ch]))
    wc_1 = gdouble(vs([w_lc, w_lc]))
    wl_a = gdouble(vs([wih_a, w_cl2]))
    wl_b = gdouble(vs([w_cl2, wih_a]))
    w_lh_dup = gdouble(vs([w_lh, w_lh]))
    w_cl2_dup = gdouble(vs([w_cl2, w_cl2]))
    wv_dup = vs([Wv.astype(f32), Wv.astype(f32)])

    bias_q = np.zeros((128, 5), f32)
    for x in range(4):
        scl = 2.0 if x == 2 else 1.0
        bias_q[0:64, x] = scl * cbias_c[x * H:(x + 1) * H]
        bias_q[64:128, x] = scl * cbias_c[x * H:(x + 1) * H]
    bias_q[0, 4] = bv[0]

    li_w3 = np.concatenate([g["li_w"], g["li_b"][None, :]], axis=0).astype(f32)
    ci_w3 = np.concatenate([g["ci_w"], g["ci_b"][None, :]], axis=0).astype(f32)

    # adjacency per graph from edge_index (direction-robust)
    ei = g["edge_index"].astype(np.int64)
    src, dst = ei[0], ei[1]
    src_g, dst_g = src // NPG, dst // NPG
    assert np.all(src_g == dst_g), "edges must be graph-local"
    src_l, dst_l = src % NPG, dst % NPG
    s_lit, d_lit = src_l < NL, dst_l < NL
    A_in_c = np.zeros((B, NC, NL), f32)   # clause <- literal edges
    m = (~d_lit) & s_lit
    np.add.at(A_in_c, (dst_g[m], dst_l[m] - NL, src_l[m]), 1.0)
    A_in_l = np.zeros((B, NL, NC), f32)   # literal <- clause edges
    m = d_lit & (~s_lit)
    np.add.at(A_in_l, (dst_g[m], dst_l[m], src_l[m] - NL), 1.0)
    deg_l = A_in_l.sum(axis=2)            # [B, NL]

    x = g["x"].astype(f32).reshape(B, NPG, 2)
    ones = np.ones((B, NPG, 1), f32)
    x3 = np.concatenate([x, ones], axis=2)        # [B, NPG, 3]

    shared = dict(
        wc_a=wc_a, wc_b=wc_b, wc_1=wc_1, wl_a=wl_a, wl_b=wl_b,
        w_lh_dup=w_lh_dup, w_cl2_dup=w_cl2_dup, wv_dup=wv_dup,
        li_w3=li_w3, ci_w3=ci_w3, bias_q=bias_q,
    )
    in_maps = []
    for c in range(NCORES):
        gs = slice(c * GPC, (c + 1) * GPC)
        x3c = x3[gs]                               # [GPC, NPG, 3]
        xt_lit = np.ascontiguousarray(
            x3c[:, :NL].transpose(2, 0, 1).reshape(3, GPC * NL))
        xt_cl = np.ascontiguousarray(
            x3c[:, NL:].transpose(2, 0, 1).reshape(3, GPC * NC))
        # dxr rows: (deg+1) per literal, ones; wdq rows: q, cbias_l
        # (g-gate block doubled to match the pre-doubled weights)
        dxr = np.ones((2, GPC * NL), f32)
        for gg in range(GPC):
            dxr[0, gg * NL:(gg + 1) * NL] = deg_l[c * GPC + gg] + 1.0
        wdq = np.stack([q_l, cbias_l]).astype(f32)
        wdq[:, 2 * H:3 * H] *= 2.0
        # pre-chunk adjacency into full-128-row K-chunks; the final chunk
        # overlaps the previous one with its overlap rows zeroed
        atc = np.zeros((GPC, 8, 128, NC), f32)
        ac = np.zeros((GPC, 4, 128, NL), f32)
        for gg in range(GPC):
            at_full = A_in_c[c * GPC + gg].T       # [NL, NC]
            a_full = A_in_l[c * GPC + gg].T        # [NC, NL]
            for x in range(2):
                for j in range(3):
                    atc[gg, 4 * x + j] = at_full[x * 400 + 128 * j:
                                                 x * 400 + 128 * (j + 1)]
                atc[gg, 4 * x + 3, 112:128] = at_full[x * 400 + 384:
                                                      x * 400 + 400]
            for kk in range(3):
                ac[gg, kk] = a_full[128 * kk:128 * (kk + 1)]
            ac[gg, 3, 128 - (NC - 384):] = a_full[384:]
        in_maps.append(dict(
            xt_lit=xt_lit, xt_cl=xt_cl, at_rm=atc, a_rm=ac,
            dxr=dxr, wdq=wdq, **shared,
        ))
    return in_maps


_LAST_RESULTS = {}


def kernel(**inputs):
    from concourse.bass_utils import run_bass_kernel_spmd

    in_maps = _fold_and_shard(inputs)
    if "nc" not in _PROGRAM_CACHE:
        _PROGRAM_CACHE["nc"] = _build_program()
    nc = _PROGRAM_CACHE["nc"]
    res = run_bass_kernel_spmd(nc, in_maps, core_ids=list(range(NCORES)))
    _LAST_RESULTS["res"] = res
    out = np.zeros((N, 1), np.float32)
    for c in range(NCORES):
        vote = res.results[c]["vote"].reshape(GPC, NL)
        for g in range(GPC):
            base = (c * GPC + g) * NPG
            out[base:base + NL, 0] = vote[g]
    return out


# revision 10
# speedup vs baseline: 1.0946x; 1.0195x over previous
"""NeuroSAT GNN message passing on 8 Trainium2 NeuronCores — v2.

Speedups over the v1 graph-data-parallel kernel:
  * All large matmuls run as fp32r (hw-rounded fp32, ~11 mantissa bits) at
    1 cycle/row instead of fp32's 4. Accuracy is restored with a hi/lo
    split: x = hi + lo with hi = round_f32r(x) (free: the producing op
    writes an f32r tile), lo = x - hi. A matmul A@B becomes
    Ah@Bh + Al@Bh + Ah@Bl (dropped lo*lo term is ~2^-24 relative).
    Aggregation matmuls need only 2 terms: the adjacency matrices are
    small integers, exact in f32r.
  * Gate matmuls pair two gates on the 128 output partitions (M=128
    instead of 64), halving streamed rows. The pair-packed PSUM is
    repacked to graph-packed tiles by the sigmoid activations themselves
    (single-input acts may cross partition offsets; 2-input DVE ops may
    not), so the LSTM pointwise stays full-height.
  * The per-literal degree bias (+ lit gate biases) is added once per
    gate-pair psum on DVE; clause gate biases ride the activation bias.
    The g-gate's tanh(x)=2*sigmoid(2x)-1 input doubling is pre-folded
    into the host-side weights/biases, keeping every activation a plain
    table sigmoid.

Layout: per core 2 graphs; feature-major state tiles [128, nodes] with
graph0 on partitions 0:64, graph1 on 64:128, kept in split (hi, lo)
f32r form. Row-major (transposed) hi/lo copies feed the aggregation
matmuls against constant f32r adjacency chunk tiles.
"""

import numpy as np

H = 64
ITERS = 24
B, NV, NC, K = 16, 400, 440, 12
NL = 2 * NV                  # literals/graph = 800
NPG = NL + NC                # nodes/graph = 1240
N = B * NPG                  # 19840
NCORES = 8
GPC = B // NCORES            # graphs per core = 2
CHK = 400                    # literal column chunk (aligned to NV flip halves)

_PROGRAM_CACHE = {}


def _build_program():
    from contextlib import ExitStack

    import concourse.bacc as bacc
    import concourse.mybir as mybir
    from concourse.masks import make_identity
    from concourse.tile import TileContext, add_dep_helper

    F32 = mybir.dt.float32
    F32R = mybir.dt.float32r
    SIG = mybir.ActivationFunctionType.Sigmoid
    MULT = mybir.AluOpType.mult
    SUB = mybir.AluOpType.subtract
    ADD = mybir.AluOpType.add

    nc = bacc.Bacc(
        "TRN2", target_bir_lowering=False, debug=False, num_devices=NCORES
    )

    # ---- DRAM I/O (per-core shards; weights replicated) ----
    d_xt_lit = nc.dram_tensor("xt_lit", [3, GPC * NL], F32, kind="ExternalInput")
    d_xt_cl = nc.dram_tensor("xt_cl", [3, GPC * NC], F32, kind="ExternalInput")
    d_at = nc.dram_tensor("at_rm", [GPC, 8, 128, NC], F32, kind="ExternalInput")
    d_a = nc.dram_tensor("a_rm", [GPC, 4, 128, NL], F32, kind="ExternalInput")
    WNAMES = ("wc_a", "wc_b", "wc_1", "wl_a", "wl_b", "w_lh_dup", "w_cl2_dup")
    d_w = {nm: nc.dram_tensor(nm, [128, 256], F32, kind="ExternalInput")
           for nm in WNAMES}
    d_wv = nc.dram_tensor("wv_dup", [128, 1], F32, kind="ExternalInput")
    d_liw = nc.dram_tensor("li_w3", [3, H], F32, kind="ExternalInput")
    d_ciw = nc.dram_tensor("ci_w3", [3, H], F32, kind="ExternalInput")
    d_bias = nc.dram_tensor("bias_q", [128, 5], F32, kind="ExternalInput")
    d_dxr = nc.dram_tensor("dxr", [2, GPC * NL], F32, kind="ExternalInput")
    d_wdq = nc.dram_tensor("wdq", [2, 256], F32, kind="ExternalInput")
    d_out = nc.dram_tensor("vote", [1, GPC * NL], F32, kind="ExternalOutput")

    with TileContext(nc) as tc, ExitStack() as ctx:
        const = ctx.enter_context(tc.tile_pool(name="const", bufs=1))
        state = ctx.enter_context(tc.tile_pool(name="state", bufs=2))
        work = ctx.enter_context(tc.tile_pool(name="work", bufs=1))
        pstp = ctx.enter_context(tc.tile_pool(name="pstp", bufs=1, space="PSUM"))
        psag = ctx.enter_context(tc.tile_pool(name="psag", bufs=2, space="PSUM"))
        psg = ctx.enter_context(tc.tile_pool(name="psg", bufs=5, space="PSUM"))

        LO, HI = slice(0, 64), slice(64, 128)
        HALF = (LO, HI)

        # ---- constants ----
        ident = const.tile([128, 128], F32, name="ident")
        make_identity(nc, ident)
        identr = const.tile([128, 128], F32R, name="identr")
        nc.scalar.copy(identr[:, :], ident[:, :])

        # adjacency chunks -> f32r const tiles (integers: cvt exact)
        at_r = const.tile([128, GPC * 8 * NC], F32R, name="at_r")
        for g in range(GPC):
            for kk in range(8):
                stg = work.tile([128, NC], F32, tag="ld", bufs=1,
                                name=f"ld_at_{g}_{kk}")
                nc.sync.dma_start(out=stg[:, :], in_=d_at[g, kk])
                c0 = NC * (8 * g + kk)
                nc.scalar.copy(at_r[:, c0:c0 + NC], stg[:, :])
        a_r = const.tile([128, GPC * 4 * NL], F32R, name="a_r")
        for g in range(GPC):
            for kk in range(4):
                stg = work.tile([128, NL], F32, tag="ld", bufs=1,
                                name=f"ld_a_{g}_{kk}")
                nc.sync.dma_start(out=stg[:, :], in_=d_a[g, kk])
                c0 = NL * (4 * g + kk)
                nc.scalar.copy(a_r[:, c0:c0 + NL], stg[:, :])

        # gate weights -> (hi, lo) f32r pairs
        wsp = {}
        for nm in WNAMES:
            stg = work.tile([128, 256], F32, tag="ld", bufs=1, name=f"ldw_{nm}")
            nc.sync.dma_start(out=stg[:, :], in_=d_w[nm][:, :])
            wh = const.tile([128, 256], F32R, name=f"{nm}_h")
            wl = const.tile([128, 256], F32R, name=f"{nm}_l")
            nc.scalar.copy(wh[:, :], stg[:, :])
            nc.vector.tensor_tensor(wl[:, :], stg[:, :], wh[:, :], op=SUB)
            wsp[nm] = (wh, wl)
        stg = work.tile([128, 1], F32, tag="ld", bufs=1, name="ldw_wv")
        nc.sync.dma_start(out=stg[:, :], in_=d_wv[:, :])
        wv_h = const.tile([128, 1], F32R, name="wv_h")
        wv_l = const.tile([128, 1], F32R, name="wv_l")
        nc.scalar.copy(wv_h[:, :], stg[:, :])
        nc.vector.tensor_tensor(wv_l[:, :], stg[:, :], wv_h[:, :], op=SUB)

        def load(dram, shape, nm):
            t = const.tile(shape, F32, name=nm)
            nc.sync.dma_start(out=t[:, :], in_=dram[:, :])
            return t

        xt_lit = load(d_xt_lit, [3, GPC * NL], "xt_lit_sb")
        xt_cl = load(d_xt_cl, [3, GPC * NC], "xt_cl_sb")
        li_w = load(d_liw, [3, H], "li_w_sb")
        ci_w = load(d_ciw, [3, H], "ci_w_sb")
        bias = load(d_bias, [128, 5], "bias_sb")
        stg = work.tile([2, GPC * NL], F32, tag="ld2", bufs=1, name="ld_dxr")
        nc.sync.dma_start(out=stg[:, :], in_=d_dxr[:, :])
        dxr = const.tile([2, GPC * NL], F32R, name="dxr_sb")
        nc.scalar.copy(dxr[:, :], stg[:, :])
        stg = work.tile([2, 256], F32, tag="ld3", bufs=1, name="ld_wdq")
        nc.sync.dma_start(out=stg[:, :], in_=d_wdq[:, :])
        wdq_h = const.tile([2, 256], F32R, name="wdq_h")
        wdq_l = const.tile([2, 256], F32R, name="wdq_l")
        nc.scalar.copy(wdq_h[:, :], stg[:, :])
        nc.vector.tensor_tensor(wdq_l[:, :], stg[:, :], wdq_h[:, :], op=SUB)

        def MM(*a, **kw):
            kw.setdefault("skip_group_check", True)
            return nc.tensor.matmul(*a, **kw)

        # ---- initial node states (bias via ones row of xt) ----
        Lh = state.tile([128, NL], F32R, tag="Lh", name="Lh0")
        Ll = state.tile([128, NL], F32R, tag="Ll", name="Ll0")
        lit_hf_a = state.tile([128, CHK], F32, tag="lit_hf_a", name="lit_hf_a0")
        lit_hf_b = state.tile([128, CHK], F32, tag="lit_hf_b", name="lit_hf_b0")
        for hf in range(2):
            p = psg.tile([128, CHK], F32, tag="g", name=f"ini_{hf}")
            prev = None
            for g in range(GPC):
                mm = MM(p[HALF[g], :], li_w[0:3, :],
                        xt_lit[0:3, g * NL + hf * CHK:g * NL + (hf + 1) * CHK],
                        start=True, stop=True, tile_position=(0, 64 * g))
                if prev is not None:
                    add_dep_helper(mm.ins, prev.ins, sync=True,
                                   reason="psum half order")
                prev = mm
            cs = slice(hf * CHK, (hf + 1) * CHK)
            nc.vector.tensor_copy((lit_hf_a if hf == 0 else lit_hf_b)[:, :],
                                  p[:, :])
            nc.scalar.copy(Lh[:, cs], p[:, :])
            nc.vector.tensor_tensor(Ll[:, cs], p[:, :], Lh[:, cs], op=SUB)
        Chh = state.tile([128, NC], F32R, tag="Chh", name="Chh0")
        Chl = state.tile([128, NC], F32R, tag="Chl", name="Chl0")
        cl_hf = state.tile([128, NC], F32, tag="cl_hf", name="cl_hf0")
        pc = psg.tile([128, NC], F32, tag="g", name="ini_c")
        prev = None
        for g in range(GPC):
            mm = MM(pc[HALF[g], :], ci_w[0:3, :], xt_cl[0:3, g * NC:(g + 1) * NC],
                    start=True, stop=True, tile_position=(0, 64 * g))
            if prev is not None:
                add_dep_helper(mm.ins, prev.ins, sync=True, reason="psum half order")
            prev = mm
        nc.vector.tensor_copy(cl_hf[:, :], pc[:, :])
        nc.scalar.copy(Chh[:, :], pc[:, :])
        nc.vector.tensor_tensor(Chl[:, :], pc[:, :], Chh[:, :], op=SUB)

        lit_c = None
        cl_c = None

        for t in range(1, ITERS):
            first = t == 1

            # ==== clause phase ====
            # dependency-free copies first: clause stack state-halves and the
            # lit-phase flip halves (keeps Pool busy off the critical path)
            st0h = work.tile([128, NC], F32R, tag="st0h", name=f"st0h_{t}")
            st0l = work.tile([128, NC], F32R, tag="st0l", name=f"st0l_{t}")
            st1h = work.tile([128, NC], F32R, tag="st1h", name=f"st1h_{t}")
            st1l = work.tile([128, NC], F32R, tag="st1l", name=f"st1l_{t}")
            nc.scalar.copy(st0h[HI, :], Chh[LO, :])
            nc.scalar.copy(st0l[HI, :], Chl[LO, :])
            nc.gpsimd.tensor_copy(st1h[HI, :], Chh[HI, :])
            nc.gpsimd.tensor_copy(st1l[HI, :], Chl[HI, :])
            lst = []
            for hf in range(2):
                fs = slice((1 - hf) * CHK, (2 - hf) * CHK)
                s0h = work.tile([128, CHK], F32R, tag="s0h", bufs=2, name=f"s0h_{t}_{hf}")
                s0l = work.tile([128, CHK], F32R, tag="s0l", bufs=2, name=f"s0l_{t}_{hf}")
                s1h = work.tile([128, CHK], F32R, tag="s1h", bufs=2, name=f"s1h_{t}_{hf}")
                s1l = work.tile([128, CHK], F32R, tag="s1l", bufs=2, name=f"s1l_{t}_{hf}")
                nc.scalar.copy(s0h[HI, :], Lh[LO, fs])
                nc.scalar.copy(s0l[HI, :], Ll[LO, fs])
                nc.gpsimd.tensor_copy(s1h[HI, :], Lh[HI, fs])
                nc.gpsimd.tensor_copy(s1l[HI, :], Ll[HI, fs])
                lst.append((s0h, s0l, s1h, s1l))

            # transpose lit state per half-grid (chunks 128,128,128,16 per
            # half) so the half-0 transposes + agg terms start while the
            # half-1 pointwise of the previous lit phase is still draining
            GRID = ((0, 128), (128, 128), (256, 128), (272, 128))
            rml = [[None, None], [None, None]]   # rml[g][half] = (rm_h, rm_l)
            for x, src in ((0, lit_hf_a), (1, lit_hf_b)):
                for g in range(GPC):
                    tp = pstp.tile([128, 4 * H], F32, tag="tp",
                                   name=f"tpl_{t}_{g}_{x}")
                    for kk, (c0, sz) in enumerate(GRID):
                        nc.tensor.transpose(
                            tp[:, kk * H:(kk + 1) * H],
                            src[HALF[g], c0:c0 + sz],
                            ident[HALF[g], HALF[g]],
                        )
                    rm_h = work.tile([128, 4 * H], F32R, tag=f"rmlh{g}{x}",
                                     name=f"rmlh_{t}_{g}_{x}")
                    rm_l = work.tile([128, 4 * H], F32R, tag=f"rmll{g}{x}",
                                     name=f"rmll_{t}_{g}_{x}")
                    nc.vector.tensor_copy(rm_h[:, :], tp[:, :])
                    nc.vector.tensor_tensor(rm_l[:, :], tp[:, :], rm_h[:, :],
                                            op=SUB)
                    rml[g][x] = (rm_h, rm_l)

            # clause agg A^T @ L: per-graph psum tiles (g0 rows HI, g1 LO) so
            # the hi/lo term groups interleave without bank conflicts
            agc0 = psag.tile([128, NC], F32, tag="ag", name=f"agc0_{t}")
            agc1 = psag.tile([128, NC], F32, tag="ag", name=f"agc1_{t}")
            agp = (agc0[LO, :], agc1[LO, :])
            for x in range(2):
                for term in range(2):
                    for g in range(GPC):
                        for kk in range(4):
                            c = NC * (8 * g + 4 * x + kk)
                            MM(agp[g],
                               rml[g][x][term][:, kk * H:(kk + 1) * H],
                               at_r[:, c:c + NC],
                               start=(x == 0 and term == 0 and kk == 0),
                               stop=(x == 1 and term == 1 and kk == 3),
                               tile_position=(0, 0))

            # stack agg halves (aligned at LO)
            nc.scalar.copy(st0h[LO, :], agc0[LO, :])
            nc.vector.tensor_tensor(st0l[LO, :], agc0[LO, :], st0h[LO, :], op=SUB)
            nc.scalar.copy(st1h[LO, :], agc1[LO, :])
            nc.vector.tensor_tensor(st1l[LO, :], agc1[LO, :], st1h[LO, :], op=SUB)

            # clause gates: per graph, 2 gate-pairs, 3-term split
            cg = [[None, None], [None, None]]
            for g in range(GPC):
                wnm = "wc_1" if first else "wc_b"
                wh, wl = wsp[wnm]
                sth, stl = (st0h, st0l) if g == 0 else (st1h, st1l)
                for p in range(2):
                    ps_ = slice(p * 128, (p + 1) * 128)
                    gp = psg.tile([128, NC], F32, tag="g", name=f"cg{g}{p}_{t}")
                    MM(gp[:, :], wh[:, ps_], sth[:, :], start=True, stop=False)
                    MM(gp[:, :], wl[:, ps_], sth[:, :], start=False, stop=False)
                    MM(gp[:, :], wh[:, ps_], stl[:, :], start=False, stop=True)
                    cg[g][p] = gp

            # repack sigmoids: pair-psum -> graph-packed s tiles
            s_i = work.tile([128, NC], F32, tag="si", name=f"csi_{t}")
            s_f = work.tile([128, NC], F32, tag="sf", name=f"csf_{t}")
            s_g = work.tile([128, NC], F32, tag="sg", name=f"csg_{t}")
            s_o = work.tile([128, NC], F32, tag="so", name=f"cso_{t}")
            nc.scalar.activation(s_i[LO, :], cg[0][0][LO, :], SIG,
                                 bias=bias[LO, 0:1])
            nc.scalar.activation(s_f[LO, :], cg[0][0][HI, :], SIG,
                                 bias=bias[LO, 1:2])
            nc.scalar.activation(s_g[LO, :], cg[0][1][LO, :], SIG,
                                 bias=bias[LO, 2:3])
            nc.scalar.activation(s_i[HI, :], cg[1][0][LO, :], SIG,
                                 bias=bias[HI, 0:1])
            nc.scalar.activation(s_f[HI, :], cg[1][0][HI, :], SIG,
                                 bias=bias[HI, 1:2])
            nc.scalar.activation(s_g[HI, :], cg[1][1][LO, :], SIG,
                                 bias=bias[HI, 2:3])
            nc.scalar.activation(s_o[LO, :], cg[0][1][HI, :], SIG,
                                 bias=bias[LO, 3:4])
            nc.scalar.activation(s_o[HI, :], cg[1][1][HI, :], SIG,
                                 bias=bias[HI, 3:4])

            # clause LSTM pointwise (graph-packed, full height)
            cc_new = state.tile([128, NC], F32, tag="cl_c", name=f"cc_{t}")
            # pointwise chain column-split across DVE (lo cols) and Pool (hi);
            # tanh realized as tg = 2*sigmoid-1 first so every link is a
            # splittable tensor_scalar/tensor_tensor (no DVE-only stt)
            CSP = ((nc.vector, slice(0, 220)), (nc.gpsimd, slice(220, NC)))
            tg = work.tile([128, NC], F32, tag="t1", name=f"ctg_{t}")
            for eng, q in CSP:
                eng.tensor_scalar(tg[:, q], s_g[:, q], 2.0, -1.0,
                                  op0=MULT, op1=ADD)
            if first:
                for eng, q in CSP:
                    eng.tensor_mul(cc_new[:, q], s_i[:, q], tg[:, q])
            else:
                u = work.tile([128, NC], F32, tag="u", name=f"cu_{t}")
                t2 = work.tile([128, NC], F32, tag="t2", name=f"ct2_{t}")
                for eng, q in CSP:
                    eng.tensor_mul(u[:, q], s_i[:, q], tg[:, q])
                for eng, q in CSP:
                    eng.tensor_mul(t2[:, q], s_f[:, q], cl_c[:, q])
                for eng, q in CSP:
                    eng.tensor_add(cc_new[:, q], u[:, q], t2[:, q])
            tnc = work.tile([128, NC], F32, tag="tnc", name=f"ctn_{t}")
            nc.scalar.activation(tnc[:, 0:220], cc_new[:, 0:220], SIG, scale=2.0)
            nc.scalar.activation(tnc[:, 220:NC], cc_new[:, 220:NC], SIG, scale=2.0)
            th = work.tile([128, NC], F32, tag="t3", name=f"cth_{t}")
            cl_hf_new = state.tile([128, NC], F32, tag="cl_hf", name=f"chf_{t}")
            for eng, q in CSP:
                eng.tensor_scalar(th[:, q], tnc[:, q], 2.0, -1.0,
                                  op0=MULT, op1=ADD)
            for eng, q in CSP:
                eng.tensor_mul(cl_hf_new[:, q], s_o[:, q], th[:, q])
            # split off the critical path (consumers are next iteration)
            Chh_new = state.tile([128, NC], F32R, tag="Chh", name=f"Chh_{t}")
            Chl_new = state.tile([128, NC], F32R, tag="Chl", name=f"Chl_{t}")
            nc.gpsimd.tensor_copy(Chh_new[:, :], cl_hf_new[:, :])
            nc.vector.tensor_tensor(Chl_new[:, :], cl_hf_new[:, :], Chh_new[:, :],
                                    op=SUB)

            # ==== lit phase ====
            # transpose full clause state; split rides the psum copy
            rmc = []
            for g in range(GPC):
                tp = pstp.tile([128, 4 * H], F32, tag="tp", name=f"tpc_{t}_{g}")
                for kk in range(4):
                    c0 = 128 * kk if kk < 3 else NC - 128
                    nc.tensor.transpose(
                        tp[:, kk * H:(kk + 1) * H],
                        cl_hf_new[HALF[g], c0:c0 + 128],
                        ident[HALF[g], HALF[g]],
                    )
                rm_h = work.tile([128, 4 * H], F32R, tag=f"rmch{g}",
                                 name=f"rmch_{t}_{g}")
                rm_l = work.tile([128, 4 * H], F32R, tag=f"rmcl{g}",
                                 name=f"rmcl_{t}_{g}")
                nc.scalar.copy(rm_h[:, :], tp[:, :])
                nc.vector.tensor_tensor(rm_l[:, :], tp[:, :], rm_h[:, :], op=SUB)
                rmc.append((rm_h, rm_l))

            Lh_new = state.tile([128, NL], F32R, tag="Lh", name=f"Lh_{t}")
            Ll_new = state.tile([128, NL], F32R, tag="Ll", name=f"Ll_{t}")
            lit_hf_na = state.tile([128, CHK], F32, tag="lit_hf_a", name=f"lhfa_{t}")
            lit_hf_nb = state.tile([128, CHK], F32, tag="lit_hf_b", name=f"lhfb_{t}")
            lc_new = state.tile([128, NL], F32, tag="lit_c", name=f"lc_{t}")
            wSnm = "w_cl2_dup" if first else "w_lh_dup"
            wSh, wSl = wsp[wSnm]
            for hf in range(2):
                cs = slice(hf * CHK, (hf + 1) * CHK)
                s0h, s0l, s1h, s1l = lst[hf]
                # literal agg A @ C: per-graph psums, interleaved term groups
                agl0 = psag.tile([128, CHK], F32, tag="ag", name=f"agl0_{t}_{hf}")
                agl1 = psag.tile([128, CHK], F32, tag="ag", name=f"agl1_{t}_{hf}")
                agp = (agl0[LO, :], agl1[LO, :])
                for term in range(2):
                    for g in range(GPC):
                        for kk in range(4):
                            MM(agp[g], rmc[g][term][:, kk * H:(kk + 1) * H],
                               a_r[:, NL * (4 * g + kk) + hf * CHK:
                                   NL * (4 * g + kk) + (hf + 1) * CHK],
                               start=(term == 0 and kk == 0),
                               stop=(term == 1 and kk == 3),
                               tile_position=(0, 0))

                # lit gates: dq + rec terms first (no stack dependency -> they
                # fill the PE pipeline while stacks build)
                gps = [[None, None], [None, None]]
                for g in range(GPC):
                    hg = HALF[g]
                    for p in range(2):
                        ps_ = slice(p * 128, (p + 1) * 128)
                        gp = psg.tile([128, CHK], F32, tag="g", name=f"lg{g}{p}_{t}_{hf}")
                        dc = slice(g * NL + hf * CHK, g * NL + (hf + 1) * CHK)
                        MM(gp[:, :], wdq_h[0:2, ps_], dxr[0:2, dc],
                           start=True, stop=False)
                        MM(gp[:, :], wdq_l[0:2, ps_], dxr[0:2, dc],
                           start=False, stop=False)
                        MM(gp[:, :], wSh[hg, ps_], Lh[hg, cs], start=False,
                           stop=False, tile_position=(64 * g, 0))
                        MM(gp[:, :], wSl[hg, ps_], Lh[hg, cs], start=False,
                           stop=False, tile_position=(64 * g, 0))
                        MM(gp[:, :], wSh[hg, ps_], Ll[hg, cs], start=False,
                           stop=False, tile_position=(64 * g, 0))
                        gps[g][p] = gp

                # stack agg halves (aligned at LO)
                nc.scalar.copy(s0h[LO, :], agl0[LO, :])
                nc.vector.tensor_tensor(s0l[LO, :], agl0[LO, :], s0h[LO, :], op=SUB)
                nc.scalar.copy(s1h[LO, :], agl1[LO, :])
                nc.vector.tensor_tensor(s1l[LO, :], agl1[LO, :], s1h[LO, :], op=SUB)

                # stack-dependent gate terms
                for g in range(GPC):
                    wnm = "wl_b"
                    wh, wl = wsp[wnm]
                    sth, stl = (s0h, s0l) if g == 0 else (s1h, s1l)
                    for p in range(2):
                        ps_ = slice(p * 128, (p + 1) * 128)
                        gp = gps[g][p]
                        MM(gp[:, :], wh[:, ps_], sth[:, :], start=False, stop=False)
                        MM(gp[:, :], wl[:, ps_], sth[:, :], start=False, stop=False)
                        MM(gp[:, :], wh[:, ps_], stl[:, :], start=False, stop=True)

                s_i = work.tile([128, CHK], F32, tag="lsi", bufs=2, name=f"lsi_{t}_{hf}")
                s_f = work.tile([128, CHK], F32, tag="lsf", bufs=2, name=f"lsf_{t}_{hf}")
                s_g = work.tile([128, CHK], F32, tag="lsg", bufs=2, name=f"lsg_{t}_{hf}")
                s_o = work.tile([128, CHK], F32, tag="lso", bufs=2, name=f"lso_{t}_{hf}")
                nc.scalar.activation(s_i[LO, :], gps[0][0][LO, :], SIG)
                nc.scalar.activation(s_f[LO, :], gps[0][0][HI, :], SIG)
                nc.scalar.activation(s_g[LO, :], gps[0][1][LO, :], SIG)
                nc.scalar.activation(s_i[HI, :], gps[1][0][LO, :], SIG)
                nc.scalar.activation(s_f[HI, :], gps[1][0][HI, :], SIG)
                nc.scalar.activation(s_g[HI, :], gps[1][1][LO, :], SIG)
                nc.scalar.activation(s_o[LO, :], gps[0][1][HI, :], SIG)
                nc.scalar.activation(s_o[HI, :], gps[1][1][HI, :], SIG)

                tg = work.tile([128, CHK], F32, tag="lt1", bufs=2, name=f"ltg_{t}_{hf}")
                LSP = ((nc.vector, slice(0, 200)), (nc.gpsimd, slice(200, CHK)))
                cq = [slice(cs.start, cs.start + 200),
                      slice(cs.start + 200, cs.stop)]
                for eng, q in LSP:
                    eng.tensor_scalar(tg[:, q], s_g[:, q], 2.0, -1.0,
                                      op0=MULT, op1=ADD)
                if first:
                    for (eng, q), c2 in zip(LSP, cq):
                        eng.tensor_mul(lc_new[:, c2], s_i[:, q], tg[:, q])
                else:
                    u = work.tile([128, CHK], F32, tag="lu", bufs=2, name=f"lu_{t}_{hf}")
                    t2 = work.tile([128, CHK], F32, tag="lt2", bufs=1, name=f"lt2_{t}_{hf}")
                    for eng, q in LSP:
                        eng.tensor_mul(u[:, q], s_i[:, q], tg[:, q])
                    for (eng, q), c2 in zip(LSP, cq):
                        eng.tensor_mul(t2[:, q], s_f[:, q], lit_c[:, c2])
                    for (eng, q), c2 in zip(LSP, cq):
                        eng.tensor_add(lc_new[:, c2], u[:, q], t2[:, q])
                tnc = work.tile([128, CHK], F32, tag="ltn", bufs=1, name=f"ltn_{t}_{hf}")
                nc.scalar.activation(tnc[:, 0:200],
                                     lc_new[:, cs.start:cs.start + 200],
                                     SIG, scale=2.0)
                nc.scalar.activation(tnc[:, 200:CHK],
                                     lc_new[:, cs.start + 200:cs.stop],
                                     SIG, scale=2.0)
                th = work.tile([128, CHK], F32, tag="lt3", bufs=1, name=f"lth_{t}_{hf}")
                lhf_x = lit_hf_na if hf == 0 else lit_hf_nb
                for eng, q in LSP:
                    eng.tensor_scalar(th[:, q], tnc[:, q], 2.0, -1.0,
                                      op0=MULT, op1=ADD)
                for eng, q in LSP:
                    eng.tensor_mul(lhf_x[:, q], s_o[:, q], th[:, q])
                # split off the critical path (consumers are next iteration)
                nc.gpsimd.tensor_copy(Lh_new[:, cs], lhf_x[:, :])
                nc.vector.tensor_tensor(Ll_new[:, cs], lhf_x[:, :],
                                        Lh_new[:, cs], op=SUB)

            Lh, Ll = Lh_new, Ll_new
            lit_hf_a, lit_hf_b = lit_hf_na, lit_hf_nb
            Chh, Chl, cl_hf = Chh_new, Chl_new, cl_hf_new
            lit_c, cl_c = lc_new, cc_new

        # ---- vote head: (Lh+Ll) @ Wv, 3-term split ----
        for g in range(GPC):
            hg = HALF[g]
            for hf in range(2):
                cs = slice(hf * CHK, (hf + 1) * CHK)
                p = psg.tile([1, CHK], F32, tag="g", name=f"vps_{g}_{hf}")
                MM(p[:, :], wv_h[hg, 0:1], Lh[hg, cs], start=True, stop=False,
                   tile_position=(64 * g, 0))
                MM(p[:, :], wv_l[hg, 0:1], Lh[hg, cs], start=False, stop=False,
                   tile_position=(64 * g, 0))
                MM(p[:, :], wv_h[hg, 0:1], Ll[hg, cs], start=False, stop=True,
                   tile_position=(64 * g, 0))
                vc = work.tile([1, CHK], F32, tag="vote", bufs=1,
                               name=f"vote_{g}_{hf}")
                nc.scalar.activation(
                    vc[:, :], p[:, :], mybir.ActivationFunctionType.Identity,
                    bias=bias[0:1, 4:5],
                )
                nc.sync.dma_start(
                    out=d_out[0:1, g * NL + hf * CHK:g * NL + (hf + 1) * CHK],
                    in_=vc[:, :])

    nc.compile()
    return nc


def _fold_and_shard(inputs):
    """Host-side preprocessing: fold weights, build adjacency, shard by graph."""
    f32 = np.float32
    g = {k: np.asarray(v) for k, v in inputs.items()}

    def collapse(w1, b1, w2, b2, w3, b3):
        return w1 @ w2 @ w3, ((b1 @ w2) + b2) @ w3 + b3

    Wl, bl = collapse(g["lm1_w"], g["lm1_b"], g["lm2_w"], g["lm2_b"],
                      g["lm3_w"], g["lm3_b"])
    Wc, bc = collapse(g["cm1_w"], g["cm1_b"], g["cm2_w"], g["cm2_b"],
                      g["cm3_w"], g["cm3_b"])
    Wv, bv = collapse(g["lv1_w"], g["lv1_b"], g["lv2_w"], g["lv2_b"],
                      g["lv3_w"], g["lv3_b"])

    cu_wih, lu_wih = g["cu_wih"], g["lu_wih"]
    w_lc = (Wl @ cu_wih).astype(f32)                 # agg_c -> clause gates
    w_ch = (w_lc + g["cu_whh"]).astype(f32)          # t>=2 merged recurrent
    cbias_c = ((K + 1) * (bl @ cu_wih) + g["cu_bih"] + g["cu_bhh"]).astype(f32)
    wih_a = lu_wih[0:H].astype(f32)                  # flip -> lit gates
    w_cl2 = (Wc @ lu_wih[H:2 * H]).astype(f32)       # agg_l -> lit gates
    w_lh = (w_cl2 + g["lu_whh"]).astype(f32)         # t>=2 merged recurrent
    q_l = (bc @ lu_wih[H:2 * H]).astype(f32)         # [256]
    cbias_l = (g["lu_bih"] + g["lu_bhh"]).astype(f32)

    def gdouble(w):
        w = w.copy()
        w[:, 2 * H:3 * H] *= 2.0     # g-gate runs as sigmoid(2x+2b)
        return w

    vs = np.vstack
    wc_a = gdouble(vs([w_ch, w_lc]))
    wc_b = gdouble(vs([w_lc, w_ch]))
    wc_1 = gdouble(vs([w_lc, w_lc]))
    wl_a = gdouble(vs([wih_a, w_cl2]))
    wl_b = gdouble(vs([w_cl2, wih_a]))
    w_lh_dup = gdouble(vs([w_lh, w_lh]))
    w_cl2_dup = gdouble(vs([w_cl2, w_cl2]))
    wv_dup = vs([Wv.astype(f32), Wv.astype(f32)])

    bias_q = np.zeros((128, 5), f32)
    for x in range(4):
        scl = 2.0 if x == 2 else 1.0
        bias_q[0:64, x] = scl * cbias_c[x * H:(x + 1) * H]
        bias_q[64:128, x] = scl * cbias_c[x * H:(x + 1) * H]
    bias_q[0, 4] = bv[0]

    li_w3 = np.concatenate([g["li_w"], g["li_b"][None, :]], axis=0).astype(f32)
    ci_w3 = np.concatenate([g["ci_w"], g["ci_b"][None, :]], axis=0).astype(f32)

    # adjacency per graph from edge_index (direction-robust)
    ei = g["edge_index"].astype(np.int64)
    src, dst = ei[0], ei[1]
    src_g, dst_g = src // NPG, dst // NPG
    assert np.all(src_g == dst_g), "edges must be graph-local"
    src_l, dst_l = src % NPG, dst % NPG
    s_lit, d_lit = src_l < NL, dst_l < NL
    A_in_c = np.zeros((B, NC, NL), f32)   # clause <- literal edges
    m = (~d_lit) & s_lit
    np.add.at(A_in_c, (dst_g[m], dst_l[m] - NL, src_l[m]), 1.0)
    A_in_l = np.zeros((B, NL, NC), f32)   # literal <- clause edges
    m = d_lit & (~s_lit)
    np.add.at(A_in_l, (dst_g[m], dst_l[m], src_l[m] - NL), 1.0)
    deg_l = A_in_l.sum(axis=2)            # [B, NL]

    x = g["x"].astype(f32).reshape(B, NPG, 2)
    ones = np.ones((B, NPG, 1), f32)
    x3 = np.concatenate([x, ones], axis=2)        # [B, NPG, 3]

    shared = dict(
        wc_a=wc_a, wc_b=wc_b, wc_1=wc_1, wl_a=wl_a, wl_b=wl_b,
        w_lh_dup=w_lh_dup, w_cl2_dup=w_cl2_dup, wv_dup=wv_dup,
        li_w3=li_w3, ci_w3=ci_w3, bias_q=bias_q,
    )
    in_maps = []
    for c in range(NCORES):
        gs = slice(c * GPC, (c + 1) * GPC)
        x3c = x3[gs]                               # [GPC, NPG, 3]
        xt_lit = np.ascontiguousarray(
            x3c[:, :NL].transpose(2, 0, 1).reshape(3, GPC * NL))
        xt_cl = np.ascontiguousarray(
            x3c[:, NL:].transpose(2, 0, 1).reshape(3, GPC * NC))
        # dxr rows: (deg+1) per literal, ones; wdq rows: q, cbias_l
        # (g-gate block doubled to match the pre-doubled weights)
        dxr = np.ones((2, GPC * NL), f32)
        for gg in range(GPC):
            dxr[0, gg * NL:(gg + 1) * NL] = deg_l[c * GPC + gg] + 1.0
        wdq = np.stack([q_l, cbias_l]).astype(f32)
        wdq[:, 2 * H:3 * H] *= 2.0
        # pre-chunk adjacency into full-128-row K-chunks; the final chunk
        # overlaps the previous one with its overlap rows zeroed
        atc = np.zeros((GPC, 8, 128, NC), f32)
        ac = np.zeros((GPC, 4, 128, NL), f32)
        for gg in range(GPC):
            at_full = A_in_c[c * GPC + gg].T       # [NL, NC]
            a_full = A_in_l[c * GPC + gg].T        # [NC, NL]
            for x in range(2):
                for j in range(3):
                    atc[gg, 4 * x + j] = at_full[x * 400 + 128 * j:
                                                 x * 400 + 128 * (j + 1)]
                atc[gg, 4 * x + 3, 112:128] = at_full[x * 400 + 384:
                                                      x * 400 + 400]
            for kk in range(3):
                ac[gg, kk] = a_full[128 * kk:128 * (kk + 1)]
            ac[gg, 3, 128 - (NC - 384):] = a_full[384:]
        in_maps.append(dict(
            xt_lit=xt_lit, xt_cl=xt_cl, at_rm=atc, a_rm=ac,
            dxr=dxr, wdq=wdq, **shared,
        ))
    return in_maps


_LAST_RESULTS = {}


def kernel(**inputs):
    from concourse.bass_utils import run_bass_kernel_spmd

    in_maps = _fold_and_shard(inputs)
    if "nc" not in _PROGRAM_CACHE:
        _PROGRAM_CACHE["nc"] = _build_program()
    nc = _PROGRAM_CACHE["nc"]
    res = run_bass_kernel_spmd(nc, in_maps, core_ids=list(range(NCORES)))
    _LAST_RESULTS["res"] = res
    out = np.zeros((N, 1), np.float32)
    for c in range(NCORES):
        vote = res.results[c]["vote"].reshape(GPC, NL)
        for g in range(GPC):
            base = (c * GPC + g) * NPG
            out[base:base + NL, 0] = vote[g]
    return out


# revision 11
# speedup vs baseline: 1.1725x; 1.0711x over previous
"""NeuroSAT GNN message passing on 8 Trainium2 NeuronCores — v2.

Speedups over the v1 graph-data-parallel kernel:
  * All large matmuls run as fp32r (hw-rounded fp32, ~11 mantissa bits) at
    1 cycle/row instead of fp32's 4. Accuracy is restored with a hi/lo
    split: x = hi + lo with hi = round_f32r(x) (free: the producing op
    writes an f32r tile), lo = x - hi. A matmul A@B becomes
    Ah@Bh + Al@Bh + Ah@Bl (dropped lo*lo term is ~2^-24 relative).
    Aggregation matmuls need only 2 terms: the adjacency matrices are
    small integers, exact in f32r.
  * Gate matmuls pair two gates on the 128 output partitions (M=128
    instead of 64), halving streamed rows. The pair-packed PSUM is
    repacked to graph-packed tiles by the sigmoid activations themselves
    (single-input acts may cross partition offsets; 2-input DVE ops may
    not), so the LSTM pointwise stays full-height.
  * The per-literal degree bias (+ lit gate biases) is added once per
    gate-pair psum on DVE; clause gate biases ride the activation bias.
    The g-gate's tanh(x)=2*sigmoid(2x)-1 input doubling is pre-folded
    into the host-side weights/biases, keeping every activation a plain
    table sigmoid.

Layout: per core 2 graphs; feature-major state tiles [128, nodes] with
graph0 on partitions 0:64, graph1 on 64:128, kept in split (hi, lo)
f32r form. Row-major (transposed) hi/lo copies feed the aggregation
matmuls against constant f32r adjacency chunk tiles.
"""

import numpy as np

H = 64
ITERS = 24
B, NV, NC, K = 16, 400, 440, 12
NL = 2 * NV                  # literals/graph = 800
NPG = NL + NC                # nodes/graph = 1240
N = B * NPG                  # 19840
NCORES = 8
GPC = B // NCORES            # graphs per core = 2
CHK = 400                    # literal column chunk (aligned to NV flip halves)

_PROGRAM_CACHE = {}


def _build_program():
    from contextlib import ExitStack

    import concourse.bacc as bacc
    import concourse.mybir as mybir
    from concourse.masks import make_identity
    from concourse.tile import TileContext, add_dep_helper

    F32 = mybir.dt.float32
    F32R = mybir.dt.float32r
    SIG = mybir.ActivationFunctionType.Sigmoid
    MULT = mybir.AluOpType.mult
    SUB = mybir.AluOpType.subtract
    ADD = mybir.AluOpType.add

    nc = bacc.Bacc(
        "TRN2", target_bir_lowering=False, debug=False, num_devices=NCORES
    )

    # ---- DRAM I/O (per-core shards; weights replicated) ----
    d_xt_lit = nc.dram_tensor("xt_lit", [3, GPC * NL], F32, kind="ExternalInput")
    d_xt_cl = nc.dram_tensor("xt_cl", [3, GPC * NC], F32, kind="ExternalInput")
    d_at = nc.dram_tensor("at_rm", [GPC, 8, 128, NC], F32, kind="ExternalInput")
    d_a = nc.dram_tensor("a_rm", [GPC, 4, 128, NL], F32, kind="ExternalInput")
    WNAMES = ("wc_a", "wc_b", "wc_1", "wl_a", "wl_b", "w_lh_dup", "w_cl2_dup")
    d_w = {nm: nc.dram_tensor(nm, [128, 256], F32, kind="ExternalInput")
           for nm in WNAMES}
    d_wv = nc.dram_tensor("wv_dup", [128, 1], F32, kind="ExternalInput")
    d_liw = nc.dram_tensor("li_w3", [3, H], F32, kind="ExternalInput")
    d_ciw = nc.dram_tensor("ci_w3", [3, H], F32, kind="ExternalInput")
    d_bias = nc.dram_tensor("bias_q", [128, 5], F32, kind="ExternalInput")
    d_dxr = nc.dram_tensor("dxr", [2, GPC * NL], F32, kind="ExternalInput")
    d_wdq = nc.dram_tensor("wdq", [2, 256], F32, kind="ExternalInput")
    d_out = nc.dram_tensor("vote", [1, GPC * NL], F32, kind="ExternalOutput")

    with TileContext(nc) as tc, ExitStack() as ctx:
        const = ctx.enter_context(tc.tile_pool(name="const", bufs=1))
        state = ctx.enter_context(tc.tile_pool(name="state", bufs=2))
        work = ctx.enter_context(tc.tile_pool(name="work", bufs=1))
        pstp = ctx.enter_context(tc.tile_pool(name="pstp", bufs=1, space="PSUM"))
        psag = ctx.enter_context(tc.tile_pool(name="psag", bufs=2, space="PSUM"))
        psg = ctx.enter_context(tc.tile_pool(name="psg", bufs=5, space="PSUM"))

        LO, HI = slice(0, 64), slice(64, 128)
        HALF = (LO, HI)

        # ---- constants ----
        ident = const.tile([128, 128], F32, name="ident")
        make_identity(nc, ident)
        identr = const.tile([128, 128], F32R, name="identr")
        nc.scalar.copy(identr[:, :], ident[:, :])

        # adjacency chunks -> f32r const tiles (integers: cvt exact)
        at_r = const.tile([128, GPC * 8 * NC], F32R, name="at_r")
        for g in range(GPC):
            for kk in range(8):
                stg = work.tile([128, NC], F32, tag="ld", bufs=2,
                                name=f"ld_at_{g}_{kk}")
                nc.sync.dma_start(out=stg[:, :], in_=d_at[g, kk])
                c0 = NC * (8 * g + kk)
                nc.scalar.copy(at_r[:, c0:c0 + NC], stg[:, :])
        a_r = const.tile([128, GPC * 4 * NL], F32R, name="a_r")
        for g in range(GPC):
            for kk in range(4):
                stg = work.tile([128, NL], F32, tag="ld", bufs=2,
                                name=f"ld_a_{g}_{kk}")
                nc.sync.dma_start(out=stg[:, :], in_=d_a[g, kk])
                c0 = NL * (4 * g + kk)
                nc.scalar.copy(a_r[:, c0:c0 + NL], stg[:, :])

        # gate weights -> (hi, lo) f32r pairs (wc_a/wl_a unused since both
        # graphs share the (agg | state) stack row order)
        wsp = {}
        for nm in WNAMES:
            if nm in ("wc_a", "wl_a"):
                continue
            stg = work.tile([128, 256], F32, tag="ld", bufs=2, name=f"ldw_{nm}")
            nc.sync.dma_start(out=stg[:, :], in_=d_w[nm][:, :])
            wh = const.tile([128, 256], F32R, name=f"{nm}_h")
            wl = const.tile([128, 256], F32R, name=f"{nm}_l")
            nc.scalar.copy(wh[:, :], stg[:, :])
            nc.vector.tensor_tensor(wl[:, :], stg[:, :], wh[:, :], op=SUB)
            wsp[nm] = (wh, wl)
        stg = work.tile([128, 1], F32, tag="ld", bufs=2, name="ldw_wv")
        nc.sync.dma_start(out=stg[:, :], in_=d_wv[:, :])
        wv_h = const.tile([128, 1], F32R, name="wv_h")
        wv_l = const.tile([128, 1], F32R, name="wv_l")
        nc.scalar.copy(wv_h[:, :], stg[:, :])
        nc.vector.tensor_tensor(wv_l[:, :], stg[:, :], wv_h[:, :], op=SUB)

        def load(dram, shape, nm):
            t = const.tile(shape, F32, name=nm)
            nc.sync.dma_start(out=t[:, :], in_=dram[:, :])
            return t

        xt_lit = load(d_xt_lit, [3, GPC * NL], "xt_lit_sb")
        xt_cl = load(d_xt_cl, [3, GPC * NC], "xt_cl_sb")
        li_w = load(d_liw, [3, H], "li_w_sb")
        ci_w = load(d_ciw, [3, H], "ci_w_sb")
        bias = load(d_bias, [128, 5], "bias_sb")
        stg = work.tile([2, GPC * NL], F32, tag="ld2", bufs=1, name="ld_dxr")
        nc.sync.dma_start(out=stg[:, :], in_=d_dxr[:, :])
        dxr = const.tile([2, GPC * NL], F32R, name="dxr_sb")
        nc.scalar.copy(dxr[:, :], stg[:, :])
        stg = work.tile([2, 256], F32, tag="ld3", bufs=1, name="ld_wdq")
        nc.sync.dma_start(out=stg[:, :], in_=d_wdq[:, :])
        wdq_h = const.tile([2, 256], F32R, name="wdq_h")
        wdq_l = const.tile([2, 256], F32R, name="wdq_l")
        nc.scalar.copy(wdq_h[:, :], stg[:, :])
        nc.vector.tensor_tensor(wdq_l[:, :], stg[:, :], wdq_h[:, :], op=SUB)

        def MM(*a, **kw):
            kw.setdefault("skip_group_check", True)
            return nc.tensor.matmul(*a, **kw)

        # ---- initial node states (bias via ones row of xt) ----
        Lh = state.tile([128, NL], F32R, tag="Lh", name="Lh0")
        Ll = state.tile([128, NL], F32R, tag="Ll", name="Ll0")
        lit_hf_a = state.tile([128, CHK], F32, tag="lit_hf_a", name="lit_hf_a0")
        lit_hf_b = state.tile([128, CHK], F32, tag="lit_hf_b", name="lit_hf_b0")
        for hf in range(2):
            p = psg.tile([128, CHK], F32, tag="g", name=f"ini_{hf}")
            prev = None
            for g in range(GPC):
                mm = MM(p[HALF[g], :], li_w[0:3, :],
                        xt_lit[0:3, g * NL + hf * CHK:g * NL + (hf + 1) * CHK],
                        start=True, stop=True, tile_position=(0, 64 * g))
                if prev is not None:
                    add_dep_helper(mm.ins, prev.ins, sync=True,
                                   reason="psum half order")
                prev = mm
            cs = slice(hf * CHK, (hf + 1) * CHK)
            nc.vector.tensor_copy((lit_hf_a if hf == 0 else lit_hf_b)[:, :],
                                  p[:, :])
            nc.scalar.copy(Lh[:, cs], p[:, :])
            nc.vector.tensor_tensor(Ll[:, cs], p[:, :], Lh[:, cs], op=SUB)
        Chh = state.tile([128, NC], F32R, tag="Chh", name="Chh0")
        Chl = state.tile([128, NC], F32R, tag="Chl", name="Chl0")
        cl_hf = state.tile([128, NC], F32, tag="cl_hf", name="cl_hf0")
        pc = psg.tile([128, NC], F32, tag="g", name="ini_c")
        prev = None
        for g in range(GPC):
            mm = MM(pc[HALF[g], :], ci_w[0:3, :], xt_cl[0:3, g * NC:(g + 1) * NC],
                    start=True, stop=True, tile_position=(0, 64 * g))
            if prev is not None:
                add_dep_helper(mm.ins, prev.ins, sync=True, reason="psum half order")
            prev = mm
        nc.vector.tensor_copy(cl_hf[:, :], pc[:, :])
        nc.scalar.copy(Chh[:, :], pc[:, :])
        nc.vector.tensor_tensor(Chl[:, :], pc[:, :], Chh[:, :], op=SUB)

        lit_c = None
        cl_c = None

        for t in range(1, ITERS):
            first = t == 1

            # ==== clause phase ====
            # dependency-free copies first: clause stack state-halves and the
            # lit-phase flip halves (keeps Pool busy off the critical path)
            st0h = work.tile([128, NC], F32R, tag="st0h", name=f"st0h_{t}")
            st0l = work.tile([128, NC], F32R, tag="st0l", name=f"st0l_{t}")
            st1h = work.tile([128, NC], F32R, tag="st1h", name=f"st1h_{t}")
            st1l = work.tile([128, NC], F32R, tag="st1l", name=f"st1l_{t}")
            nc.scalar.copy(st0h[HI, :], Chh[LO, :])
            nc.scalar.copy(st0l[HI, :], Chl[LO, :])
            nc.gpsimd.tensor_copy(st1h[HI, :], Chh[HI, :])
            nc.gpsimd.tensor_copy(st1l[HI, :], Chl[HI, :])
            lst = []
            for hf in range(2):
                fs = slice((1 - hf) * CHK, (2 - hf) * CHK)
                s0h = work.tile([128, CHK], F32R, tag="s0h", bufs=2, name=f"s0h_{t}_{hf}")
                s0l = work.tile([128, CHK], F32R, tag="s0l", bufs=2, name=f"s0l_{t}_{hf}")
                s1h = work.tile([128, CHK], F32R, tag="s1h", bufs=2, name=f"s1h_{t}_{hf}")
                s1l = work.tile([128, CHK], F32R, tag="s1l", bufs=2, name=f"s1l_{t}_{hf}")
                nc.scalar.copy(s0h[HI, :], Lh[LO, fs])
                nc.scalar.copy(s0l[HI, :], Ll[LO, fs])
                nc.gpsimd.tensor_copy(s1h[HI, :], Lh[HI, fs])
                nc.gpsimd.tensor_copy(s1l[HI, :], Ll[HI, fs])
                lst.append((s0h, s0l, s1h, s1l))

            # transpose lit state per half-grid (chunks 128,128,128,16 per
            # half) so the half-0 transposes + agg terms start while the
            # half-1 pointwise of the previous lit phase is still draining
            GRID = ((0, 128), (128, 128), (256, 128), (272, 128))
            rml = [[None, None], [None, None]]   # rml[g][half] = (rm_h, rm_l)
            for x, src in ((0, lit_hf_a), (1, lit_hf_b)):
                for g in range(GPC):
                    tp = pstp.tile([128, 4 * H], F32, tag="tp",
                                   name=f"tpl_{t}_{g}_{x}")
                    for kk, (c0, sz) in enumerate(GRID):
                        nc.tensor.transpose(
                            tp[:, kk * H:(kk + 1) * H],
                            src[HALF[g], c0:c0 + sz],
                            ident[HALF[g], HALF[g]],
                        )
                    rm_h = work.tile([128, 4 * H], F32R, tag=f"rmlh{g}{x}",
                                     name=f"rmlh_{t}_{g}_{x}")
                    rm_l = work.tile([128, 4 * H], F32R, tag=f"rmll{g}{x}",
                                     name=f"rmll_{t}_{g}_{x}")
                    nc.vector.tensor_copy(rm_h[:, :], tp[:, :])
                    nc.vector.tensor_tensor(rm_l[:, :], tp[:, :], rm_h[:, :],
                                            op=SUB)
                    rml[g][x] = (rm_h, rm_l)

            # clause agg A^T @ L: per-graph psum tiles (g0 rows HI, g1 LO) so
            # the hi/lo term groups interleave without bank conflicts
            agc0 = psag.tile([128, NC], F32, tag="ag", name=f"agc0_{t}")
            agc1 = psag.tile([128, NC], F32, tag="ag", name=f"agc1_{t}")
            agp = (agc0[LO, :], agc1[LO, :])
            for x in range(2):
                for term in range(2):
                    for g in range(GPC):
                        for kk in range(4):
                            c = NC * (8 * g + 4 * x + kk)
                            MM(agp[g],
                               rml[g][x][term][:, kk * H:(kk + 1) * H],
                               at_r[:, c:c + NC],
                               start=(x == 0 and term == 0 and kk == 0),
                               stop=(x == 1 and term == 1 and kk == 3),
                               tile_position=(0, 0))

            # stack agg halves (aligned at LO)
            nc.scalar.copy(st0h[LO, :], agc0[LO, :])
            nc.vector.tensor_tensor(st0l[LO, :], agc0[LO, :], st0h[LO, :], op=SUB)
            nc.scalar.copy(st1h[LO, :], agc1[LO, :])
            nc.vector.tensor_tensor(st1l[LO, :], agc1[LO, :], st1h[LO, :], op=SUB)

            # clause gates: per graph, 2 gate-pairs, 3-term split
            cg = [[None, None], [None, None]]
            for g in range(GPC):
                wnm = "wc_1" if first else "wc_b"
                wh, wl = wsp[wnm]
                sth, stl = (st0h, st0l) if g == 0 else (st1h, st1l)
                for p in range(2):
                    ps_ = slice(p * 128, (p + 1) * 128)
                    gp = psg.tile([128, NC], F32, tag="g", name=f"cg{g}{p}_{t}")
                    MM(gp[:, :], wh[:, ps_], sth[:, :], start=True, stop=False)
                    MM(gp[:, :], wl[:, ps_], sth[:, :], start=False, stop=False)
                    MM(gp[:, :], wh[:, ps_], stl[:, :], start=False, stop=True)
                    cg[g][p] = gp

            # repack sigmoids: pair-psum -> graph-packed s tiles
            s_i = work.tile([128, NC], F32, tag="si", name=f"csi_{t}")
            s_f = work.tile([128, NC], F32, tag="sf", name=f"csf_{t}")
            s_g = work.tile([128, NC], F32, tag="sg", name=f"csg_{t}")
            s_o = work.tile([128, NC], F32, tag="so", name=f"cso_{t}")
            nc.scalar.activation(s_i[LO, :], cg[0][0][LO, :], SIG,
                                 bias=bias[LO, 0:1])
            nc.scalar.activation(s_f[LO, :], cg[0][0][HI, :], SIG,
                                 bias=bias[LO, 1:2])
            nc.scalar.activation(s_g[LO, :], cg[0][1][LO, :], SIG,
                                 bias=bias[LO, 2:3])
            nc.scalar.activation(s_i[HI, :], cg[1][0][LO, :], SIG,
                                 bias=bias[HI, 0:1])
            nc.scalar.activation(s_f[HI, :], cg[1][0][HI, :], SIG,
                                 bias=bias[HI, 1:2])
            nc.scalar.activation(s_g[HI, :], cg[1][1][LO, :], SIG,
                                 bias=bias[HI, 2:3])
            nc.scalar.activation(s_o[LO, :], cg[0][1][HI, :], SIG,
                                 bias=bias[LO, 3:4])
            nc.scalar.activation(s_o[HI, :], cg[1][1][HI, :], SIG,
                                 bias=bias[HI, 3:4])

            # clause LSTM pointwise (graph-packed, full height)
            cc_new = state.tile([128, NC], F32, tag="cl_c", name=f"cc_{t}")
            # pointwise chain column-split across DVE (lo cols) and Pool (hi);
            # tanh realized as tg = 2*sigmoid-1 first so every link is a
            # splittable tensor_scalar/tensor_tensor (no DVE-only stt)
            CSP = ((nc.vector, slice(0, 220)), (nc.gpsimd, slice(220, NC)))
            tg = work.tile([128, NC], F32, tag="t1", name=f"ctg_{t}")
            for eng, q in CSP:
                eng.tensor_scalar(tg[:, q], s_g[:, q], 2.0, -1.0,
                                  op0=MULT, op1=ADD)
            if first:
                for eng, q in CSP:
                    eng.tensor_mul(cc_new[:, q], s_i[:, q], tg[:, q])
            else:
                u = work.tile([128, NC], F32, tag="u", name=f"cu_{t}")
                t2 = work.tile([128, NC], F32, tag="t2", name=f"ct2_{t}")
                for eng, q in CSP:
                    eng.tensor_mul(u[:, q], s_i[:, q], tg[:, q])
                for eng, q in CSP:
                    eng.tensor_mul(t2[:, q], s_f[:, q], cl_c[:, q])
                for eng, q in CSP:
                    eng.tensor_add(cc_new[:, q], u[:, q], t2[:, q])
            tnc = work.tile([128, NC], F32, tag="tnc", name=f"ctn_{t}")
            nc.scalar.activation(tnc[:, 0:220], cc_new[:, 0:220], SIG, scale=2.0)
            nc.scalar.activation(tnc[:, 220:NC], cc_new[:, 220:NC], SIG, scale=2.0)
            th = work.tile([128, NC], F32, tag="t3", name=f"cth_{t}")
            cl_hf_new = state.tile([128, NC], F32, tag="cl_hf", name=f"chf_{t}")
            for eng, q in CSP:
                eng.tensor_scalar(th[:, q], tnc[:, q], 2.0, -1.0,
                                  op0=MULT, op1=ADD)
            for eng, q in CSP:
                eng.tensor_mul(cl_hf_new[:, q], s_o[:, q], th[:, q])
            # split off the critical path (consumers are next iteration)
            Chh_new = state.tile([128, NC], F32R, tag="Chh", name=f"Chh_{t}")
            Chl_new = state.tile([128, NC], F32R, tag="Chl", name=f"Chl_{t}")
            nc.gpsimd.tensor_copy(Chh_new[:, :], cl_hf_new[:, :])
            nc.vector.tensor_tensor(Chl_new[:, :], cl_hf_new[:, :], Chh_new[:, :],
                                    op=SUB)

            # ==== lit phase ====
            # transpose full clause state; split rides the psum copy
            rmc = []
            for g in range(GPC):
                tp = pstp.tile([128, 4 * H], F32, tag="tp", name=f"tpc_{t}_{g}")
                for kk in range(4):
                    c0 = 128 * kk if kk < 3 else NC - 128
                    nc.tensor.transpose(
                        tp[:, kk * H:(kk + 1) * H],
                        cl_hf_new[HALF[g], c0:c0 + 128],
                        ident[HALF[g], HALF[g]],
                    )
                rm_h = work.tile([128, 4 * H], F32R, tag=f"rmch{g}",
                                 name=f"rmch_{t}_{g}")
                rm_l = work.tile([128, 4 * H], F32R, tag=f"rmcl{g}",
                                 name=f"rmcl_{t}_{g}")
                nc.scalar.copy(rm_h[:, :], tp[:, :])
                nc.vector.tensor_tensor(rm_l[:, :], tp[:, :], rm_h[:, :], op=SUB)
                rmc.append((rm_h, rm_l))

            Lh_new = state.tile([128, NL], F32R, tag="Lh", name=f"Lh_{t}")
            Ll_new = state.tile([128, NL], F32R, tag="Ll", name=f"Ll_{t}")
            lit_hf_na = state.tile([128, CHK], F32, tag="lit_hf_a", name=f"lhfa_{t}")
            lit_hf_nb = state.tile([128, CHK], F32, tag="lit_hf_b", name=f"lhfb_{t}")
            lc_new = state.tile([128, NL], F32, tag="lit_c", name=f"lc_{t}")
            wSnm = "w_cl2_dup" if first else "w_lh_dup"
            wSh, wSl = wsp[wSnm]
            for hf in range(2):
                cs = slice(hf * CHK, (hf + 1) * CHK)
                s0h, s0l, s1h, s1l = lst[hf]
                # literal agg A @ C: per-graph psums, interleaved term groups
                agl0 = psag.tile([128, CHK], F32, tag="ag", name=f"agl0_{t}_{hf}")
                agl1 = psag.tile([128, CHK], F32, tag="ag", name=f"agl1_{t}_{hf}")
                agp = (agl0[LO, :], agl1[LO, :])
                for term in range(2):
                    for g in range(GPC):
                        for kk in range(4):
                            MM(agp[g], rmc[g][term][:, kk * H:(kk + 1) * H],
                               a_r[:, NL * (4 * g + kk) + hf * CHK:
                                   NL * (4 * g + kk) + (hf + 1) * CHK],
                               start=(term == 0 and kk == 0),
                               stop=(term == 1 and kk == 3),
                               tile_position=(0, 0))

                # lit gates: dq + rec terms first (no stack dependency -> they
                # fill the PE pipeline while stacks build)
                gps = [[None, None], [None, None]]
                for g in range(GPC):
                    hg = HALF[g]
                    for p in range(2):
                        ps_ = slice(p * 128, (p + 1) * 128)
                        gp = psg.tile([128, CHK], F32, tag="g", name=f"lg{g}{p}_{t}_{hf}")
                        dc = slice(g * NL + hf * CHK, g * NL + (hf + 1) * CHK)
                        MM(gp[:, :], wdq_h[0:2, ps_], dxr[0:2, dc],
                           start=True, stop=False)
                        MM(gp[:, :], wdq_l[0:2, ps_], dxr[0:2, dc],
                           start=False, stop=False)
                        MM(gp[:, :], wSh[hg, ps_], Lh[hg, cs], start=False,
                           stop=False, tile_position=(64 * g, 0))
                        MM(gp[:, :], wSl[hg, ps_], Lh[hg, cs], start=False,
                           stop=False, tile_position=(64 * g, 0))
                        MM(gp[:, :], wSh[hg, ps_], Ll[hg, cs], start=False,
                           stop=False, tile_position=(64 * g, 0))
                        gps[g][p] = gp

                # stack agg halves (aligned at LO)
                nc.scalar.copy(s0h[LO, :], agl0[LO, :])
                nc.vector.tensor_tensor(s0l[LO, :], agl0[LO, :], s0h[LO, :], op=SUB)
                nc.scalar.copy(s1h[LO, :], agl1[LO, :])
                nc.vector.tensor_tensor(s1l[LO, :], agl1[LO, :], s1h[LO, :], op=SUB)

                # stack-dependent gate terms
                for g in range(GPC):
                    wnm = "wl_b"
                    wh, wl = wsp[wnm]
                    sth, stl = (s0h, s0l) if g == 0 else (s1h, s1l)
                    for p in range(2):
                        ps_ = slice(p * 128, (p + 1) * 128)
                        gp = gps[g][p]
                        MM(gp[:, :], wh[:, ps_], sth[:, :], start=False, stop=False)
                        MM(gp[:, :], wl[:, ps_], sth[:, :], start=False, stop=False)
                        MM(gp[:, :], wh[:, ps_], stl[:, :], start=False, stop=True)

                s_i = work.tile([128, CHK], F32, tag="lsi", bufs=2, name=f"lsi_{t}_{hf}")
                s_f = work.tile([128, CHK], F32, tag="lsf", bufs=2, name=f"lsf_{t}_{hf}")
                s_g = work.tile([128, CHK], F32, tag="lsg", bufs=2, name=f"lsg_{t}_{hf}")
                s_o = work.tile([128, CHK], F32, tag="lso", bufs=2, name=f"lso_{t}_{hf}")
                nc.scalar.activation(s_i[LO, :], gps[0][0][LO, :], SIG)
                nc.scalar.activation(s_f[LO, :], gps[0][0][HI, :], SIG)
                nc.scalar.activation(s_g[LO, :], gps[0][1][LO, :], SIG)
                nc.scalar.activation(s_i[HI, :], gps[1][0][LO, :], SIG)
                nc.scalar.activation(s_f[HI, :], gps[1][0][HI, :], SIG)
                nc.scalar.activation(s_g[HI, :], gps[1][1][LO, :], SIG)
                nc.scalar.activation(s_o[LO, :], gps[0][1][HI, :], SIG)
                nc.scalar.activation(s_o[HI, :], gps[1][1][HI, :], SIG)

                tg = work.tile([128, CHK], F32, tag="lt1", bufs=2, name=f"ltg_{t}_{hf}")
                LSP = ((nc.vector, slice(0, 200)), (nc.gpsimd, slice(200, CHK)))
                cq = [slice(cs.start, cs.start + 200),
                      slice(cs.start + 200, cs.stop)]
                for eng, q in LSP:
                    eng.tensor_scalar(tg[:, q], s_g[:, q], 2.0, -1.0,
                                      op0=MULT, op1=ADD)
                if first:
                    for (eng, q), c2 in zip(LSP, cq):
                        eng.tensor_mul(lc_new[:, c2], s_i[:, q], tg[:, q])
                else:
                    u = work.tile([128, CHK], F32, tag="lu", bufs=2, name=f"lu_{t}_{hf}")
                    t2 = work.tile([128, CHK], F32, tag="lt2", bufs=1, name=f"lt2_{t}_{hf}")
                    for eng, q in LSP:
                        eng.tensor_mul(u[:, q], s_i[:, q], tg[:, q])
                    for (eng, q), c2 in zip(LSP, cq):
                        eng.tensor_mul(t2[:, q], s_f[:, q], lit_c[:, c2])
                    for (eng, q), c2 in zip(LSP, cq):
                        eng.tensor_add(lc_new[:, c2], u[:, q], t2[:, q])
                tnc = work.tile([128, CHK], F32, tag="ltn", bufs=1, name=f"ltn_{t}_{hf}")
                nc.scalar.activation(tnc[:, 0:200],
                                     lc_new[:, cs.start:cs.start + 200],
                                     SIG, scale=2.0)
                nc.scalar.activation(tnc[:, 200:CHK],
                                     lc_new[:, cs.start + 200:cs.stop],
                                     SIG, scale=2.0)
                th = work.tile([128, CHK], F32, tag="lt3", bufs=1, name=f"lth_{t}_{hf}")
                lhf_x = lit_hf_na if hf == 0 else lit_hf_nb
                for eng, q in LSP:
                    eng.tensor_scalar(th[:, q], tnc[:, q], 2.0, -1.0,
                                      op0=MULT, op1=ADD)
                for eng, q in LSP:
                    eng.tensor_mul(lhf_x[:, q], s_o[:, q], th[:, q])
                # split off the critical path (consumers are next iteration)
                nc.gpsimd.tensor_copy(Lh_new[:, cs], lhf_x[:, :])
                nc.vector.tensor_tensor(Ll_new[:, cs], lhf_x[:, :],
                                        Lh_new[:, cs], op=SUB)

            Lh, Ll = Lh_new, Ll_new
            lit_hf_a, lit_hf_b = lit_hf_na, lit_hf_nb
            Chh, Chl, cl_hf = Chh_new, Chl_new, cl_hf_new
            lit_c, cl_c = lc_new, cc_new

        # ---- vote head: (Lh+Ll) @ Wv, 3-term split ----
        for g in range(GPC):
            hg = HALF[g]
            for hf in range(2):
                cs = slice(hf * CHK, (hf + 1) * CHK)
                p = psg.tile([1, CHK], F32, tag="g", name=f"vps_{g}_{hf}")
                MM(p[:, :], wv_h[hg, 0:1], Lh[hg, cs], start=True, stop=False,
                   tile_position=(64 * g, 0))
                MM(p[:, :], wv_l[hg, 0:1], Lh[hg, cs], start=False, stop=False,
                   tile_position=(64 * g, 0))
                MM(p[:, :], wv_h[hg, 0:1], Ll[hg, cs], start=False, stop=True,
                   tile_position=(64 * g, 0))
                vc = work.tile([1, CHK], F32, tag="vote", bufs=1,
                               name=f"vote_{g}_{hf}")
                nc.scalar.activation(
                    vc[:, :], p[:, :], mybir.ActivationFunctionType.Identity,
                    bias=bias[0:1, 4:5],
                )
                nc.sync.dma_start(
                    out=d_out[0:1, g * NL + hf * CHK:g * NL + (hf + 1) * CHK],
                    in_=vc[:, :])

    nc.compile()
    return nc


def _fold_and_shard(inputs):
    """Host-side preprocessing: fold weights, build adjacency, shard by graph."""
    f32 = np.float32
    g = {k: np.asarray(v) for k, v in inputs.items()}

    def collapse(w1, b1, w2, b2, w3, b3):
        return w1 @ w2 @ w3, ((b1 @ w2) + b2) @ w3 + b3

    Wl, bl = collapse(g["lm1_w"], g["lm1_b"], g["lm2_w"], g["lm2_b"],
                      g["lm3_w"], g["lm3_b"])
    Wc, bc = collapse(g["cm1_w"], g["cm1_b"], g["cm2_w"], g["cm2_b"],
                      g["cm3_w"], g["cm3_b"])
    Wv, bv = collapse(g["lv1_w"], g["lv1_b"], g["lv2_w"], g["lv2_b"],
                      g["lv3_w"], g["lv3_b"])

    cu_wih, lu_wih = g["cu_wih"], g["lu_wih"]
    w_lc = (Wl @ cu_wih).astype(f32)                 # agg_c -> clause gates
    w_ch = (w_lc + g["cu_whh"]).astype(f32)          # t>=2 merged recurrent
    cbias_c = ((K + 1) * (bl @ cu_wih) + g["cu_bih"] + g["cu_bhh"]).astype(f32)
    wih_a = lu_wih[0:H].astype(f32)                  # flip -> lit gates
    w_cl2 = (Wc @ lu_wih[H:2 * H]).astype(f32)       # agg_l -> lit gates
    w_lh = (w_cl2 + g["lu_whh"]).astype(f32)         # t>=2 merged recurrent
    q_l = (bc @ lu_wih[H:2 * H]).astype(f32)         # [256]
    cbias_l = (g["lu_bih"] + g["lu_bhh"]).astype(f32)

    def gdouble(w):
        w = w.copy()
        w[:, 2 * H:3 * H] *= 2.0     # g-gate runs as sigmoid(2x+2b)
        return w

    vs = np.vstack
    wc_a = gdouble(vs([w_ch, w_lc]))
    wc_b = gdouble(vs([w_lc, w_ch]))
    wc_1 = gdouble(vs([w_lc, w_lc]))
    wl_a = gdouble(vs([wih_a, w_cl2]))
    wl_b = gdouble(vs([w_cl2, wih_a]))
    w_lh_dup = gdouble(vs([w_lh, w_lh]))
    w_cl2_dup = gdouble(vs([w_cl2, w_cl2]))
    wv_dup = vs([Wv.astype(f32), Wv.astype(f32)])

    bias_q = np.zeros((128, 5), f32)
    for x in range(4):
        scl = 2.0 if x == 2 else 1.0
        bias_q[0:64, x] = scl * cbias_c[x * H:(x + 1) * H]
        bias_q[64:128, x] = scl * cbias_c[x * H:(x + 1) * H]
    bias_q[0, 4] = bv[0]

    li_w3 = np.concatenate([g["li_w"], g["li_b"][None, :]], axis=0).astype(f32)
    ci_w3 = np.concatenate([g["ci_w"], g["ci_b"][None, :]], axis=0).astype(f32)

    # adjacency per graph from edge_index (direction-robust)
    ei = g["edge_index"].astype(np.int64)
    src, dst = ei[0], ei[1]
    src_g, dst_g = src // NPG, dst // NPG
    assert np.all(src_g == dst_g), "edges must be graph-local"
    src_l, dst_l = src % NPG, dst % NPG
    s_lit, d_lit = src_l < NL, dst_l < NL
    A_in_c = np.zeros((B, NC, NL), f32)   # clause <- literal edges
    m = (~d_lit) & s_lit
    np.add.at(A_in_c, (dst_g[m], dst_l[m] - NL, src_l[m]), 1.0)
    A_in_l = np.zeros((B, NL, NC), f32)   # literal <- clause edges
    m = d_lit & (~s_lit)
    np.add.at(A_in_l, (dst_g[m], dst_l[m], src_l[m] - NL), 1.0)
    deg_l = A_in_l.sum(axis=2)            # [B, NL]

    x = g["x"].astype(f32).reshape(B, NPG, 2)
    ones = np.ones((B, NPG, 1), f32)
    x3 = np.concatenate([x, ones], axis=2)        # [B, NPG, 3]

    shared = dict(
        wc_a=wc_a, wc_b=wc_b, wc_1=wc_1, wl_a=wl_a, wl_b=wl_b,
        w_lh_dup=w_lh_dup, w_cl2_dup=w_cl2_dup, wv_dup=wv_dup,
        li_w3=li_w3, ci_w3=ci_w3, bias_q=bias_q,
    )
    in_maps = []
    for c in range(NCORES):
        gs = slice(c * GPC, (c + 1) * GPC)
        x3c = x3[gs]                               # [GPC, NPG, 3]
        xt_lit = np.ascontiguousarray(
            x3c[:, :NL].transpose(2, 0, 1).reshape(3, GPC * NL))
        xt_cl = np.ascontiguousarray(
            x3c[:, NL:].transpose(2, 0, 1).reshape(3, GPC * NC))
        # dxr rows: (deg+1) per literal, ones; wdq rows: q, cbias_l
        # (g-gate block doubled to match the pre-doubled weights)
        dxr = np.ones((2, GPC * NL), f32)
        for gg in range(GPC):
            dxr[0, gg * NL:(gg + 1) * NL] = deg_l[c * GPC + gg] + 1.0
        wdq = np.stack([q_l, cbias_l]).astype(f32)
        wdq[:, 2 * H:3 * H] *= 2.0
        # pre-chunk adjacency into full-128-row K-chunks; the final chunk
        # overlaps the previous one with its overlap rows zeroed
        atc = np.zeros((GPC, 8, 128, NC), f32)
        ac = np.zeros((GPC, 4, 128, NL), f32)
        for gg in range(GPC):
            at_full = A_in_c[c * GPC + gg].T       # [NL, NC]
            a_full = A_in_l[c * GPC + gg].T        # [NC, NL]
            for x in range(2):
                for j in range(3):
                    atc[gg, 4 * x + j] = at_full[x * 400 + 128 * j:
                                                 x * 400 + 128 * (j + 1)]
                atc[gg, 4 * x + 3, 112:128] = at_full[x * 400 + 384:
                                                      x * 400 + 400]
            for kk in range(3):
                ac[gg, kk] = a_full[128 * kk:128 * (kk + 1)]
            ac[gg, 3, 128 - (NC - 384):] = a_full[384:]
        in_maps.append(dict(
            xt_lit=xt_lit, xt_cl=xt_cl, at_rm=atc, a_rm=ac,
            dxr=dxr, wdq=wdq, **shared,
        ))
    return in_maps


_LAST_RESULTS = {}


def kernel(**inputs):
    from concourse.bass_utils import run_bass_kernel_spmd

    in_maps = _fold_and_shard(inputs)
    if "nc" not in _PROGRAM_CACHE:
        _PROGRAM_CACHE["nc"] = _build_program()
    nc = _PROGRAM_CACHE["nc"]
    res = run_bass_kernel_spmd(nc, in_maps, core_ids=list(range(NCORES)))
    _LAST_RESULTS["res"] = res
    out = np.zeros((N, 1), np.float32)
    for c in range(NCORES):
        vote = res.results[c]["vote"].reshape(GPC, NL)
        for g in range(GPC):
            base = (c * GPC + g) * NPG
            out[base:base + NL, 0] = vote[g]
    return out


# revision 12
# speedup vs baseline: 1.2063x; 1.0288x over previous
"""NeuroSAT GNN message passing on 8 Trainium2 NeuronCores — v2.

Speedups over the v1 graph-data-parallel kernel:
  * All large matmuls run as fp32r (hw-rounded fp32, ~11 mantissa bits) at
    1 cycle/row instead of fp32's 4. Accuracy is restored with a hi/lo
    split: x = hi + lo with hi = round_f32r(x) (free: the producing op
    writes an f32r tile), lo = x - hi. A matmul A@B becomes
    Ah@Bh + Al@Bh + Ah@Bl (dropped lo*lo term is ~2^-24 relative).
    Aggregation matmuls need only 2 terms: the adjacency matrices are
    small integers, exact in f32r.
  * Gate matmuls pair two gates on the 128 output partitions (M=128
    instead of 64), halving streamed rows. The pair-packed PSUM is
    repacked to graph-packed tiles by the sigmoid activations themselves
    (single-input acts may cross partition offsets; 2-input DVE ops may
    not), so the LSTM pointwise stays full-height.
  * The per-literal degree bias (+ lit gate biases) is added once per
    gate-pair psum on DVE; clause gate biases ride the activation bias.
    The g-gate's tanh(x)=2*sigmoid(2x)-1 input doubling is pre-folded
    into the host-side weights/biases, keeping every activation a plain
    table sigmoid.

Layout: per core 2 graphs; feature-major state tiles [128, nodes] with
graph0 on partitions 0:64, graph1 on 64:128, kept in split (hi, lo)
f32r form. Row-major (transposed) hi/lo copies feed the aggregation
matmuls against constant f32r adjacency chunk tiles.
"""

import numpy as np

H = 64
ITERS = 24
B, NV, NC, K = 16, 400, 440, 12
NL = 2 * NV                  # literals/graph = 800
NPG = NL + NC                # nodes/graph = 1240
N = B * NPG                  # 19840
NCORES = 8
GPC = B // NCORES            # graphs per core = 2
CHK = 400                    # literal column chunk (aligned to NV flip halves)

_PROGRAM_CACHE = {}


def _build_program():
    from contextlib import ExitStack

    import concourse.bacc as bacc
    import concourse.mybir as mybir
    from concourse.masks import make_identity
    from concourse.tile import TileContext, add_dep_helper

    F32 = mybir.dt.float32
    F32R = mybir.dt.float32r
    SIG = mybir.ActivationFunctionType.Sigmoid
    MULT = mybir.AluOpType.mult
    SUB = mybir.AluOpType.subtract
    ADD = mybir.AluOpType.add

    nc = bacc.Bacc(
        "TRN2", target_bir_lowering=False, debug=False, num_devices=NCORES
    )

    # ---- DRAM I/O (per-core shards; weights replicated) ----
    d_xt_lit = nc.dram_tensor("xt_lit", [3, GPC * NL], F32, kind="ExternalInput")
    d_xt_cl = nc.dram_tensor("xt_cl", [3, GPC * NC], F32, kind="ExternalInput")
    d_at = nc.dram_tensor("at_rm", [GPC, 8, 128, NC], F32, kind="ExternalInput")
    d_a = nc.dram_tensor("a_rm", [GPC, 4, 128, NL], F32, kind="ExternalInput")
    WNAMES = ("wc_a", "wc_b", "wc_1", "wl_a", "wl_b", "w_lh_dup", "w_cl2_dup")
    d_w = {nm: nc.dram_tensor(nm, [128, 256], F32, kind="ExternalInput")
           for nm in WNAMES}
    d_wv = nc.dram_tensor("wv_dup", [128, 1], F32, kind="ExternalInput")
    d_liw = nc.dram_tensor("li_w3", [3, H], F32, kind="ExternalInput")
    d_ciw = nc.dram_tensor("ci_w3", [3, H], F32, kind="ExternalInput")
    d_bias = nc.dram_tensor("bias_q", [128, 5], F32, kind="ExternalInput")
    d_dxr = nc.dram_tensor("dxr", [2, GPC * NL], F32, kind="ExternalInput")
    d_wdq = nc.dram_tensor("wdq", [2, 256], F32, kind="ExternalInput")
    d_out = nc.dram_tensor("vote", [1, GPC * NL], F32, kind="ExternalOutput")

    with TileContext(nc) as tc, ExitStack() as ctx:
        const = ctx.enter_context(tc.tile_pool(name="const", bufs=1))
        state = ctx.enter_context(tc.tile_pool(name="state", bufs=2))
        work = ctx.enter_context(tc.tile_pool(name="work", bufs=1))
        pstp = ctx.enter_context(tc.tile_pool(name="pstp", bufs=1, space="PSUM"))
        psag = ctx.enter_context(tc.tile_pool(name="psag", bufs=2, space="PSUM"))
        psg = ctx.enter_context(tc.tile_pool(name="psg", bufs=5, space="PSUM"))

        LO, HI = slice(0, 64), slice(64, 128)
        HALF = (LO, HI)

        # ---- constants ----
        ident = const.tile([128, 128], F32, name="ident")
        make_identity(nc, ident)
        identr = const.tile([128, 128], F32R, name="identr")
        nc.scalar.copy(identr[:, :], ident[:, :])

        # adjacency chunks -> f32r const tiles (integers: cvt exact)
        CVT = (nc.scalar.copy, nc.vector.tensor_copy, nc.gpsimd.tensor_copy)
        ncv = 0
        at_r = const.tile([128, GPC * 8 * NC], F32R, name="at_r")
        for g in range(GPC):
            for kk in range(8):
                stg = work.tile([128, NC], F32, tag="ld", bufs=4,
                                name=f"ld_at_{g}_{kk}")
                nc.sync.dma_start(out=stg[:, :], in_=d_at[g, kk])
                c0 = NC * (8 * g + kk)
                nc.scalar.copy(at_r[:, c0:c0 + NC], stg[:, :])
        a_r = const.tile([128, GPC * 4 * NL], F32R, name="a_r")
        for g in range(GPC):
            for kk in range(4):
                stg = work.tile([128, NL], F32, tag="ld", bufs=4,
                                name=f"ld_a_{g}_{kk}")
                nc.sync.dma_start(out=stg[:, :], in_=d_a[g, kk])
                c0 = NL * (4 * g + kk)
                nc.scalar.copy(a_r[:, c0:c0 + NL], stg[:, :])

        # gate weights -> (hi, lo) f32r pairs (wc_a/wl_a unused since both
        # graphs share the (agg | state) stack row order)
        wsp = {}
        for nm in WNAMES:
            if nm in ("wc_a", "wl_a"):
                continue
            stg = work.tile([128, 256], F32, tag="ld", bufs=4, name=f"ldw_{nm}")
            nc.sync.dma_start(out=stg[:, :], in_=d_w[nm][:, :])
            wh = const.tile([128, 256], F32R, name=f"{nm}_h")
            wl = const.tile([128, 256], F32R, name=f"{nm}_l")
            nc.scalar.copy(wh[:, :], stg[:, :])
            nc.vector.tensor_tensor(wl[:, :], stg[:, :], wh[:, :], op=SUB)
            wsp[nm] = (wh, wl)
        stg = work.tile([128, 1], F32, tag="ld", bufs=4, name="ldw_wv")
        nc.sync.dma_start(out=stg[:, :], in_=d_wv[:, :])
        wv_h = const.tile([128, 1], F32R, name="wv_h")
        wv_l = const.tile([128, 1], F32R, name="wv_l")
        nc.scalar.copy(wv_h[:, :], stg[:, :])
        nc.vector.tensor_tensor(wv_l[:, :], stg[:, :], wv_h[:, :], op=SUB)

        def load(dram, shape, nm):
            t = const.tile(shape, F32, name=nm)
            nc.sync.dma_start(out=t[:, :], in_=dram[:, :])
            return t

        xt_lit = load(d_xt_lit, [3, GPC * NL], "xt_lit_sb")
        xt_cl = load(d_xt_cl, [3, GPC * NC], "xt_cl_sb")
        li_w = load(d_liw, [3, H], "li_w_sb")
        ci_w = load(d_ciw, [3, H], "ci_w_sb")
        bias = load(d_bias, [128, 5], "bias_sb")
        dxr = const.tile([2, GPC * NL], F32R, name="dxr_sb")
        for g in range(GPC):
            stg = work.tile([2, NL], F32, tag="ld", bufs=4, name=f"ld_dxr{g}")
            nc.sync.dma_start(out=stg[:, :], in_=d_dxr[:, g * NL:(g + 1) * NL])
            nc.scalar.copy(dxr[:, g * NL:(g + 1) * NL], stg[:, :])
        stg = work.tile([2, 256], F32, tag="ld", bufs=4, name="ld_wdq")
        nc.sync.dma_start(out=stg[:, :], in_=d_wdq[:, :])
        wdq_h = const.tile([2, 256], F32R, name="wdq_h")
        wdq_l = const.tile([2, 256], F32R, name="wdq_l")
        nc.scalar.copy(wdq_h[:, :], stg[:, :])
        nc.vector.tensor_tensor(wdq_l[:, :], stg[:, :], wdq_h[:, :], op=SUB)

        def MM(*a, **kw):
            kw.setdefault("skip_group_check", True)
            return nc.tensor.matmul(*a, **kw)

        # ---- initial node states (bias via ones row of xt) ----
        Lh = state.tile([128, NL], F32R, tag="Lh", name="Lh0")
        Ll = state.tile([128, NL], F32R, tag="Ll", name="Ll0")
        lit_hf_a = state.tile([128, CHK], F32, tag="lit_hf_a", name="lit_hf_a0")
        lit_hf_b = state.tile([128, CHK], F32, tag="lit_hf_b", name="lit_hf_b0")
        for hf in range(2):
            p = psg.tile([128, CHK], F32, tag="g", name=f"ini_{hf}")
            prev = None
            for g in range(GPC):
                mm = MM(p[HALF[g], :], li_w[0:3, :],
                        xt_lit[0:3, g * NL + hf * CHK:g * NL + (hf + 1) * CHK],
                        start=True, stop=True, tile_position=(0, 64 * g))
                if prev is not None:
                    add_dep_helper(mm.ins, prev.ins, sync=True,
                                   reason="psum half order")
                prev = mm
            cs = slice(hf * CHK, (hf + 1) * CHK)
            nc.vector.tensor_copy((lit_hf_a if hf == 0 else lit_hf_b)[:, :],
                                  p[:, :])
            nc.scalar.copy(Lh[:, cs], p[:, :])
            nc.vector.tensor_tensor(Ll[:, cs], p[:, :], Lh[:, cs], op=SUB)
        Chh = state.tile([128, NC], F32R, tag="Chh", name="Chh0")
        Chl = state.tile([128, NC], F32R, tag="Chl", name="Chl0")
        cl_hf = state.tile([128, NC], F32, tag="cl_hf", name="cl_hf0")
        pc = psg.tile([128, NC], F32, tag="g", name="ini_c")
        prev = None
        for g in range(GPC):
            mm = MM(pc[HALF[g], :], ci_w[0:3, :], xt_cl[0:3, g * NC:(g + 1) * NC],
                    start=True, stop=True, tile_position=(0, 64 * g))
            if prev is not None:
                add_dep_helper(mm.ins, prev.ins, sync=True, reason="psum half order")
            prev = mm
        nc.vector.tensor_copy(cl_hf[:, :], pc[:, :])
        nc.scalar.copy(Chh[:, :], pc[:, :])
        nc.vector.tensor_tensor(Chl[:, :], pc[:, :], Chh[:, :], op=SUB)

        lit_c = None
        cl_c = None

        for t in range(1, ITERS):
            first = t == 1

            # ==== clause phase ====
            # dependency-free copies first: clause stack state-halves and the
            # lit-phase flip halves (keeps Pool busy off the critical path)
            st0h = work.tile([128, NC], F32R, tag="st0h", name=f"st0h_{t}")
            st0l = work.tile([128, NC], F32R, tag="st0l", name=f"st0l_{t}")
            st1h = work.tile([128, NC], F32R, tag="st1h", name=f"st1h_{t}")
            st1l = work.tile([128, NC], F32R, tag="st1l", name=f"st1l_{t}")
            nc.scalar.copy(st0h[HI, :], Chh[LO, :])
            nc.scalar.copy(st0l[HI, :], Chl[LO, :])
            nc.gpsimd.tensor_copy(st1h[HI, :], Chh[HI, :])
            nc.gpsimd.tensor_copy(st1l[HI, :], Chl[HI, :])
            lst = []
            for hf in range(2):
                fs = slice((1 - hf) * CHK, (2 - hf) * CHK)
                s0h = work.tile([128, CHK], F32R, tag="s0h", bufs=2, name=f"s0h_{t}_{hf}")
                s0l = work.tile([128, CHK], F32R, tag="s0l", bufs=2, name=f"s0l_{t}_{hf}")
                s1h = work.tile([128, CHK], F32R, tag="s1h", bufs=2, name=f"s1h_{t}_{hf}")
                s1l = work.tile([128, CHK], F32R, tag="s1l", bufs=2, name=f"s1l_{t}_{hf}")
                nc.scalar.copy(s0h[HI, :], Lh[LO, fs])
                nc.scalar.copy(s0l[HI, :], Ll[LO, fs])
                nc.gpsimd.tensor_copy(s1h[HI, :], Lh[HI, fs])
                nc.gpsimd.tensor_copy(s1l[HI, :], Ll[HI, fs])
                lst.append((s0h, s0l, s1h, s1l))

            # transpose lit state per half-grid (chunks 128,128,128,16 per
            # half) so the half-0 transposes + agg terms start while the
            # half-1 pointwise of the previous lit phase is still draining
            GRID = ((0, 128), (128, 128), (256, 128), (272, 128))
            rml = [[None, None], [None, None]]   # rml[g][half] = (rm_h, rm_l)
            for x, src in ((0, lit_hf_a), (1, lit_hf_b)):
                for g in range(GPC):
                    tp = pstp.tile([128, 4 * H], F32, tag="tp",
                                   name=f"tpl_{t}_{g}_{x}")
                    for kk, (c0, sz) in enumerate(GRID):
                        nc.tensor.transpose(
                            tp[:, kk * H:(kk + 1) * H],
                            src[HALF[g], c0:c0 + sz],
                            ident[HALF[g], HALF[g]],
                        )
                    rm_h = work.tile([128, 4 * H], F32R, tag=f"rmlh{g}{x}",
                                     name=f"rmlh_{t}_{g}_{x}")
                    rm_l = work.tile([128, 4 * H], F32R, tag=f"rmll{g}{x}",
                                     name=f"rmll_{t}_{g}_{x}")
                    nc.vector.tensor_copy(rm_h[:, :], tp[:, :])
                    nc.vector.tensor_tensor(rm_l[:, :], tp[:, :], rm_h[:, :],
                                            op=SUB)
                    rml[g][x] = (rm_h, rm_l)

            # clause agg A^T @ L: per-graph psum tiles (g0 rows HI, g1 LO) so
            # the hi/lo term groups interleave without bank conflicts
            agc0 = psag.tile([128, NC], F32, tag="ag", name=f"agc0_{t}")
            agc1 = psag.tile([128, NC], F32, tag="ag", name=f"agc1_{t}")
            agp = (agc0[LO, :], agc1[LO, :])
            for x in range(2):
                for term in range(2):
                    for g in range(GPC):
                        for kk in range(4):
                            c = NC * (8 * g + 4 * x + kk)
                            MM(agp[g],
                               rml[g][x][term][:, kk * H:(kk + 1) * H],
                               at_r[:, c:c + NC],
                               start=(x == 0 and term == 0 and kk == 0),
                               stop=(x == 1 and term == 1 and kk == 3),
                               tile_position=(0, 0))

            # stack agg halves (aligned at LO)
            nc.scalar.copy(st0h[LO, :], agc0[LO, :])
            nc.vector.tensor_tensor(st0l[LO, :], agc0[LO, :], st0h[LO, :], op=SUB)
            nc.scalar.copy(st1h[LO, :], agc1[LO, :])
            nc.vector.tensor_tensor(st1l[LO, :], agc1[LO, :], st1h[LO, :], op=SUB)

            # clause gates: per graph, 2 gate-pairs, 3-term split
            cg = [[None, None], [None, None]]
            for g in range(GPC):
                wnm = "wc_1" if first else "wc_b"
                wh, wl = wsp[wnm]
                sth, stl = (st0h, st0l) if g == 0 else (st1h, st1l)
                for p in range(2):
                    ps_ = slice(p * 128, (p + 1) * 128)
                    gp = psg.tile([128, NC], F32, tag="g", name=f"cg{g}{p}_{t}")
                    MM(gp[:, :], wh[:, ps_], sth[:, :], start=True, stop=False)
                    MM(gp[:, :], wl[:, ps_], sth[:, :], start=False, stop=False)
                    MM(gp[:, :], wh[:, ps_], stl[:, :], start=False, stop=True)
                    cg[g][p] = gp

            # repack sigmoids: pair-psum -> graph-packed s tiles
            s_i = work.tile([128, NC], F32, tag="si", name=f"csi_{t}")
            s_f = work.tile([128, NC], F32, tag="sf", name=f"csf_{t}")
            s_g = work.tile([128, NC], F32, tag="sg", name=f"csg_{t}")
            s_o = work.tile([128, NC], F32, tag="so", name=f"cso_{t}")
            nc.scalar.activation(s_i[LO, :], cg[0][0][LO, :], SIG,
                                 bias=bias[LO, 0:1])
            nc.scalar.activation(s_f[LO, :], cg[0][0][HI, :], SIG,
                                 bias=bias[LO, 1:2])
            nc.scalar.activation(s_g[LO, :], cg[0][1][LO, :], SIG,
                                 bias=bias[LO, 2:3])
            nc.scalar.activation(s_i[HI, :], cg[1][0][LO, :], SIG,
                                 bias=bias[HI, 0:1])
            nc.scalar.activation(s_f[HI, :], cg[1][0][HI, :], SIG,
                                 bias=bias[HI, 1:2])
            nc.scalar.activation(s_g[HI, :], cg[1][1][LO, :], SIG,
                                 bias=bias[HI, 2:3])
            nc.scalar.activation(s_o[LO, :], cg[0][1][HI, :], SIG,
                                 bias=bias[LO, 3:4])
            nc.scalar.activation(s_o[HI, :], cg[1][1][HI, :], SIG,
                                 bias=bias[HI, 3:4])

            # clause LSTM pointwise (graph-packed, full height)
            cc_new = state.tile([128, NC], F32, tag="cl_c", name=f"cc_{t}")
            # pointwise chain column-split across DVE (lo cols) and Pool (hi);
            # tanh realized as tg = 2*sigmoid-1 first so every link is a
            # splittable tensor_scalar/tensor_tensor (no DVE-only stt)
            CSP = ((nc.vector, slice(0, 220)), (nc.gpsimd, slice(220, NC)))
            tg = work.tile([128, NC], F32, tag="t1", name=f"ctg_{t}")
            for eng, q in CSP:
                eng.tensor_scalar(tg[:, q], s_g[:, q], 2.0, -1.0,
                                  op0=MULT, op1=ADD)
            if first:
                for eng, q in CSP:
                    eng.tensor_mul(cc_new[:, q], s_i[:, q], tg[:, q])
            else:
                u = work.tile([128, NC], F32, tag="u", name=f"cu_{t}")
                t2 = work.tile([128, NC], F32, tag="t2", name=f"ct2_{t}")
                for eng, q in CSP:
                    eng.tensor_mul(u[:, q], s_i[:, q], tg[:, q])
                for eng, q in CSP:
                    eng.tensor_mul(t2[:, q], s_f[:, q], cl_c[:, q])
                for eng, q in CSP:
                    eng.tensor_add(cc_new[:, q], u[:, q], t2[:, q])
            tnc = work.tile([128, NC], F32, tag="tnc", name=f"ctn_{t}")
            nc.scalar.activation(tnc[:, 0:220], cc_new[:, 0:220], SIG, scale=2.0)
            nc.scalar.activation(tnc[:, 220:NC], cc_new[:, 220:NC], SIG, scale=2.0)
            th = work.tile([128, NC], F32, tag="t3", name=f"cth_{t}")
            cl_hf_new = state.tile([128, NC], F32, tag="cl_hf", name=f"chf_{t}")
            for eng, q in CSP:
                eng.tensor_scalar(th[:, q], tnc[:, q], 2.0, -1.0,
                                  op0=MULT, op1=ADD)
            for eng, q in CSP:
                eng.tensor_mul(cl_hf_new[:, q], s_o[:, q], th[:, q])
            # split off the critical path (consumers are next iteration)
            Chh_new = state.tile([128, NC], F32R, tag="Chh", name=f"Chh_{t}")
            Chl_new = state.tile([128, NC], F32R, tag="Chl", name=f"Chl_{t}")
            nc.gpsimd.tensor_copy(Chh_new[:, :], cl_hf_new[:, :])
            nc.vector.tensor_tensor(Chl_new[:, :], cl_hf_new[:, :], Chh_new[:, :],
                                    op=SUB)

            # ==== lit phase ====
            # transpose full clause state; split rides the psum copy
            rmc = []
            for g in range(GPC):
                tp = pstp.tile([128, 4 * H], F32, tag="tp", name=f"tpc_{t}_{g}")
                for kk in range(4):
                    c0 = 128 * kk if kk < 3 else NC - 128
                    nc.tensor.transpose(
                        tp[:, kk * H:(kk + 1) * H],
                        cl_hf_new[HALF[g], c0:c0 + 128],
                        ident[HALF[g], HALF[g]],
                    )
                rm_h = work.tile([128, 4 * H], F32R, tag=f"rmch{g}",
                                 name=f"rmch_{t}_{g}")
                rm_l = work.tile([128, 4 * H], F32R, tag=f"rmcl{g}",
                                 name=f"rmcl_{t}_{g}")
                nc.scalar.copy(rm_h[:, :], tp[:, :])
                nc.vector.tensor_tensor(rm_l[:, :], tp[:, :], rm_h[:, :], op=SUB)
                rmc.append((rm_h, rm_l))

            Lh_new = state.tile([128, NL], F32R, tag="Lh", name=f"Lh_{t}")
            Ll_new = state.tile([128, NL], F32R, tag="Ll", name=f"Ll_{t}")
            lit_hf_na = state.tile([128, CHK], F32, tag="lit_hf_a", name=f"lhfa_{t}")
            lit_hf_nb = state.tile([128, CHK], F32, tag="lit_hf_b", name=f"lhfb_{t}")
            lc_new = state.tile([128, NL], F32, tag="lit_c", name=f"lc_{t}")
            wSnm = "w_cl2_dup" if first else "w_lh_dup"
            wSh, wSl = wsp[wSnm]
            for hf in range(2):
                cs = slice(hf * CHK, (hf + 1) * CHK)
                s0h, s0l, s1h, s1l = lst[hf]
                # literal agg A @ C: per-graph psums, interleaved term groups
                agl0 = psag.tile([128, CHK], F32, tag="ag", name=f"agl0_{t}_{hf}")
                agl1 = psag.tile([128, CHK], F32, tag="ag", name=f"agl1_{t}_{hf}")
                agp = (agl0[LO, :], agl1[LO, :])
                for term in range(2):
                    for g in range(GPC):
                        for kk in range(4):
                            MM(agp[g], rmc[g][term][:, kk * H:(kk + 1) * H],
                               a_r[:, NL * (4 * g + kk) + hf * CHK:
                                   NL * (4 * g + kk) + (hf + 1) * CHK],
                               start=(term == 0 and kk == 0),
                               stop=(term == 1 and kk == 3),
                               tile_position=(0, 0))

                # lit gates: dq + rec terms first (no stack dependency -> they
                # fill the PE pipeline while stacks build)
                gps = [[None, None], [None, None]]
                for g in range(GPC):
                    hg = HALF[g]
                    for p in range(2):
                        ps_ = slice(p * 128, (p + 1) * 128)
                        gp = psg.tile([128, CHK], F32, tag="g", name=f"lg{g}{p}_{t}_{hf}")
                        dc = slice(g * NL + hf * CHK, g * NL + (hf + 1) * CHK)
                        MM(gp[:, :], wdq_h[0:2, ps_], dxr[0:2, dc],
                           start=True, stop=False)
                        MM(gp[:, :], wdq_l[0:2, ps_], dxr[0:2, dc],
                           start=False, stop=False)
                        MM(gp[:, :], wSh[hg, ps_], Lh[hg, cs], start=False,
                           stop=False, tile_position=(64 * g, 0))
                        MM(gp[:, :], wSl[hg, ps_], Lh[hg, cs], start=False,
                           stop=False, tile_position=(64 * g, 0))
                        MM(gp[:, :], wSh[hg, ps_], Ll[hg, cs], start=False,
                           stop=False, tile_position=(64 * g, 0))
                        gps[g][p] = gp

                # stack agg halves (aligned at LO)
                nc.scalar.copy(s0h[LO, :], agl0[LO, :])
                nc.vector.tensor_tensor(s0l[LO, :], agl0[LO, :], s0h[LO, :], op=SUB)
                nc.scalar.copy(s1h[LO, :], agl1[LO, :])
                nc.vector.tensor_tensor(s1l[LO, :], agl1[LO, :], s1h[LO, :], op=SUB)

                # stack-dependent gate terms
                for g in range(GPC):
                    wnm = "wl_b"
                    wh, wl = wsp[wnm]
                    sth, stl = (s0h, s0l) if g == 0 else (s1h, s1l)
                    for p in range(2):
                        ps_ = slice(p * 128, (p + 1) * 128)
                        gp = gps[g][p]
                        MM(gp[:, :], wh[:, ps_], sth[:, :], start=False, stop=False)
                        MM(gp[:, :], wl[:, ps_], sth[:, :], start=False, stop=False)
                        MM(gp[:, :], wh[:, ps_], stl[:, :], start=False, stop=True)

                s_i = work.tile([128, CHK], F32, tag="lsi", bufs=2, name=f"lsi_{t}_{hf}")
                s_f = work.tile([128, CHK], F32, tag="lsf", bufs=2, name=f"lsf_{t}_{hf}")
                s_g = work.tile([128, CHK], F32, tag="lsg", bufs=2, name=f"lsg_{t}_{hf}")
                s_o = work.tile([128, CHK], F32, tag="lso", bufs=2, name=f"lso_{t}_{hf}")
                nc.scalar.activation(s_i[LO, :], gps[0][0][LO, :], SIG)
                nc.scalar.activation(s_f[LO, :], gps[0][0][HI, :], SIG)
                nc.scalar.activation(s_g[LO, :], gps[0][1][LO, :], SIG)
                nc.scalar.activation(s_i[HI, :], gps[1][0][LO, :], SIG)
                nc.scalar.activation(s_f[HI, :], gps[1][0][HI, :], SIG)
                nc.scalar.activation(s_g[HI, :], gps[1][1][LO, :], SIG)
                nc.scalar.activation(s_o[LO, :], gps[0][1][HI, :], SIG)
                nc.scalar.activation(s_o[HI, :], gps[1][1][HI, :], SIG)

                tg = work.tile([128, CHK], F32, tag="lt1", bufs=2, name=f"ltg_{t}_{hf}")
                LSP = ((nc.vector, slice(0, 200)), (nc.gpsimd, slice(200, CHK)))
                cq = [slice(cs.start, cs.start + 200),
                      slice(cs.start + 200, cs.stop)]
                for eng, q in LSP:
                    eng.tensor_scalar(tg[:, q], s_g[:, q], 2.0, -1.0,
                                      op0=MULT, op1=ADD)
                if first:
                    for (eng, q), c2 in zip(LSP, cq):
                        eng.tensor_mul(lc_new[:, c2], s_i[:, q], tg[:, q])
                else:
                    u = work.tile([128, CHK], F32, tag="lu", bufs=2, name=f"lu_{t}_{hf}")
                    t2 = work.tile([128, CHK], F32, tag="lt2", bufs=1, name=f"lt2_{t}_{hf}")
                    for eng, q in LSP:
                        eng.tensor_mul(u[:, q], s_i[:, q], tg[:, q])
                    for (eng, q), c2 in zip(LSP, cq):
                        eng.tensor_mul(t2[:, q], s_f[:, q], lit_c[:, c2])
                    for (eng, q), c2 in zip(LSP, cq):
                        eng.tensor_add(lc_new[:, c2], u[:, q], t2[:, q])
                tnc = work.tile([128, CHK], F32, tag="ltn", bufs=1, name=f"ltn_{t}_{hf}")
                nc.scalar.activation(tnc[:, 0:200],
                                     lc_new[:, cs.start:cs.start + 200],
                                     SIG, scale=2.0)
                nc.scalar.activation(tnc[:, 200:CHK],
                                     lc_new[:, cs.start + 200:cs.stop],
                                     SIG, scale=2.0)
                th = work.tile([128, CHK], F32, tag="lt3", bufs=1, name=f"lth_{t}_{hf}")
                lhf_x = lit_hf_na if hf == 0 else lit_hf_nb
                for eng, q in LSP:
                    eng.tensor_scalar(th[:, q], tnc[:, q], 2.0, -1.0,
                                      op0=MULT, op1=ADD)
                for eng, q in LSP:
                    eng.tensor_mul(lhf_x[:, q], s_o[:, q], th[:, q])
                # split off the critical path (consumers are next iteration)
                nc.gpsimd.tensor_copy(Lh_new[:, cs], lhf_x[:, :])
                nc.vector.tensor_tensor(Ll_new[:, cs], lhf_x[:, :],
                                        Lh_new[:, cs], op=SUB)

            Lh, Ll = Lh_new, Ll_new
            lit_hf_a, lit_hf_b = lit_hf_na, lit_hf_nb
            Chh, Chl, cl_hf = Chh_new, Chl_new, cl_hf_new
            lit_c, cl_c = lc_new, cc_new

        # ---- vote head: (Lh+Ll) @ Wv, 3-term split ----
        for g in range(GPC):
            hg = HALF[g]
            for hf in range(2):
                cs = slice(hf * CHK, (hf + 1) * CHK)
                p = psg.tile([1, CHK], F32, tag="g", name=f"vps_{g}_{hf}")
                MM(p[:, :], wv_h[hg, 0:1], Lh[hg, cs], start=True, stop=False,
                   tile_position=(64 * g, 0))
                MM(p[:, :], wv_l[hg, 0:1], Lh[hg, cs], start=False, stop=False,
                   tile_position=(64 * g, 0))
                MM(p[:, :], wv_h[hg, 0:1], Ll[hg, cs], start=False, stop=True,
                   tile_position=(64 * g, 0))
                vc = work.tile([1, CHK], F32, tag="vote", bufs=1,
                               name=f"vote_{g}_{hf}")
                nc.scalar.activation(
                    vc[:, :], p[:, :], mybir.ActivationFunctionType.Identity,
                    bias=bias[0:1, 4:5],
                )
                nc.sync.dma_start(
                    out=d_out[0:1, g * NL + hf * CHK:g * NL + (hf + 1) * CHK],
                    in_=vc[:, :])

    nc.compile()
    return nc


def _fold_and_shard(inputs):
    """Host-side preprocessing: fold weights, build adjacency, shard by graph."""
    f32 = np.float32
    g = {k: np.asarray(v) for k, v in inputs.items()}

    def collapse(w1, b1, w2, b2, w3, b3):
        return w1 @ w2 @ w3, ((b1 @ w2) + b2) @ w3 + b3

    Wl, bl = collapse(g["lm1_w"], g["lm1_b"], g["lm2_w"], g["lm2_b"],
                      g["lm3_w"], g["lm3_b"])
    Wc, bc = collapse(g["cm1_w"], g["cm1_b"], g["cm2_w"], g["cm2_b"],
                      g["cm3_w"], g["cm3_b"])
    Wv, bv = collapse(g["lv1_w"], g["lv1_b"], g["lv2_w"], g["lv2_b"],
                      g["lv3_w"], g["lv3_b"])

    cu_wih, lu_wih = g["cu_wih"], g["lu_wih"]
    w_lc = (Wl @ cu_wih).astype(f32)                 # agg_c -> clause gates
    w_ch = (w_lc + g["cu_whh"]).astype(f32)          # t>=2 merged recurrent
    cbias_c = ((K + 1) * (bl @ cu_wih) + g["cu_bih"] + g["cu_bhh"]).astype(f32)
    wih_a = lu_wih[0:H].astype(f32)                  # flip -> lit gates
    w_cl2 = (Wc @ lu_wih[H:2 * H]).astype(f32)       # agg_l -> lit gates
    w_lh = (w_cl2 + g["lu_whh"]).astype(f32)         # t>=2 merged recurrent
    q_l = (bc @ lu_wih[H:2 * H]).astype(f32)         # [256]
    cbias_l = (g["lu_bih"] + g["lu_bhh"]).astype(f32)

    def gdouble(w):
        w = w.copy()
        w[:, 2 * H:3 * H] *= 2.0     # g-gate runs as sigmoid(2x+2b)
        return w

    vs = np.vstack
    wc_a = gdouble(vs([w_ch, w_lc]))
    wc_b = gdouble(vs([w_lc, w_ch]))
    wc_1 = gdouble(vs([w_lc, w_lc]))
    wl_a = gdouble(vs([wih_a, w_cl2]))
    wl_b = gdouble(vs([w_cl2, wih_a]))
    w_lh_dup = gdouble(vs([w_lh, w_lh]))
    w_cl2_dup = gdouble(vs([w_cl2, w_cl2]))
    wv_dup = vs([Wv.astype(f32), Wv.astype(f32)])

    bias_q = np.zeros((128, 5), f32)
    for x in range(4):
        scl = 2.0 if x == 2 else 1.0
        bias_q[0:64, x] = scl * cbias_c[x * H:(x + 1) * H]
        bias_q[64:128, x] = scl * cbias_c[x * H:(x + 1) * H]
    bias_q[0, 4] = bv[0]

    li_w3 = np.concatenate([g["li_w"], g["li_b"][None, :]], axis=0).astype(f32)
    ci_w3 = np.concatenate([g["ci_w"], g["ci_b"][None, :]], axis=0).astype(f32)

    # adjacency per graph from edge_index (direction-robust)
    ei = g["edge_index"].astype(np.int64)
    src, dst = ei[0], ei[1]
    src_g, dst_g = src // NPG, dst // NPG
    assert np.all(src_g == dst_g), "edges must be graph-local"
    src_l, dst_l = src % NPG, dst % NPG
    s_lit, d_lit = src_l < NL, dst_l < NL
    A_in_c = np.zeros((B, NC, NL), f32)   # clause <- literal edges
    m = (~d_lit) & s_lit
    np.add.at(A_in_c, (dst_g[m], dst_l[m] - NL, src_l[m]), 1.0)
    A_in_l = np.zeros((B, NL, NC), f32)   # literal <- clause edges
    m = d_lit & (~s_lit)
    np.add.at(A_in_l, (dst_g[m], dst_l[m], src_l[m] - NL), 1.0)
    deg_l = A_in_l.sum(axis=2)            # [B, NL]

    x = g["x"].astype(f32).reshape(B, NPG, 2)
    ones = np.ones((B, NPG, 1), f32)
    x3 = np.concatenate([x, ones], axis=2)        # [B, NPG, 3]

    shared = dict(
        wc_a=wc_a, wc_b=wc_b, wc_1=wc_1, wl_a=wl_a, wl_b=wl_b,
        w_lh_dup=w_lh_dup, w_cl2_dup=w_cl2_dup, wv_dup=wv_dup,
        li_w3=li_w3, ci_w3=ci_w3, bias_q=bias_q,
    )
    in_maps = []
    for c in range(NCORES):
        gs = slice(c * GPC, (c + 1) * GPC)
        x3c = x3[gs]                               # [GPC, NPG, 3]
        xt_lit = np.ascontiguousarray(
            x3c[:, :NL].transpose(2, 0, 1).reshape(3, GPC * NL))
        xt_cl = np.ascontiguousarray(
            x3c[:, NL:].transpose(2, 0, 1).reshape(3, GPC * NC))
        # dxr rows: (deg+1) per literal, ones; wdq rows: q, cbias_l
        # (g-gate block doubled to match the pre-doubled weights)
        dxr = np.ones((2, GPC * NL), f32)
        for gg in range(GPC):
            dxr[0, gg * NL:(gg + 1) * NL] = deg_l[c * GPC + gg] + 1.0
        wdq = np.stack([q_l, cbias_l]).astype(f32)
        wdq[:, 2 * H:3 * H] *= 2.0
        # pre-chunk adjacency into full-128-row K-chunks; the final chunk
        # overlaps the previous one with its overlap rows zeroed
        atc = np.zeros((GPC, 8, 128, NC), f32)
        ac = np.zeros((GPC, 4, 128, NL), f32)
        for gg in range(GPC):
            at_full = A_in_c[c * GPC + gg].T       # [NL, NC]
            a_full = A_in_l[c * GPC + gg].T        # [NC, NL]
            for x in range(2):
                for j in range(3):
                    atc[gg, 4 * x + j] = at_full[x * 400 + 128 * j:
                                                 x * 400 + 128 * (j + 1)]
                atc[gg, 4 * x + 3, 112:128] = at_full[x * 400 + 384:
                                                      x * 400 + 400]
            for kk in range(3):
                ac[gg, kk] = a_full[128 * kk:128 * (kk + 1)]
            ac[gg, 3, 128 - (NC - 384):] = a_full[384:]
        in_maps.append(dict(
            xt_lit=xt_lit, xt_cl=xt_cl, at_rm=atc, a_rm=ac,
            dxr=dxr, wdq=wdq, **shared,
        ))
    return in_maps


_LAST_RESULTS = {}


def kernel(**inputs):
    from concourse.bass_utils import run_bass_kernel_spmd

    in_maps = _fold_and_shard(inputs)
    if "nc" not in _PROGRAM_CACHE:
        _PROGRAM_CACHE["nc"] = _build_program()
    nc = _PROGRAM_CACHE["nc"]
    res = run_bass_kernel_spmd(nc, in_maps, core_ids=list(range(NCORES)))
    _LAST_RESULTS["res"] = res
    out = np.zeros((N, 1), np.float32)
    for c in range(NCORES):
        vote = res.results[c]["vote"].reshape(GPC, NL)
        for g in range(GPC):
            base = (c * GPC + g) * NPG
            out[base:base + NL, 0] = vote[g]
    return out
